# revision 1
# baseline (speedup 1.0000x reference)
"""Trainium2 Bass kernel for the AgentLayer GRU-with-action-memory model, v4.

B=512 -> 8 cores x 64; two 32-batch streams per core, op-level zippered.
v4 chain optimizations over v2:
  - obs-mean window sum kept directly in a PSUM accumulator (uacc): the
    per-step W1h@h matmul accumulates in place; the window subtract is a
    negated-weight matmul reading an SBUF h-ring (off the critical chain).
    Removes two DVE hops per step.
  - qmask built per slot-row (3 small DVE ops) so fold matmuls start early.
  - weighted_h PSUM->SBUF copy on DVE (lower latency than Act).
  - thr/thz/pren live in PSUM (Act psum->psum is faster than ->SBUF).
  - final GRU combine on Pool for stream 0, DVE for stream 1 (balance).
All matmuls fp32 (fp32r measured 1.4e-4 rel err on HW -> argmax flips).
"""

import numpy as np
from contextlib import ExitStack

B, T, D, H, S, A, U = 512, 256, 256, 128, 64, 10, 64
NCORES = 8
BCORE = B // NCORES   # 64 per core
NV = 2                # streams per core
BC = BCORE // NV      # 32 per stream
NG = 128 // BC        # partition groups = 4
NS = 12               # ring slots
NR = NS // NG         # ring rows = 3
G3 = 3 * H
BIG = 1024.0
AHEAD = 2

_BUILD_CACHE = {}


def _build(Tn):
    key = Tn
    if key in _BUILD_CACHE:
        return _BUILD_CACHE[key]

    import concourse.bass as bass
    import concourse.bacc as bacc
    import concourse.tile as tile
    from concourse import mybir

    f32 = mybir.dt.float32
    Alu = mybir.AluOpType
    Act = mybir.ActivationFunctionType
    Axis = mybir.AxisListType

    nc = bacc.Bacc("TRN2", target_bir_lowering=False, debug=False)

    d_in = {}

    def din(name, shape):
        d_in[name] = nc.dram_tensor(name, list(shape), f32, kind="ExternalInput").ap()
        return d_in[name]

    xT = din("xT", (2, 128, Tn // 8, NV, 8 * BC))
    wihT = din("wihT", (2, 128, G3))
    whhT = din("whhT", (H, G3))
    w1h10T = din("w1h10T", (H, U))
    w2b1 = din("w2b1", (U + 1, A))
    a2w1xT = din("a2w1xT", (2, 128, U))
    a2w1sT = din("a2w1sT", (S, U))
    w2b2 = din("w2b2", (U + 1, A))
    fuswhT = din("fuswhT", (H, H))
    fus_staticT = din("fus_staticT", (NV, BC, H))
    staticrep = din("staticrep", (NV, S, 8 * BC))
    cur0T = din("cur0T", (H, BCORE))
    ubase = din("ubase", (U, BCORE))           # W1s@static
    a1b1 = din("a1b1", (U, 1))
    a2b1 = din("a2b1", (U, 1))
    halfbr = din("halfbr", (H, 1))
    halfbz = din("halfbz", (H, 1))
    biasn = din("biasn", (H, 1))
    iotamb10 = din("iotamb10", (128, A))
    iotamb80 = din("iotamb80", (128, 8 * A))
    iotamb24 = din("iotamb24", (128, 24, NR))
    delta24 = din("delta24", (128, 24, NR))
    diagpat = din("diagpat", (128, BC))
    dup32 = din("dup32", (BC, 128))
    i32rep4 = din("i32rep4", (BC, 4 * BC))
    ident = din("ident", (128, 128))

    out_d = nc.dram_tensor("out", [H, Tn, BCORE], f32, kind="ExternalOutput").ap()

    NBLK = Tn // 8
    assert Tn % 16 == 0

    with ExitStack() as ctx:
        tc = ctx.enter_context(tile.TileContext(nc))
        singles = ctx.enter_context(tc.tile_pool(name="singles", bufs=1))
        work = ctx.enter_context(tc.tile_pool(name="work", bufs=3))
        pstate = ctx.enter_context(tc.tile_pool(name="pstate", bufs=1, space="PSUM"))
        pwork = ctx.enter_context(tc.tile_pool(name="pwork", bufs=2, space="PSUM"))
        pA = ctx.enter_context(tc.tile_pool(name="pA", bufs=1, space="PSUM"))
        outsb = ctx.enter_context(tc.tile_pool(name="outsb", bufs=2))

        sb = {}
        for name, ap in d_in.items():
            if name == "xT":
                continue
            if name in ("wihT", "a2w1xT"):
                t = singles.tile([128, 2, ap.shape[2]], f32, tag=f"w_{name}",
                                 name=f"w_{name}")
                for c in range(2):
                    nc.sync.dma_start(out=t[:, c, :], in_=ap[c])
            elif name in ("fus_staticT", "staticrep"):
                t = singles.tile([ap.shape[1], NV, ap.shape[2]], f32,
                                 tag=f"w_{name}", name=f"w_{name}")
                for v in range(NV):
                    nc.sync.dma_start(out=t[:, v, :], in_=ap[v])
            else:
                t = singles.tile(list(ap.shape), f32, tag=f"w_{name}",
                                 name=f"w_{name}")
                nc.sync.dma_start(out=t, in_=ap)
            sb[name] = t

        xsb = singles.tile([128, 2, Tn // 8, NV, 8 * BC], f32, tag="xsb")
        for c in range(2):
            nc.sync.dma_start(out=xsb[:, c], in_=xT[c])

        # ---- per-stream persistent state ----
        ring_lo = [singles.tile([64, NR, H], f32, tag=f"ringl{v}", name=f"ringl{v}")
                   for v in range(NV)]
        ring_hi = [singles.tile([64, NR, H], f32, tag=f"ringh{v}", name=f"ringh{v}")
                   for v in range(NV)]
        cur_h = [singles.tile([H, BC], f32, tag=f"cur{v}", name=f"cur{v}")
                 for v in range(NV)]
        u1t65 = [singles.tile([U + 1, 2 * BC], f32, tag=f"u1t{v}", name=f"u1t{v}")
                 for v in range(NV)]
        u2t65 = [singles.tile([U + 1, 8 * BC], f32, tag=f"u2t{v}", name=f"u2t{v}")
                 for v in range(NV)]

        # PSUM banks (8 total):
        #  bank_cp[v] (x2): uacc [0:U, 0:BC], pout4 [:, 384:512]
        #  bankA[v] (x2): pu2 [0:U, 0:256], pl2 [32:64, 256:336],
        #                 ptr [0:32, 256:384], pwh [:, 384:416],
        #                 pl1 [:, 416:426], pdup [:, 426:434]
        #  gates[v] (x2 bufs x2): pr/pz/pin/phn + thr/thz/pren
        # c-ring: one PSUM bank per stream; each slot written by its own
        # start+stop matmul group (overwrite), read by DVE window-sum updates
        bankC = [pstate.tile([U, 512], f32, tag=f"bankC{v}", name=f"bankC{v}")
                 for v in range(NV)]

        def pcring_slot(v, s):
            return bankC[v][0:U, s * BC:(s + 1) * BC]

        usum = [singles.tile([U, BC], f32, tag=f"usum{v}", name=f"usum{v}")
                for v in range(NV)]
        # bankS[v]: strictly sequential groups: phase-A pu2/pl2/pdup, then
        # per-step ptr -> pl1 -> fold group (emission order = group order)
        bankS = [pA.tile([128, 512], f32, tag=f"bankS{v}", name=f"bankS{v}")
                 for v in range(NV)]

        pre2_tiles = [[singles.tile([128, 8, NR], f32, tag=f"pre2_{v}_{b}",
                                    name=f"pre2_{v}_{b}")
                       for b in range(NBLK)] for v in range(NV)]
        for v in range(NV):
            nc.vector.memset(ring_lo[v], 0.0)
            nc.vector.memset(ring_hi[v], 0.0)
            nc.vector.memset(u1t65[v], 1.0)
            nc.vector.memset(u2t65[v], 1.0)
            nc.sync.dma_start(out=cur_h[v], in_=cur0T[:, v * BC:(v + 1) * BC])
            nc.sync.dma_start(out=usum[v], in_=ubase[:, v * BC:(v + 1) * BC])

        out_tiles = [{} for _ in range(NV)]

        def bsl(v):
            return slice(v * BC, (v + 1) * BC)

        # ---------- phase A ----------
        def gen_phaseA(v, blk):
            t0 = blk * 8
            pu2 = bankS[v][0:U, 0:8 * BC]
            for c in range(2):
                nc.tensor.matmul(pu2, sb["a2w1xT"][:, c, :],
                                 xsb[:, c, blk, v, :],
                                 start=(c == 0), stop=False)
                yield
            nc.tensor.matmul(pu2, sb["a2w1sT"], sb["staticrep"][:, v, :],
                             start=False, stop=True)
            yield
            nc.scalar.activation(u2t65[v][0:U, :], pu2, Act.Tanh,
                                 bias=sb["a2b1"], scale=1.0)
            yield
            pl2 = bankS[v][0:BC, 256:336].rearrange("p (f a) -> p f a", a=A)
            for j in range(8):
                nc.tensor.matmul(pl2[:, j, :],
                                 u2t65[v][:, j * BC:(j + 1) * BC], sb["w2b2"],
                                 start=(j == 0), stop=(j == 7))
                if j % 2 == 1:
                    yield
            rmax2 = work.tile([BC, 8], f32, tag=f"rmax2{v}", name=f"rmax2{v}_{blk}")
            nc.vector.tensor_reduce(out=rmax2, in_=pl2, axis=Axis.X, op=Alu.max)
            yield
            rmax2_b = bass.AP(tensor=rmax2.tensor, offset=rmax2.offset,
                              ap=[rmax2.ap[0], rmax2.ap[1], [0, A]])
            ge2 = work.tile([BC, 8, A], f32, tag=f"ge2{v}", name=f"ge2{v}_{blk}")
            nc.vector.tensor_tensor(out=ge2, in0=pl2, in1=rmax2_b, op=Alu.is_ge)
            yield
            iota_b = sb["iotamb80"][0:BC, :].rearrange("p (f a) -> p f a", a=A)
            mi2 = work.tile([BC, 8, A], f32, tag=f"mi2{v}", name=f"mi2{v}_{blk}")
            nc.vector.tensor_tensor(out=mi2, in0=ge2, in1=iota_b, op=Alu.mult)
            yield
            idx2f = work.tile([BC, 8], f32, tag=f"idx2f{v}", name=f"idx2f{v}_{blk}")
            nc.vector.tensor_reduce(out=idx2f, in_=mi2, axis=Axis.X, op=Alu.min)
            yield
            pdup = bankS[v][:, 336:344]
            nc.tensor.matmul(pdup, sb["dup32"], idx2f, start=True, stop=True)
            yield
            pdup_b = bass.AP(tensor=pdup.tensor, offset=pdup.offset,
                             ap=[pdup.ap[0], pdup.ap[1], [0, NR]])
            tm = t0 % 24
            oh2 = work.tile([128, 8, NR], f32, tag=f"oh2{v}", name=f"oh2{v}_{blk}")
            nc.vector.tensor_tensor(out=oh2, in0=sb["iotamb24"][:, tm:tm + 8, :],
                                    in1=pdup_b, op=Alu.is_equal)
            yield
            nc.vector.tensor_tensor(out=pre2_tiles[v][blk], in0=oh2,
                                    in1=sb["delta24"][:, tm:tm + 8, :], op=Alu.add)
            yield

        # gates bank layout: pr/pz/pin/phn [0:4*BC], thr [4BC:5BC],
        # thz [5BC:6BC], pren [6BC:7BC]
        def mk_gates(v, t):
            return pwork.tile([128, 512], f32, tag=f"gat{v}", name=f"gat{v}_{t}")

        def gen_gru(v, t, wh_sb, gat):
            pr = gat[:, 0:BC]
            pz = gat[:, BC:2 * BC]
            pin = gat[:, 2 * BC:3 * BC]
            phn = gat[:, 3 * BC:4 * BC]
            pren = gat[:, 6 * BC:7 * BC]
            thr = work.tile([H, BC], f32, tag=f"thr{v}", name=f"thr{v}_{t}")
            thz = work.tile([H, BC], f32, tag=f"thz{v}", name=f"thz{v}_{t}")
            xx = xsb[:, :, t // 8, v, (t % 8) * BC:(t % 8) * BC + BC]
            for c in range(2):
                nc.tensor.matmul(pr, sb["wihT"][:, c, 0:H], xx[:, c, :],
                                 start=(c == 0 and t % 1 == 0) if c == 0 else False,
                                 stop=False)
                nc.tensor.matmul(pz, sb["wihT"][:, c, H:2 * H], xx[:, c, :],
                                 start=False, stop=False)
                yield
                nc.tensor.matmul(pin, sb["wihT"][:, c, 2 * H:3 * H], xx[:, c, :],
                                 start=False, stop=False)
                yield
            nc.tensor.matmul(phn, sb["whhT"][:, 2 * H:3 * H], wh_sb,
                             start=False, stop=False)
            yield
            nc.tensor.matmul(pr, sb["whhT"][:, 0:H], wh_sb, start=False, stop=False)
            yield
            nc.tensor.matmul(pz, sb["whhT"][:, H:2 * H], wh_sb, start=False,
                             stop=True)
            yield
            nc.scalar.activation(thr, pr, Act.Tanh, bias=sb["halfbr"], scale=0.5)
            yield
            q = work.tile([H, BC], f32, tag=f"q{v}", name=f"q{v}_{t}")
            nc.vector.scalar_tensor_tensor(out=q, in0=thr, scalar=1.0, in1=phn,
                                           op0=Alu.add, op1=Alu.mult)
            yield
            nc.vector.scalar_tensor_tensor(out=pren, in0=q, scalar=0.5, in1=pin,
                                           op0=Alu.mult, op1=Alu.add)
            yield
            thn = work.tile([H, BC], f32, tag=f"thn{v}", name=f"thn{v}_{t}")
            nc.scalar.activation(thn, pren, Act.Tanh, bias=sb["biasn"], scale=1.0)
            yield
            nc.scalar.activation(thz, pz, Act.Tanh, bias=sb["halfbz"], scale=0.5)
            yield
            eng = nc.vector
            dgf = work.tile([H, BC], f32, tag=f"dgf{v}", name=f"dgf{v}_{t}")
            nc.vector.tensor_tensor(out=dgf, in0=wh_sb, in1=thn, op=Alu.subtract)
            yield
            e = work.tile([H, BC], f32, tag=f"e{v}", name=f"e{v}_{t}")
            eng.scalar_tensor_tensor(out=e, in0=thz, scalar=1.0, in1=dgf,
                                     op0=Alu.add, op1=Alu.mult)
            yield
            eng.scalar_tensor_tensor(out=cur_h[v], in0=e, scalar=0.5, in1=thn,
                                     op0=Alu.mult, op1=Alu.add)
            yield
            # fusion output: own sequential group in this step's gates bank
            pout = gat[:, 7 * BC:8 * BC]
            nc.tensor.matmul(pout, sb["fus_staticT"][:, v, :],
                             sb["i32rep4"][:, 0:BC], start=True, stop=False)
            yield
            nc.tensor.matmul(pout, sb["fuswhT"], cur_h[v], start=False, stop=True)
            yield
            ob16 = t % 16
            if ob16 == 0:
                out_tiles[v][t // 16] = outsb.tile(
                    [H, 16, BC], f32, tag=f"osb{v}", name=f"osb{v}_{t // 16}")
            ot = out_tiles[v][t // 16]
            nc.scalar.copy(ot[:, ob16, :], pout)
            yield
            if ob16 == 15 or t == Tn - 1:
                nc.sync.dma_start(
                    out=out_d[:, t - ob16:t + 1, bsl(v)],
                    in_=ot[:, 0:ob16 + 1, :])
                del out_tiles[v][t // 16]

        def gen_step(v, t):
            if t % 8 == 0 and (t // 8 + AHEAD - 1) < NBLK and not NOPHASEA:
                yield from gen_phaseA(v, t // 8 + AHEAD - 1)
            s_new = (t - 1) % NS
            g0, r0 = s_new % NG, s_new // NG
            tm = t % 24
            gat = mk_gates(v, t)

            # ring writes of h_{t-1} (off critical chain)
            if not NORING:
                ptr = bankS[v][0:BC, 128:256]
                nc.tensor.matmul(ptr, cur_h[v], sb["ident"],
                                 is_transpose=True, start=True, stop=True)
                yield
                rhalf = ring_lo[v] if g0 < 2 else ring_hi[v]
                gb = (g0 % 2) * BC
                nc.vector.tensor_copy(rhalf[gb:gb + BC, r0, :], ptr)
                yield
            # c_k = W1h@h_{t-1} into its own PSUM ring slot (own group)
            nc.tensor.matmul(pcring_slot(v, s_new), sb["w1h10T"], cur_h[v],
                             start=True, stop=True)
            yield
            nc.vector.tensor_tensor(out=usum[v], in0=usum[v],
                                    in1=pcring_slot(v, s_new), op=Alu.add)
            yield
            # MLP1 + packed argmax
            u1out = u1t65[v][0:U, :].rearrange("p (d b) -> p d b", b=BC)
            usum_b = bass.AP(tensor=usum[v].tensor, offset=usum[v].offset,
                             ap=[usum[v].ap[0], [0, 2], [1, BC]])
            nc.scalar.activation(u1out, usum_b, Act.Tanh,
                                 bias=sb["a1b1"], scale=1.0)
            yield
            if NOGATHER:
                yield from gen_gru(v, t, cur_h[v], gat)
                return
            for half in range(2):
                nc.tensor.matmul(bankS[v][half * 64:half * 64 + 64, 376:386],
                                 u1t65[v], sb["w2b1"],
                                 start=True, stop=True)
                yield
            pl1 = bankS[v][:, 376:386]
            rmax = work.tile([128, 1], f32, tag=f"rmax{v}", name=f"rmax{v}_{t}")
            nc.vector.tensor_reduce(out=rmax, in_=pl1, axis=Axis.X, op=Alu.max)
            yield
            mi = work.tile([128, A], f32, tag=f"mi{v}", name=f"mi{v}_{t}")
            nc.vector.scalar_tensor_tensor(out=mi, in0=pl1, scalar=rmax[:, 0:1],
                                           in1=sb["iotamb10"],
                                           op0=Alu.is_ge, op1=Alu.mult)
            yield
            idxf = work.tile([128, 1], f32, tag=f"idxf{v}", name=f"idxf{v}_{t}")
            nc.vector.tensor_reduce(out=idxf, in_=mi, axis=Axis.X, op=Alu.min)
            yield
            cm_l = work.tile([64, NR], f32, tag=f"cml{v}", name=f"cml{v}_{t}")
            cm_h = work.tile([64, NR], f32, tag=f"cmh{v}", name=f"cmh{v}_{t}")
            nc.vector.scalar_tensor_tensor(
                out=cm_l, in0=sb["iotamb24"][0:64, tm, :], scalar=idxf[0:64, 0:1],
                in1=pre2_tiles[v][t // 8][0:64, t % 8, :],
                op0=Alu.is_equal, op1=Alu.add)
            yield
            nc.vector.scalar_tensor_tensor(
                out=cm_h, in0=sb["iotamb24"][64:128, tm, :],
                scalar=idxf[64:128, 0:1],
                in1=pre2_tiles[v][t // 8][64:128, t % 8, :],
                op0=Alu.is_equal, op1=Alu.add)
            yield
            # qmask per row as lo/hi base-0 tiles; folds use only base-0 APs
            qm_lo = work.tile([64, NR, BC], f32, tag=f"qmlo{v}", name=f"qmlo{v}_{t}")
            qm_hi = work.tile([64, NR, BC], f32, tag=f"qmhi{v}", name=f"qmhi{v}_{t}")
            pwh = bankS[v][:, 344:376]
            # one DVE op per half: diag broadcast over rows (stride-0 middle),
            # cmask broadcast over columns (stride-0 inner)
            diag3 = bass.AP(tensor=sb["diagpat"].tensor,
                            offset=sb["diagpat"].offset,
                            ap=[[sb["diagpat"].ap[0][0], 64], [0, NR], [1, BC]])
            cm_lo3 = bass.AP(tensor=cm_l.tensor, offset=cm_l.offset,
                             ap=[cm_l.ap[0], [1, NR], [0, BC]])
            cm_hi3 = bass.AP(tensor=cm_h.tensor, offset=cm_h.offset,
                             ap=[cm_h.ap[0], [1, NR], [0, BC]])
            nc.vector.tensor_tensor(out=qm_lo, in0=diag3, in1=cm_lo3, op=Alu.mult)
            yield
            nc.vector.tensor_tensor(out=qm_hi, in0=diag3, in1=cm_hi3, op=Alu.mult)
            yield
            for r in range(NR):
                nc.tensor.matmul(pwh, ring_lo[v][:, r, :], qm_lo[:, r, :],
                                 start=(r == 0), stop=False)
                nc.tensor.matmul(pwh, ring_hi[v][:, r, :], qm_hi[:, r, :],
                                 start=False, stop=(r == NR - 1))
                yield
            whs = work.tile([H, BC], f32, tag=f"whs{v}", name=f"whs{v}_{t}")
            nc.vector.tensor_copy(whs, pwh)
            yield
            if t >= 10:
                nc.vector.tensor_tensor(
                    out=usum[v], in0=usum[v],
                    in1=pcring_slot(v, (t - 10) % NS), op=Alu.subtract)
                yield
            yield from gen_gru(v, t, whs, gat)

        import os as _os
        TMAX = int(_os.environ.get("K4_TMAX", "0")) or Tn
        NOGATHER = bool(int(_os.environ.get("K4_NOGATHER", "0")))
        NOPHASEA = bool(int(_os.environ.get("K4_NOPHASEA", "0")))
        NORING = bool(int(_os.environ.get("K4_NORING", "0")))

        def gen_stream(v):
            for blk in range(AHEAD):
                yield from gen_phaseA(v, blk)
            yield from gen_gru(v, 0, cur_h[v], mk_gates(v, 0))
            for t in range(1, TMAX):
                yield from gen_step(v, t)

        gens = [gen_stream(v) for v in range(NV)]
        live = list(gens)
        while live:
            nxt = []
            for g in live:
                try:
                    next(g)
                    nxt.append(g)
                except StopIteration:
                    pass
            live = nxt

    nc.compile()
    _BUILD_CACHE[key] = (nc, "out")
    return _BUILD_CACHE[key]


def _prep_core_inputs(inputs, core, Tn=T):
    f = np.float32
    b0 = core * BCORE
    x = np.ascontiguousarray(inputs["x"][b0:b0 + BCORE, :Tn, :]).astype(f)
    xT = (x.transpose(2, 1, 0).reshape(2, 128, Tn // 8, 8, NV, BC)
          .transpose(0, 1, 2, 4, 3, 5).reshape(2, 128, Tn // 8, NV, 8 * BC))
    xT = np.ascontiguousarray(xT)
    static = inputs["static"][b0:b0 + BCORE].astype(f)
    wih = inputs["gru_wih"].astype(f); whh = inputs["gru_whh"].astype(f)
    a1w1 = inputs["a1_w1"].astype(f); a2w1 = inputs["a2_w1"].astype(f)
    bih = inputs["gru_bih"].astype(f); bhh = inputs["gru_bhh"].astype(f)
    fusw = inputs["fus_w"].astype(f); fusb = inputs["fus_b"].astype(f)

    iotamb24 = np.zeros((128, 24, NR), f)
    delta24 = np.zeros((128, 24, NR), f)
    for p in range(128):
        g = p // BC
        for j in range(24):
            for r in range(NR):
                s = r * NG + g
                a = (s - j + 10) % NS
                if a < A:
                    iotamb24[p, j, r] = a - BIG
            s_new = (j - 1) % NS
            if s_new % NG == g:
                delta24[p, j, s_new // NG] = 2.0
    diagpat = np.zeros((128, BC), f)
    for p in range(128):
        diagpat[p, p % BC] = 0.25
    dup32 = np.zeros((BC, 128), f)
    for b in range(BC):
        for g in range(NG):
            dup32[b, g * BC + b] = 1.0
    i32rep4 = np.tile(np.eye(BC, dtype=f), (1, 4))

    fus_staticT = np.stack([
        (static[v * BC:(v + 1) * BC] @ fusw[:, H:].T + fusb) for v in range(NV)
    ])
    staticrep = np.stack([
        np.tile(static[v * BC:(v + 1) * BC].T, (1, 8)) for v in range(NV)
    ])
    cur0 = static @ inputs["init_w"].astype(f).T + inputs["init_b"].astype(f)
    ubase = (static @ a1w1[:, H:].T).T                        # [U, 64]

    w1h10 = (a1w1[:, :H] / 10.0).T
    m = {
        "xT": xT,
        "wihT": np.ascontiguousarray(wih.T.reshape(2, 128, G3)),
        "whhT": np.ascontiguousarray(whh.T),
        "w1h10T": np.ascontiguousarray(w1h10),
        "w2b1": np.vstack([inputs["a1_w2"].astype(f).T,
                           inputs["a1_b2"].astype(f).reshape(1, A)]),
        "a2w1xT": np.ascontiguousarray(a2w1[:, :D].T.reshape(2, 128, U)),
        "a2w1sT": np.ascontiguousarray(a2w1[:, D:].T),
        "w2b2": np.vstack([inputs["a2_w2"].astype(f).T,
                           inputs["a2_b2"].astype(f).reshape(1, A)]),
        "fuswhT": np.ascontiguousarray(fusw[:, :H].T),
        "fus_staticT": fus_staticT,
        "staticrep": staticrep,
        "cur0T": np.ascontiguousarray(cur0.T),
        "ubase": ubase,
        "a1b1": inputs["a1_b1"].astype(f).reshape(U, 1),
        "a2b1": inputs["a2_b1"].astype(f).reshape(U, 1),
        "halfbr": (0.5 * (bih[:H] + bhh[:H])).reshape(H, 1),
        "halfbz": (0.5 * (bih[H:2 * H] + bhh[H:2 * H])).reshape(H, 1),
        "biasn": (bih[2 * H:] + bhh[2 * H:]).reshape(H, 1),
        "iotamb10": np.tile(np.arange(A, dtype=f) - BIG, (128, 1)),
        "iotamb80": np.tile(np.arange(A, dtype=f) - BIG, (128, 8)),
        "iotamb24": iotamb24,
        "delta24": delta24,
        "diagpat": diagpat,
        "dup32": dup32,
        "i32rep4": i32rep4,
        "ident": np.eye(128, dtype=f),
    }
    return {k: np.ascontiguousarray(v, dtype=f) for k, v in m.items()}


def kernel(**inputs):
    from concourse.bass_utils import run_bass_kernel_spmd
    nc, _ = _build(T)
    in_maps = [_prep_core_inputs(inputs, c) for c in range(NCORES)]
    res = run_bass_kernel_spmd(nc, in_maps, core_ids=list(range(NCORES)))
    out = np.empty((B, T, H), np.float32)
    for c in range(NCORES):
        oc = res.results[c]["out"]
        out[c * BCORE:(c + 1) * BCORE] = oc.transpose(2, 1, 0)
    return out



# revision 2
# speedup vs baseline: 1.2780x; 1.2780x over previous
"""Trainium2 Bass kernel for the AgentLayer GRU-with-action-memory model, v5.

B=512 -> 8 cores x 64; two 32-batch streams per core, op-level zippered.
v5 chain restructure over v4 (v4 wall ~7.9us/step-pair, chain-latency bound):
  - obs-window sum lives in a persistent PSUM accumulator (uacc): per-step
    +W1h@h_new / -W1h@h_old matmuls (h-ring in SBUF, [H,NS,BC] layout);
    removes the DVE usum add/sub from the chain head.
  - argmax1 = Pool reduce-max + ONE Pool TSP with accum_out (masked-iota
    sum == first-max index since max is unique); was 3 DVE hops.
  - single 128-partition transposed ring -> 3 fold matmuls (was 6);
    qm built in ONE DVE TSP vs precomputed diag-masked patterns
    (iotaJunk24 / diagpre2 per block).
  - lambda scaling (0.25/0.5) folded into the whs copy (x0.125) with
    whh r,z columns x2 host-side; whs holds wh/2.
  - GRU tail: cur = z*wh + (1-z)*n via off-chain zc/a from thz ->
    only 2 hops after thn.
  - engine rebalance: PSUM-touching chain ops on Pool (idle in v4, no
    modeled PSUM access penalty), ring copy on Act, fusion static add as
    Pool TT (kills 1 matmul + Act copy per step).
"""

import numpy as np
from contextlib import ExitStack

B, T, D, H, S, A, U = 512, 256, 256, 128, 64, 10, 64
NCORES = 8
BCORE = B // NCORES   # 64 per core
NV = 2                # streams per core
BC = BCORE // NV      # 32 per stream
NG = 128 // BC        # partition groups = 4
NS = 12               # ring slots
NR = NS // NG         # ring rows = 3
G3 = 3 * H
BIG = 1024.0
AHEAD = 2

_BUILD_CACHE = {}


def _build(Tn):
    key = Tn
    if key in _BUILD_CACHE:
        return _BUILD_CACHE[key]

    import concourse.bass as bass
    import concourse.bacc as bacc
    import concourse.tile as tile
    from concourse import mybir

    f32 = mybir.dt.float32
    Alu = mybir.AluOpType
    Act = mybir.ActivationFunctionType
    Axis = mybir.AxisListType

    nc = bacc.Bacc("TRN2", target_bir_lowering=False, debug=False)

    d_in = {}

    def din(name, shape):
        d_in[name] = nc.dram_tensor(name, list(shape), f32, kind="ExternalInput").ap()
        return d_in[name]

    xT = din("xT", (2, 128, Tn // 8, NV, 8 * BC))
    wihT = din("wihT", (2, 128, G3))
    whhT = din("whhT", (H, G3))            # r,z cols x2 host-side
    w1h10T = din("w1h10T", (H, U))
    negw1h10T = din("negw1h10T", (H, U))
    w2b1 = din("w2b1", (U + 1, A))
    a2w1xT = din("a2w1xT", (2, 128, U))
    a2w1sT = din("a2w1sT", (S, U))
    w2b2 = din("w2b2", (U + 1, A))
    fuswhT = din("fuswhT", (H, H))
    fus_statT = din("fus_statT", (NV, H, BC))   # [H,BC] = (static@fus_s.T+b).T
    staticrep = din("staticrep", (NV, S, 8 * BC))
    cur0T = din("cur0T", (H, BCORE))       # 0.5 * initial h (wh/2 convention)
    ubaseT = din("ubaseT", (NV, BC, U))    # (W1s@static).T per stream
    a1b1 = din("a1b1", (U, 1))
    a2b1 = din("a2b1", (U, 1))
    halfbr = din("halfbr", (H, 1))
    halfbz = din("halfbz", (H, 1))
    biasn = din("biasn", (H, 1))
    iotamb10 = din("iotamb10", (128, A))
    iotamb80 = din("iotamb80", (128, 8 * A))
    iotamb24 = din("iotamb24", (128, 24, NR))
    delta24 = din("delta24", (128, 24, NR))
    iotaJ24 = din("iotaJ24", (128, 24, NR, BC))
    diag01 = din("diag01", (128, BC))
    dup32 = din("dup32", (BC, 128))
    eye32 = din("eye32", (BC, BC))
    ident = din("ident", (128, 128))

    out_d = nc.dram_tensor("out", [H, Tn, BCORE], f32, kind="ExternalOutput").ap()

    NBLK = Tn // 8
    assert Tn % 16 == 0

    with ExitStack() as ctx:
        tc = ctx.enter_context(tile.TileContext(nc))
        singles = ctx.enter_context(tc.tile_pool(name="singles", bufs=1))
        work = ctx.enter_context(tc.tile_pool(name="work", bufs=3))
        dpre = ctx.enter_context(tc.tile_pool(name="dpre", bufs=3))
        pstate = ctx.enter_context(tc.tile_pool(name="pstate", bufs=1, space="PSUM"))
        pwork = ctx.enter_context(tc.tile_pool(name="pwork", bufs=2, space="PSUM"))
        pA = ctx.enter_context(tc.tile_pool(name="pA", bufs=1, space="PSUM"))
        outsb = ctx.enter_context(tc.tile_pool(name="outsb", bufs=2))

        sb = {}
        for name, ap in d_in.items():
            if name == "xT":
                continue
            if name in ("wihT", "a2w1xT"):
                t = singles.tile([128, 2, ap.shape[2]], f32, tag=f"w_{name}",
                                 name=f"w_{name}")
                for c in range(2):
                    nc.sync.dma_start(out=t[:, c, :], in_=ap[c])
            elif name in ("fus_statT", "staticrep", "ubaseT"):
                t = singles.tile([ap.shape[1], NV, ap.shape[2]], f32,
                                 tag=f"w_{name}", name=f"w_{name}")
                for v in range(NV):
                    nc.sync.dma_start(out=t[:, v, :], in_=ap[v])
            else:
                t = singles.tile(list(ap.shape), f32, tag=f"w_{name}",
                                 name=f"w_{name}")
                nc.sync.dma_start(out=t, in_=ap)
            sb[name] = t

        xsb = singles.tile([128, 2, Tn // 8, NV, 8 * BC], f32, tag="xsb")
        for c in range(2):
            nc.sync.dma_start(out=xsb[:, c], in_=xT[c])

        # ---- per-stream persistent state ----
        # ring2: h_t in natural [H, slot, BC] layout (matmul rhs for uacc)
        ring2 = [singles.tile([H, NS, BC], f32, tag=f"ring2_{v}", name=f"ring2_{v}")
                 for v in range(NV)]
        # ringT: transposed ring for folds: partition p = g*BC+b, row r,
        # slot s = r*NG + g, free = H
        ringT = [singles.tile([128, NR, H], f32, tag=f"ringT{v}", name=f"ringT{v}")
                 for v in range(NV)]
        cur0 = [singles.tile([H, BC], f32, tag=f"cur0{v}", name=f"cur0{v}")
                for v in range(NV)]
        u1t65 = [singles.tile([U + 1, 4 * BC], f32, tag=f"u1t{v}", name=f"u1t{v}")
                 for v in range(NV)]
        u2t65 = [singles.tile([U + 1, 8 * BC], f32, tag=f"u2t{v}", name=f"u2t{v}")
                 for v in range(NV)]

        # PSUM banks (8). start=True lazy-zeroes the WHOLE 2KB row of the
        # touched partitions (pending-zero); matmul writes consume pending
        # (fresh) else accumulate. So: bankU partitions 0:64 hold ONLY uacc;
        # gates bank has ONE start per step (xgates-c0) and every other
        # per-step matmul region (pl1/pwh/ptr/pout) rides the pending row
        # with start=False + skip_group_check (fresh-overwrite / accumulate).
        #  bankU[v] (x2): uacc [0:U, 0:BC] persistent accumulate
        #  bankS[v] (x2): phase-A only: pu2 [0:U, 0:256] (quarter-pumped),
        #                 pl2 [0:BC, 256:336], pdup [:, 336:344]
        #  gates[v] (x2 bufs x2): pr/pz/pin/phn [0:4BC], pren [4BC:5BC],
        #                 pout [5BC:6BC], pl1 [:, 6BC:6BC+10],
        #                 pwh [7BC:8BC], ptr [0:BC, 8BC:12BC]
        bankU = [pstate.tile([128, 512], f32, tag=f"bankU{v}", name=f"bankU{v}")
                 for v in range(NV)]
        bankS = [pA.tile([128, 512], f32, tag=f"bankS{v}", name=f"bankS{v}")
                 for v in range(NV)]

        dpre_tiles = [{} for _ in range(NV)]
        for v in range(NV):
            nc.vector.memset(ring2[v], 0.0)
            nc.vector.memset(ringT[v], 0.0)
            nc.vector.memset(u1t65[v], 1.0)
            nc.vector.memset(u2t65[v], 1.0)
            nc.sync.dma_start(out=cur0[v], in_=cur0T[:, v * BC:(v + 1) * BC])
            # uacc init = W1s@static (ubase): lhsT=[BC,U] rhs=eye32
            nc.tensor.matmul(bankU[v][0:U, 0:BC], sb["ubaseT"][:, v, :],
                             sb["eye32"], start=True, stop=True)

        out_tiles = [{} for _ in range(NV)]

        def bsl(v):
            return slice(v * BC, (v + 1) * BC)

        # ---------- phase A (argmax2 precompute per 8-step block) ----------
        def gen_phaseA(v, blk):
            t0 = blk * 8
            pu2 = bankS[v][0:U, 0:8 * BC]
            for q in range(4):
                qs = slice(q * 64, (q + 1) * 64)
                for c in range(2):
                    nc.tensor.matmul(pu2[:, qs], sb["a2w1xT"][:, c, :],
                                     xsb[:, c, blk, v, qs],
                                     start=(c == 0 and q == 0), stop=False)
                nc.tensor.matmul(pu2[:, qs], sb["a2w1sT"],
                                 sb["staticrep"][:, v, qs],
                                 start=False, stop=(q == 3))
                yield
            nc.scalar.activation(u2t65[v][0:U, :], pu2, Act.Tanh,
                                 bias=sb["a2b1"], scale=1.0)
            yield
            pl2 = bankS[v][0:BC, 256:336].rearrange("p (f a) -> p f a", a=A)
            for j in range(8):
                nc.tensor.matmul(pl2[:, j, :],
                                 u2t65[v][:, j * BC:(j + 1) * BC], sb["w2b2"],
                                 start=(j == 0), stop=(j == 7))
            yield
            rmax2 = work.tile([BC, 8], f32, tag=f"rmax2{v}", name=f"rmax2{v}_{blk}")
            nc.vector.tensor_reduce(out=rmax2, in_=pl2, axis=Axis.X, op=Alu.max)
            yield
            rmax2_b = bass.AP(tensor=rmax2.tensor, offset=rmax2.offset,
                              ap=[rmax2.ap[0], rmax2.ap[1], [0, A]])
            ge2 = work.tile([BC, 8, A], f32, tag=f"ge2{v}", name=f"ge2{v}_{blk}")
            nc.vector.tensor_tensor(out=ge2, in0=pl2, in1=rmax2_b, op=Alu.is_ge)
            yield
            iota_b = sb["iotamb80"][0:BC, :].rearrange("p (f a) -> p f a", a=A)
            mi2 = work.tile([BC, 8, A], f32, tag=f"mi2{v}", name=f"mi2{v}_{blk}")
            nc.gpsimd.tensor_tensor(out=mi2, in0=ge2, in1=iota_b, op=Alu.mult)
            yield
            idx2f = work.tile([BC, 8], f32, tag=f"idx2f{v}", name=f"idx2f{v}_{blk}")
            nc.vector.tensor_reduce(out=idx2f, in_=mi2, axis=Axis.X, op=Alu.min)
            yield
            pdup = bankS[v][:, 336:344]
            nc.tensor.matmul(pdup, sb["dup32"], idx2f, start=True, stop=True)
            yield
            pdup_b = bass.AP(tensor=pdup.tensor, offset=pdup.offset,
                             ap=[pdup.ap[0], pdup.ap[1], [0, NR]])
            tm = t0 % 24
            oh2 = work.tile([128, 8, NR], f32, tag=f"oh2{v}", name=f"oh2{v}_{blk}")
            nc.vector.tensor_tensor(out=oh2, in0=sb["iotamb24"][:, tm:tm + 8, :],
                                    in1=pdup_b, op=Alu.is_equal)
            yield
            pre2 = work.tile([128, 8, NR], f32, tag=f"pre2{v}", name=f"pre2{v}_{blk}")
            nc.vector.tensor_tensor(out=pre2, in0=oh2,
                                    in1=sb["delta24"][:, tm:tm + 8, :], op=Alu.add)
            yield
            # diagpre2[p,j,r,b] = pre2[p,j,r] * diag01[p,b]
            dp = dpre.tile([128, 8, NR, BC], f32, tag=f"dp{v}", name=f"dp{v}_{blk}")
            pre2_b = bass.AP(tensor=pre2.tensor, offset=pre2.offset,
                             ap=[pre2.ap[0], pre2.ap[1], pre2.ap[2], [0, BC]])
            diag_b = bass.AP(tensor=sb["diag01"].tensor, offset=sb["diag01"].offset,
                             ap=[sb["diag01"].ap[0], [0, 8], [0, NR],
                                 sb["diag01"].ap[1]])
            nc.vector.tensor_tensor(out=dp, in0=pre2_b, in1=diag_b, op=Alu.mult)
            dpre_tiles[v][blk] = dp
            yield

        # gates bank layout: pr/pz/pin/phn [0:4BC], thr [4BC:5BC],
        # thz [5BC:6BC], pren [6BC:7BC], thn [7BC:8BC], pout [8BC:9BC]
        def mk_gates(v, t):
            return pwork.tile([128, 512], f32, tag=f"gat{v}", name=f"gat{v}_{t}")

        def gen_gru(v, t, wh_sb, gat):
            # wh_sb holds weighted_h / 2 (whh r,z cols are x2 host-side)
            pr = gat[:, 0:BC]
            pz = gat[:, BC:2 * BC]
            pin = gat[:, 2 * BC:3 * BC]
            phn = gat[:, 3 * BC:4 * BC]
            pren = gat[:, 4 * BC:5 * BC]
            pout = gat[:, 5 * BC:6 * BC]
            thr = work.tile([H, BC], f32, tag=f"thr{v}", name=f"thr{v}_{t}")
            thz = work.tile([H, BC], f32, tag=f"thz{v}", name=f"thz{v}_{t}")
            thn = work.tile([H, BC], f32, tag=f"thn{v}", name=f"thn{v}_{t}")
            # whh parts (need wh_sb; x parts were emitted earlier).
            # pr first: thr is the chain-critical activation.
            nc.tensor.matmul(pr, sb["whhT"][:, 0:H], wh_sb, start=False, stop=False)
            nc.tensor.matmul(phn, sb["whhT"][:, 2 * H:3 * H], wh_sb,
                             start=False, stop=False)
            yield
            nc.tensor.matmul(pz, sb["whhT"][:, H:2 * H], wh_sb, start=False,
                             stop=False, skip_group_check=True)
            yield
            nc.scalar.activation(thr, pr, Act.Tanh, bias=sb["halfbr"], scale=0.5)
            yield
            nc.scalar.activation(thz, pz, Act.Tanh, bias=sb["halfbz"], scale=0.5)
            yield
            # q = (thr+1)*phn_half = sigmoid(r)*phn; pren = q + pin
            # (one DVE block: same-engine deps run back-to-back)
            qsb = work.tile([H, BC], f32, tag=f"qsb{v}", name=f"qsb{v}_{t}")
            nc.vector.scalar_tensor_tensor(out=qsb, in0=thr, scalar=1.0, in1=phn,
                                           op0=Alu.add, op1=Alu.mult)
            nc.vector.tensor_tensor(out=pren, in0=qsb, in1=pin, op=Alu.add)
            yield
            zc = work.tile([H, BC], f32, tag=f"zc{v}", name=f"zc{v}_{t}")
            nc.vector.tensor_scalar(out=zc, in0=thz, scalar1=-0.5, scalar2=0.5,
                                    op0=Alu.mult, op1=Alu.add)
            av = work.tile([H, BC], f32, tag=f"av{v}", name=f"av{v}_{t}")
            nc.vector.scalar_tensor_tensor(out=av, in0=thz, scalar=1.0, in1=wh_sb,
                                           op0=Alu.add, op1=Alu.mult)
            yield
            nc.scalar.activation(thn, pren, Act.Tanh, bias=sb["biasn"], scale=1.0)
            yield
            pump(v)
            bv = work.tile([H, BC], f32, tag=f"bv{v}", name=f"bv{v}_{t}")
            nc.gpsimd.tensor_tensor(out=bv, in0=zc, in1=thn, op=Alu.mult)
            cur = ring2[v][:, t % NS, :]
            nc.gpsimd.tensor_tensor(out=cur, in0=av, in1=bv, op=Alu.add)
            yield
            # next step's chain head immediately behind cur on PE/Act:
            # uacc window update (+c_t, -c_{t-10}) then u1 tanh
            if t + 1 < TMAX:
                uacc = bankU[v][0:U, 0:BC]
                nc.tensor.matmul(uacc, sb["w1h10T"], ring2[v][:, t % NS, :],
                                 start=False, stop=False, skip_group_check=True)
                nc.tensor.matmul(uacc, sb["negw1h10T"],
                                 ring2[v][:, (t - 10) % NS, :],
                                 start=False, stop=True, skip_group_check=True)
                u1out = u1t65[v][0:U, :].rearrange("p (d b) -> p d b", b=BC)
                uacc_b = bass.AP(tensor=uacc.tensor, offset=uacc.offset,
                                 ap=[uacc.ap[0], [0, 4], uacc.ap[1]])
                nc.scalar.activation(u1out, uacc_b, Act.Tanh,
                                     bias=sb["a1b1"], scale=1.0)
                yield
            # transposed ring write: ptr = cur.T (PE), then Act copy into ringT
            s_me = t % NS
            g_me, r_me = s_me % NG, s_me // NG
            ptr = gat[0:BC, 8 * BC:12 * BC]
            nc.tensor.matmul(ptr, cur, sb["ident"], is_transpose=True,
                             start=False, stop=False, skip_group_check=True)
            yield
            nc.scalar.copy(ringT[v][g_me * BC:(g_me + 1) * BC, r_me, :], ptr)
            yield
            pump(v)
            # fusion output (stop=True closes this step's gates-bank group)
            nc.tensor.matmul(pout, sb["fuswhT"], cur, start=False, stop=True,
                             skip_group_check=True)
            yield
            ob16 = t % 16
            if ob16 == 0:
                out_tiles[v][t // 16] = outsb.tile(
                    [H, 16, BC], f32, tag=f"osb{v}", name=f"osb{v}_{t // 16}")
            ot = out_tiles[v][t // 16]
            nc.vector.tensor_tensor(out=ot[:, ob16, :], in0=pout,
                                    in1=sb["fus_statT"][:, v, :], op=Alu.add)
            yield
            if ob16 == 15 or t == Tn - 1:
                nc.sync.dma_start(
                    out=out_d[:, t - ob16:t + 1, bsl(v)],
                    in_=ot[:, 0:ob16 + 1, :])
                del out_tiles[v][t // 16]

        def emit_xgates(v, t, gat):
            # one PSUM group for pr/pz/pin/phn: single start here (pr@c0),
            # single stop at the last whh matmul (pz) in gen_gru
            xx = xsb[:, :, t // 8, v, (t % 8) * BC:(t % 8) * BC + BC]
            for c in range(2):
                nc.tensor.matmul(gat[:, 0:BC], sb["wihT"][:, c, 0:H], xx[:, c, :],
                                 start=(c == 0), stop=False)
                nc.tensor.matmul(gat[:, BC:2 * BC], sb["wihT"][:, c, H:2 * H],
                                 xx[:, c, :], start=False, stop=False)
                yield
                nc.tensor.matmul(gat[:, 2 * BC:3 * BC], sb["wihT"][:, c, 2 * H:3 * H],
                                 xx[:, c, :], start=False, stop=False)
                yield

        def gen_step(v, t):
            if t % 8 == 0 and (t // 8 + AHEAD - 1) < NBLK:
                while pa_gen[v] is not None:   # should already be drained
                    pump(v)
                pa_gen[v] = gen_phaseA(v, t // 8 + AHEAD - 1)
            gat = mk_gates(v, t)
            tm = t % 24
            # uacc/u1 for this step were emitted in the previous step's tail.
            # x-gate matmuls first: they fill PE while u1 finishes on Act.
            yield from emit_xgates(v, t, gat)
            pl1 = gat[:, 6 * BC:6 * BC + A]
            nc.tensor.matmul(pl1, u1t65[v], sb["w2b1"], start=False, stop=False,
                             skip_group_check=True)
            yield
            # argmax1: rmax -> masked-iota-sum idx -> qm, all DVE, emitted as
            # one block so they run back-to-back (same-engine deps are free)
            rmax = work.tile([128, 1], f32, tag=f"rmax{v}", name=f"rmax{v}_{t}")
            nc.vector.tensor_reduce(out=rmax, in_=pl1, axis=Axis.X, op=Alu.max)
            junk = work.tile([128, A], f32, tag=f"junk{v}", name=f"junk{v}_{t}")
            idxf = work.tile([128, 1], f32, tag=f"idxf{v}", name=f"idxf{v}_{t}")
            nc.vector.scalar_tensor_tensor(out=junk, in0=pl1, scalar=rmax[:, 0:1],
                                           in1=sb["iotamb10"],
                                           op0=Alu.is_ge, op1=Alu.mult,
                                           accum_out=idxf)
            qm = work.tile([128, NR, BC], f32, tag=f"qm{v}", name=f"qm{v}_{t}")
            nc.vector.scalar_tensor_tensor(
                out=qm, in0=sb["iotaJ24"][:, tm, :, :], scalar=idxf[:, 0:1],
                in1=dpre_tiles[v][t // 8][:, t % 8, :, :],
                op0=Alu.is_equal, op1=Alu.add)
            yield
            pump(v)
            pwh = gat[:, 7 * BC:8 * BC]
            for r in range(NR):
                nc.tensor.matmul(pwh, ringT[v][:, r, :], qm[:, r, :],
                                 start=False, stop=False, skip_group_check=True)
            yield
            # whs = pwh * 0.125 = weighted_h / 2
            whs = work.tile([H, BC], f32, tag=f"whs{v}", name=f"whs{v}_{t}")
            nc.vector.tensor_scalar(out=whs, in0=pwh, scalar1=0.125, scalar2=None,
                                    op0=Alu.mult)
            yield
            yield from gen_gru(v, t, whs, gat)

        import os as _os
        TMAX = int(_os.environ.get("K5_TMAX", "0")) or Tn

        pa_gen = [None for _ in range(NV)]

        def pump(v):
            g = pa_gen[v]
            if g is not None:
                try:
                    next(g)
                except StopIteration:
                    pa_gen[v] = None

        def gen_stream(v):
            for blk in range(AHEAD):
                yield from gen_phaseA(v, blk)
            gat0 = mk_gates(v, 0)
            yield from emit_xgates(v, 0, gat0)
            yield from gen_gru(v, 0, cur0[v], gat0)
            for t in range(1, TMAX):
                yield from gen_step(v, t)

        gens = [gen_stream(v) for v in range(NV)]
        live = list(gens)
        while live:
            nxt = []
            for g in live:
                try:
                    next(g)
                    nxt.append(g)
                except StopIteration:
                    pass
            live = nxt

    nc.compile()
    _BUILD_CACHE[key] = (nc, "out")
    return _BUILD_CACHE[key]


def _prep_core_inputs(inputs, core, Tn=T):
    f = np.float32
    b0 = core * BCORE
    x = np.ascontiguousarray(inputs["x"][b0:b0 + BCORE, :Tn, :]).astype(f)
    xT = (x.transpose(2, 1, 0).reshape(2, 128, Tn // 8, 8, NV, BC)
          .transpose(0, 1, 2, 4, 3, 5).reshape(2, 128, Tn // 8, NV, 8 * BC))
    xT = np.ascontiguousarray(xT)
    static = inputs["static"][b0:b0 + BCORE].astype(f)
    wih = inputs["gru_wih"].astype(f); whh = inputs["gru_whh"].astype(f)
    a1w1 = inputs["a1_w1"].astype(f); a2w1 = inputs["a2_w1"].astype(f)
    bih = inputs["gru_bih"].astype(f); bhh = inputs["gru_bhh"].astype(f)
    fusw = inputs["fus_w"].astype(f); fusb = inputs["fus_b"].astype(f)

    iotamb24 = np.zeros((128, 24, NR), f)
    delta24 = np.zeros((128, 24, NR), f)
    for p in range(128):
        g = p // BC
        for j in range(24):
            for r in range(NR):
                s = r * NG + g
                a = (s - j + 10) % NS
                if a < A:
                    iotamb24[p, j, r] = a - BIG
            s_new = (j - 1) % NS
            if s_new % NG == g:
                delta24[p, j, s_new // NG] = 2.0
    # iotaJ24[p,j,r,b] = iotamb24[p,j,r] on the diagonal b==p%BC, +BIG off
    iotaJ24 = np.full((128, 24, NR, BC), BIG, f)
    for p in range(128):
        iotaJ24[p, :, :, p % BC] = iotamb24[p]
    diag01 = np.zeros((128, BC), f)
    for p in range(128):
        diag01[p, p % BC] = 1.0
    dup32 = np.zeros((BC, 128), f)
    for b in range(BC):
        for g in range(NG):
            dup32[b, g * BC + b] = 1.0

    fus_statT = np.stack([
        (static[v * BC:(v + 1) * BC] @ fusw[:, H:].T + fusb).T for v in range(NV)
    ])
    staticrep = np.stack([
        np.tile(static[v * BC:(v + 1) * BC].T, (1, 8)) for v in range(NV)
    ])
    cur0 = static @ inputs["init_w"].astype(f).T + inputs["init_b"].astype(f)
    ubase = (static @ a1w1[:, H:].T).T                        # [U, 64]
    ubaseT = np.stack([ubase[:, v * BC:(v + 1) * BC].T for v in range(NV)])

    w1h10 = (a1w1[:, :H] / 10.0).T
    whhT = whh.T.copy()
    whhT[:, 0:2 * H] *= 2.0       # r,z gates see wh/2
    m = {
        "xT": xT,
        "wihT": np.ascontiguousarray(wih.T.reshape(2, 128, G3)),
        "whhT": np.ascontiguousarray(whhT),
        "w1h10T": np.ascontiguousarray(w1h10),
        "negw1h10T": np.ascontiguousarray(-w1h10),
        "w2b1": np.vstack([inputs["a1_w2"].astype(f).T,
                           inputs["a1_b2"].astype(f).reshape(1, A)]),
        "a2w1xT": np.ascontiguousarray(a2w1[:, :D].T.reshape(2, 128, U)),
        "a2w1sT": np.ascontiguousarray(a2w1[:, D:].T),
        "w2b2": np.vstack([inputs["a2_w2"].astype(f).T,
                           inputs["a2_b2"].astype(f).reshape(1, A)]),
        "fuswhT": np.ascontiguousarray(fusw[:, :H].T),
        "fus_statT": fus_statT,
        "staticrep": staticrep,
        "cur0T": np.ascontiguousarray(cur0.T) * 0.5,
        "ubaseT": ubaseT,
        "a1b1": inputs["a1_b1"].astype(f).reshape(U, 1),
        "a2b1": inputs["a2_b1"].astype(f).reshape(U, 1),
        "halfbr": (0.5 * (bih[:H] + bhh[:H])).reshape(H, 1),
        "halfbz": (0.5 * (bih[H:2 * H] + bhh[H:2 * H])).reshape(H, 1),
        "biasn": (bih[2 * H:] + bhh[2 * H:]).reshape(H, 1),
        "iotamb10": np.tile(np.arange(A, dtype=f) - BIG, (128, 1)),
        "iotamb80": np.tile(np.arange(A, dtype=f) - BIG, (128, 8)),
        "iotamb24": iotamb24,
        "delta24": delta24,
        "iotaJ24": iotaJ24,
        "diag01": diag01,
        "dup32": dup32,
        "eye32": np.eye(BC, dtype=f),
        "ident": np.eye(128, dtype=f),
    }
    return {k: np.ascontiguousarray(v, dtype=f) for k, v in m.items()}


def kernel(**inputs):
    from concourse.bass_utils import run_bass_kernel_spmd
    nc, _ = _build(T)
    in_maps = [_prep_core_inputs(inputs, c) for c in range(NCORES)]
    res = run_bass_kernel_spmd(nc, in_maps, core_ids=list(range(NCORES)))
    out = np.empty((B, T, H), np.float32)
    for c in range(NCORES):
        oc = res.results[c]["out"]
        out[c * BCORE:(c + 1) * BCORE] = oc.transpose(2, 1, 0)
    return out


# revision 3
# speedup vs baseline: 1.3293x; 1.0401x over previous
"""Trainium2 Bass kernel for the AgentLayer GRU-with-action-memory model, v5.

B=512 -> 8 cores x 64; two 32-batch streams per core, op-level zippered.
v5 chain restructure over v4 (v4 wall ~7.9us/step-pair, chain-latency bound):
  - obs-window sum lives in a persistent PSUM accumulator (uacc): per-step
    +W1h@h_new / -W1h@h_old matmuls (h-ring in SBUF, [H,NS,BC] layout);
    removes the DVE usum add/sub from the chain head.
  - argmax1 = Pool reduce-max + ONE Pool TSP with accum_out (masked-iota
    sum == first-max index since max is unique); was 3 DVE hops.
  - single 128-partition transposed ring -> 3 fold matmuls (was 6);
    qm built in ONE DVE TSP vs precomputed diag-masked patterns
    (iotaJunk24 / diagpre2 per block).
  - lambda scaling (0.25/0.5) folded into the whs copy (x0.125) with
    whh r,z columns x2 host-side; whs holds wh/2.
  - GRU tail: cur = z*wh + (1-z)*n via off-chain zc/a from thz ->
    only 2 hops after thn.
  - engine rebalance: PSUM-touching chain ops on Pool (idle in v4, no
    modeled PSUM access penalty), ring copy on Act, fusion static add as
    Pool TT (kills 1 matmul + Act copy per step).
"""

import numpy as np
from contextlib import ExitStack

B, T, D, H, S, A, U = 512, 256, 256, 128, 64, 10, 64
NCORES = 8
BCORE = B // NCORES   # 64 per core
NV = 2                # streams per core
BC = BCORE // NV      # 32 per stream
NG = 128 // BC        # partition groups = 4
NS = 12               # ring slots
NR = NS // NG         # ring rows = 3
G3 = 3 * H
BIG = 1024.0
AHEAD = 2

_BUILD_CACHE = {}


def _build(Tn):
    key = Tn
    if key in _BUILD_CACHE:
        return _BUILD_CACHE[key]

    import concourse.bass as bass
    import concourse.bacc as bacc
    import concourse.tile as tile
    from concourse import mybir

    f32 = mybir.dt.float32
    Alu = mybir.AluOpType
    Act = mybir.ActivationFunctionType
    Axis = mybir.AxisListType

    nc = bacc.Bacc("TRN2", target_bir_lowering=False, debug=False)

    d_in = {}

    def din(name, shape):
        d_in[name] = nc.dram_tensor(name, list(shape), f32, kind="ExternalInput").ap()
        return d_in[name]

    xT = din("xT", (2, 128, Tn // 8, NV, 8 * BC))
    wihT = din("wihT", (2, 128, G3))
    whhT = din("whhT", (H, G3))            # r,z cols x2 host-side
    w1h10T = din("w1h10T", (H, U))
    negw1h10T = din("negw1h10T", (H, U))
    w2b1 = din("w2b1", (U + 1, A))
    a2w1xT = din("a2w1xT", (2, 128, U))
    a2w1sT = din("a2w1sT", (S, U))
    w2b2 = din("w2b2", (U + 1, A))
    fuswhT = din("fuswhT", (H, H))
    fus_statT = din("fus_statT", (NV, H, BC))   # [H,BC] = (static@fus_s.T+b).T
    staticrep = din("staticrep", (NV, S, 8 * BC))
    cur0T = din("cur0T", (H, BCORE))       # 0.5 * initial h (wh/2 convention)
    ubaseT = din("ubaseT", (NV, BC, U))    # (W1s@static).T per stream
    a1b1 = din("a1b1", (U, 1))
    a2b1 = din("a2b1", (U, 1))
    halfbr = din("halfbr", (H, 1))
    halfbz = din("halfbz", (H, 1))
    biasn = din("biasn", (H, 1))
    iotamb10 = din("iotamb10", (128, A))
    iotamb80 = din("iotamb80", (128, 8 * A))
    iotamb24 = din("iotamb24", (128, 24, NR))
    delta24 = din("delta24", (128, 24, NR))
    iotaJ24 = din("iotaJ24", (128, 24, NR, BC))
    diag01 = din("diag01", (128, BC))
    dup32 = din("dup32", (BC, 128))
    eye32 = din("eye32", (BC, BC))
    ident = din("ident", (128, 128))

    out_d = nc.dram_tensor("out", [H, Tn, BCORE], f32, kind="ExternalOutput").ap()

    NBLK = Tn // 8
    assert Tn % 16 == 0

    with ExitStack() as ctx:
        tc = ctx.enter_context(tile.TileContext(nc))
        singles = ctx.enter_context(tc.tile_pool(name="singles", bufs=1))
        work = ctx.enter_context(tc.tile_pool(name="work", bufs=3))
        dpre = ctx.enter_context(tc.tile_pool(name="dpre", bufs=3))
        pstate = ctx.enter_context(tc.tile_pool(name="pstate", bufs=1, space="PSUM"))
        pwork = ctx.enter_context(tc.tile_pool(name="pwork", bufs=2, space="PSUM"))
        pA = ctx.enter_context(tc.tile_pool(name="pA", bufs=1, space="PSUM"))
        outsb = ctx.enter_context(tc.tile_pool(name="outsb", bufs=2))

        sb = {}
        for name, ap in d_in.items():
            if name == "xT":
                continue
            if name in ("wihT", "a2w1xT"):
                t = singles.tile([128, 2, ap.shape[2]], f32, tag=f"w_{name}",
                                 name=f"w_{name}")
                for c in range(2):
                    nc.sync.dma_start(out=t[:, c, :], in_=ap[c])
            elif name in ("fus_statT", "staticrep", "ubaseT"):
                t = singles.tile([ap.shape[1], NV, ap.shape[2]], f32,
                                 tag=f"w_{name}", name=f"w_{name}")
                for v in range(NV):
                    nc.sync.dma_start(out=t[:, v, :], in_=ap[v])
            else:
                t = singles.tile(list(ap.shape), f32, tag=f"w_{name}",
                                 name=f"w_{name}")
                nc.sync.dma_start(out=t, in_=ap)
            sb[name] = t

        xsb = singles.tile([128, 2, Tn // 8, NV, 8 * BC], f32, tag="xsb")
        for c in range(2):
            nc.sync.dma_start(out=xsb[:, c], in_=xT[c])

        # ---- per-stream persistent state ----
        # ring2: h_t in natural [H, slot, BC] layout (matmul rhs for uacc)
        ring2 = [singles.tile([H, NS, BC], f32, tag=f"ring2_{v}", name=f"ring2_{v}")
                 for v in range(NV)]
        # ringT: transposed ring for folds: partition p = g*BC+b, row r,
        # slot s = r*NG + g, free = H
        ringT = [singles.tile([128, NR, H], f32, tag=f"ringT{v}", name=f"ringT{v}")
                 for v in range(NV)]
        cur0 = [singles.tile([H, BC], f32, tag=f"cur0{v}", name=f"cur0{v}")
                for v in range(NV)]
        u1t65 = [singles.tile([U + 1, 4 * BC], f32, tag=f"u1t{v}", name=f"u1t{v}")
                 for v in range(NV)]
        u2t65 = [singles.tile([U + 1, 8 * BC], f32, tag=f"u2t{v}", name=f"u2t{v}")
                 for v in range(NV)]

        # PSUM banks (8). start=True lazy-zeroes the WHOLE 2KB row of the
        # touched partitions (pending-zero); matmul writes consume pending
        # (fresh) else accumulate. So: bankU partitions 0:64 hold ONLY uacc;
        # gates bank has ONE start per step (xgates-c0) and every other
        # per-step matmul region (pl1/pwh/ptr/pout) rides the pending row
        # with start=False + skip_group_check (fresh-overwrite / accumulate).
        #  bankU[v] (x2): uacc [0:U, 0:BC] persistent accumulate
        #  bankS[v] (x2): phase-A only: pu2 [0:U, 0:256] (quarter-pumped),
        #                 pl2 [0:BC, 256:336], pdup [:, 336:344]
        #  gates[v] (x2 bufs x2): pr/pz/pin/phn [0:4BC], pren [4BC:5BC],
        #                 pout [5BC:6BC], pl1 [:, 6BC:6BC+10],
        #                 pwh [7BC:8BC], ptr [0:BC, 8BC:12BC]
        bankU = [pstate.tile([128, 512], f32, tag=f"bankU{v}", name=f"bankU{v}")
                 for v in range(NV)]
        bankS = [pA.tile([128, 512], f32, tag=f"bankS{v}", name=f"bankS{v}")
                 for v in range(NV)]

        dpre_tiles = [{} for _ in range(NV)]
        for v in range(NV):
            nc.vector.memset(ring2[v], 0.0)
            nc.vector.memset(ringT[v], 0.0)
            nc.vector.memset(u1t65[v], 1.0)
            nc.vector.memset(u2t65[v], 1.0)
            nc.sync.dma_start(out=cur0[v], in_=cur0T[:, v * BC:(v + 1) * BC])
            # uacc init = W1s@static (ubase): lhsT=[BC,U] rhs=eye32
            nc.tensor.matmul(bankU[v][0:U, 0:BC], sb["ubaseT"][:, v, :],
                             sb["eye32"], start=True, stop=True)

        out_tiles = [{} for _ in range(NV)]

        def bsl(v):
            return slice(v * BC, (v + 1) * BC)

        # ---------- phase A (argmax2 precompute per 8-step block) ----------
        def gen_phaseA(v, blk):
            t0 = blk * 8
            pu2 = bankS[v][0:U, 0:8 * BC]
            for q in range(4):
                qs = slice(q * 64, (q + 1) * 64)
                for c in range(2):
                    nc.tensor.matmul(pu2[:, qs], sb["a2w1xT"][:, c, :],
                                     xsb[:, c, blk, v, qs],
                                     start=(c == 0 and q == 0), stop=False)
                nc.tensor.matmul(pu2[:, qs], sb["a2w1sT"],
                                 sb["staticrep"][:, v, qs],
                                 start=False, stop=(q == 3))
                yield
            nc.scalar.activation(u2t65[v][0:U, :], pu2, Act.Tanh,
                                 bias=sb["a2b1"], scale=1.0)
            yield
            pl2 = bankS[v][0:BC, 256:336].rearrange("p (f a) -> p f a", a=A)
            for j in range(8):
                nc.tensor.matmul(pl2[:, j, :],
                                 u2t65[v][:, j * BC:(j + 1) * BC], sb["w2b2"],
                                 start=(j == 0), stop=(j == 7))
            yield
            rmax2 = work.tile([BC, 8], f32, tag=f"rmax2{v}", name=f"rmax2{v}_{blk}")
            nc.vector.tensor_reduce(out=rmax2, in_=pl2, axis=Axis.X, op=Alu.max)
            yield
            rmax2_b = bass.AP(tensor=rmax2.tensor, offset=rmax2.offset,
                              ap=[rmax2.ap[0], rmax2.ap[1], [0, A]])
            ge2 = work.tile([BC, 8, A], f32, tag=f"ge2{v}", name=f"ge2{v}_{blk}")
            nc.vector.tensor_tensor(out=ge2, in0=pl2, in1=rmax2_b, op=Alu.is_ge)
            yield
            iota_b = sb["iotamb80"][0:BC, :].rearrange("p (f a) -> p f a", a=A)
            mi2 = work.tile([BC, 8, A], f32, tag=f"mi2{v}", name=f"mi2{v}_{blk}")
            nc.gpsimd.tensor_tensor(out=mi2, in0=ge2, in1=iota_b, op=Alu.mult)
            yield
            idx2f = work.tile([BC, 8], f32, tag=f"idx2f{v}", name=f"idx2f{v}_{blk}")
            nc.vector.tensor_reduce(out=idx2f, in_=mi2, axis=Axis.X, op=Alu.min)
            yield
            pdup = bankS[v][:, 336:344]
            nc.tensor.matmul(pdup, sb["dup32"], idx2f, start=True, stop=True)
            yield
            pdup_b = bass.AP(tensor=pdup.tensor, offset=pdup.offset,
                             ap=[pdup.ap[0], pdup.ap[1], [0, NR]])
            tm = t0 % 24
            oh2 = work.tile([128, 8, NR], f32, tag=f"oh2{v}", name=f"oh2{v}_{blk}")
            nc.vector.tensor_tensor(out=oh2, in0=sb["iotamb24"][:, tm:tm + 8, :],
                                    in1=pdup_b, op=Alu.is_equal)
            yield
            pre2 = work.tile([128, 8, NR], f32, tag=f"pre2{v}", name=f"pre2{v}_{blk}")
            nc.vector.tensor_tensor(out=pre2, in0=oh2,
                                    in1=sb["delta24"][:, tm:tm + 8, :], op=Alu.add)
            yield
            # diagpre2[p,j,r,b] = pre2[p,j,r] * diag01[p,b]  (two half-j ops
            # so a pumped op never blocks the DVE chain for more than ~450ns)
            dp = dpre.tile([128, 8, NR, BC], f32, tag=f"dp{v}", name=f"dp{v}_{blk}")
            for jh in range(2):
                js = slice(jh * 4, (jh + 1) * 4)
                pre2_b = bass.AP(tensor=pre2.tensor,
                                 offset=pre2.offset + jh * 4 * NR,
                                 ap=[pre2.ap[0], [NR, 4], [1, NR], [0, BC]])
                diag_b = bass.AP(tensor=sb["diag01"].tensor,
                                 offset=sb["diag01"].offset,
                                 ap=[sb["diag01"].ap[0], [0, 4], [0, NR],
                                     sb["diag01"].ap[1]])
                nc.vector.tensor_tensor(out=dp[:, js], in0=pre2_b, in1=diag_b,
                                        op=Alu.mult)
                yield
            dpre_tiles[v][blk] = dp

        # gates bank layout: pr/pz/pin/phn [0:4BC], thr [4BC:5BC],
        # thz [5BC:6BC], pren [6BC:7BC], thn [7BC:8BC], pout [8BC:9BC]
        def mk_gates(v, t):
            return pwork.tile([128, 512], f32, tag=f"gat{v}", name=f"gat{v}_{t}")

        def gen_gru(v, t, wh_sb, gat):
            # wh_sb holds weighted_h / 2 (whh r,z cols are x2 host-side)
            pr = gat[:, 0:BC]
            pz = gat[:, BC:2 * BC]
            pin = gat[:, 2 * BC:3 * BC]
            phn = gat[:, 3 * BC:4 * BC]
            pren = gat[:, 4 * BC:5 * BC]
            pout = gat[:, 5 * BC:6 * BC]
            thr = work.tile([H, BC], f32, tag=f"thr{v}", name=f"thr{v}_{t}")
            thz = work.tile([H, BC], f32, tag=f"thz{v}", name=f"thz{v}_{t}")
            thn = work.tile([H, BC], f32, tag=f"thn{v}", name=f"thn{v}_{t}")
            # whh parts (need wh_sb; x parts were emitted earlier).
            # pr first: thr is the chain-critical activation.
            nc.tensor.matmul(pr, sb["whhT"][:, 0:H], wh_sb, start=False, stop=False)
            nc.tensor.matmul(phn, sb["whhT"][:, 2 * H:3 * H], wh_sb,
                             start=False, stop=False)
            yield
            nc.tensor.matmul(pz, sb["whhT"][:, H:2 * H], wh_sb, start=False,
                             stop=False, skip_group_check=True)
            yield
            nc.scalar.activation(thr, pr, Act.Tanh, bias=sb["halfbr"], scale=0.5)
            yield
            nc.scalar.activation(thz, pz, Act.Tanh, bias=sb["halfbz"], scale=0.5)
            yield
            # q = (thr+1)*phn_half = sigmoid(r)*phn; pren = q + pin
            # (one DVE block: same-engine deps run back-to-back)
            qsb = work.tile([H, BC], f32, tag=f"qsb{v}", name=f"qsb{v}_{t}")
            nc.vector.scalar_tensor_tensor(out=qsb, in0=thr, scalar=1.0, in1=phn,
                                           op0=Alu.add, op1=Alu.mult)
            nc.vector.tensor_tensor(out=pren, in0=qsb, in1=pin, op=Alu.add)
            yield
            zc = work.tile([H, BC], f32, tag=f"zc{v}", name=f"zc{v}_{t}")
            nc.scalar.activation(zc, thz, Act.Copy, bias=0.5, scale=-0.5)
            av = work.tile([H, BC], f32, tag=f"av{v}", name=f"av{v}_{t}")
            nc.vector.scalar_tensor_tensor(out=av, in0=thz, scalar=1.0, in1=wh_sb,
                                           op0=Alu.add, op1=Alu.mult)
            yield
            nc.scalar.activation(thn, pren, Act.Tanh, bias=sb["biasn"], scale=1.0)
            yield
            pump(v)
            bv = work.tile([H, BC], f32, tag=f"bv{v}", name=f"bv{v}_{t}")
            nc.gpsimd.tensor_tensor(out=bv, in0=zc, in1=thn, op=Alu.mult)
            cur = ring2[v][:, t % NS, :]
            nc.gpsimd.tensor_tensor(out=cur, in0=av, in1=bv, op=Alu.add)
            yield
            # next step's chain head immediately behind cur on PE/Act:
            # uacc window update (+c_t, -c_{t-10}) then u1 tanh
            if t + 1 < TMAX:
                uacc = bankU[v][0:U, 0:BC]
                nc.tensor.matmul(uacc, sb["w1h10T"], ring2[v][:, t % NS, :],
                                 start=False, stop=False, skip_group_check=True)
                nc.tensor.matmul(uacc, sb["negw1h10T"],
                                 ring2[v][:, (t - 10) % NS, :],
                                 start=False, stop=True, skip_group_check=True)
                u1out = u1t65[v][0:U, :].rearrange("p (d b) -> p d b", b=BC)
                uacc_b = bass.AP(tensor=uacc.tensor, offset=uacc.offset,
                                 ap=[uacc.ap[0], [0, 4], uacc.ap[1]])
                nc.scalar.activation(u1out, uacc_b, Act.Tanh,
                                     bias=sb["a1b1"], scale=1.0)
                yield
            # transposed ring write: ptr = cur.T (PE), then Act copy into ringT
            s_me = t % NS
            g_me, r_me = s_me % NG, s_me // NG
            ptr = gat[0:BC, 8 * BC:12 * BC]
            nc.tensor.matmul(ptr, cur, sb["ident"], is_transpose=True,
                             start=False, stop=False, skip_group_check=True)
            yield
            nc.scalar.copy(ringT[v][g_me * BC:(g_me + 1) * BC, r_me, :], ptr)
            yield
            pump(v)
            # fusion output (stop=True closes this step's gates-bank group)
            nc.tensor.matmul(pout, sb["fuswhT"], cur, start=False, stop=True,
                             skip_group_check=True)
            yield
            ob16 = t % 16
            if ob16 == 0:
                out_tiles[v][t // 16] = outsb.tile(
                    [H, 16, BC], f32, tag=f"osb{v}", name=f"osb{v}_{t // 16}")
            ot = out_tiles[v][t // 16]
            nc.vector.tensor_tensor(out=ot[:, ob16, :], in0=pout,
                                    in1=sb["fus_statT"][:, v, :], op=Alu.add)
            yield
            if ob16 == 15 or t == Tn - 1:
                nc.sync.dma_start(
                    out=out_d[:, t - ob16:t + 1, bsl(v)],
                    in_=ot[:, 0:ob16 + 1, :])
                del out_tiles[v][t // 16]

        def emit_xgates(v, t, gat):
            # one PSUM group for pr/pz/pin/phn: single start here (pr@c0),
            # single stop at the last whh matmul (pz) in gen_gru
            xx = xsb[:, :, t // 8, v, (t % 8) * BC:(t % 8) * BC + BC]
            for c in range(2):
                nc.tensor.matmul(gat[:, 0:BC], sb["wihT"][:, c, 0:H], xx[:, c, :],
                                 start=(c == 0), stop=False)
                nc.tensor.matmul(gat[:, BC:2 * BC], sb["wihT"][:, c, H:2 * H],
                                 xx[:, c, :], start=False, stop=False)
                yield
                nc.tensor.matmul(gat[:, 2 * BC:3 * BC], sb["wihT"][:, c, 2 * H:3 * H],
                                 xx[:, c, :], start=False, stop=False)
                yield

        def gen_step(v, t):
            if t % 8 == 0 and (t // 8 + AHEAD - 1) < NBLK:
                while pa_gen[v] is not None:   # should already be drained
                    pump(v)
                pa_gen[v] = gen_phaseA(v, t // 8 + AHEAD - 1)
            gat = mk_gates(v, t)
            tm = t % 24
            # uacc/u1 for this step were emitted in the previous step's tail.
            # x-gate matmuls first: they fill PE while u1 finishes on Act.
            yield from emit_xgates(v, t, gat)
            pl1 = gat[:, 6 * BC:6 * BC + A]
            nc.tensor.matmul(pl1, u1t65[v], sb["w2b1"], start=False, stop=False,
                             skip_group_check=True)
            yield
            # argmax1: rmax -> masked-iota-sum idx -> qm, all DVE, emitted as
            # one block so they run back-to-back (same-engine deps are free)
            rmax = work.tile([128, 1], f32, tag=f"rmax{v}", name=f"rmax{v}_{t}")
            nc.vector.tensor_reduce(out=rmax, in_=pl1, axis=Axis.X, op=Alu.max)
            junk = work.tile([128, A], f32, tag=f"junk{v}", name=f"junk{v}_{t}")
            idxf = work.tile([128, 1], f32, tag=f"idxf{v}", name=f"idxf{v}_{t}")
            nc.vector.scalar_tensor_tensor(out=junk, in0=pl1, scalar=rmax[:, 0:1],
                                           in1=sb["iotamb10"],
                                           op0=Alu.is_ge, op1=Alu.mult,
                                           accum_out=idxf)
            qm = work.tile([128, NR, BC], f32, tag=f"qm{v}", name=f"qm{v}_{t}")
            nc.vector.scalar_tensor_tensor(
                out=qm, in0=sb["iotaJ24"][:, tm, :, :], scalar=idxf[:, 0:1],
                in1=dpre_tiles[v][t // 8][:, t % 8, :, :],
                op0=Alu.is_equal, op1=Alu.add)
            yield
            pump(v)
            pwh = gat[:, 7 * BC:8 * BC]
            for r in range(NR):
                nc.tensor.matmul(pwh, ringT[v][:, r, :], qm[:, r, :],
                                 start=False, stop=False, skip_group_check=True)
            yield
            # whs = pwh * 0.125 = weighted_h / 2
            whs = work.tile([H, BC], f32, tag=f"whs{v}", name=f"whs{v}_{t}")
            nc.vector.tensor_scalar(out=whs, in0=pwh, scalar1=0.125, scalar2=None,
                                    op0=Alu.mult)
            yield
            yield from gen_gru(v, t, whs, gat)

        import os as _os
        TMAX = int(_os.environ.get("K5_TMAX", "0")) or Tn

        pa_gen = [None for _ in range(NV)]

        def pump(v):
            g = pa_gen[v]
            if g is not None:
                try:
                    next(g)
                except StopIteration:
                    pa_gen[v] = None

        def gen_stream(v):
            for blk in range(AHEAD):
                yield from gen_phaseA(v, blk)
            gat0 = mk_gates(v, 0)
            yield from emit_xgates(v, 0, gat0)
            yield from gen_gru(v, 0, cur0[v], gat0)
            for t in range(1, TMAX):
                yield from gen_step(v, t)

        gens = [gen_stream(v) for v in range(NV)]
        live = list(gens)
        while live:
            nxt = []
            for g in live:
                try:
                    next(g)
                    nxt.append(g)
                except StopIteration:
                    pass
            live = nxt

    nc.compile()
    _BUILD_CACHE[key] = (nc, "out")
    return _BUILD_CACHE[key]


def _prep_core_inputs(inputs, core, Tn=T):
    f = np.float32
    b0 = core * BCORE
    x = np.ascontiguousarray(inputs["x"][b0:b0 + BCORE, :Tn, :]).astype(f)
    xT = (x.transpose(2, 1, 0).reshape(2, 128, Tn // 8, 8, NV, BC)
          .transpose(0, 1, 2, 4, 3, 5).reshape(2, 128, Tn // 8, NV, 8 * BC))
    xT = np.ascontiguousarray(xT)
    static = inputs["static"][b0:b0 + BCORE].astype(f)
    wih = inputs["gru_wih"].astype(f); whh = inputs["gru_whh"].astype(f)
    a1w1 = inputs["a1_w1"].astype(f); a2w1 = inputs["a2_w1"].astype(f)
    bih = inputs["gru_bih"].astype(f); bhh = inputs["gru_bhh"].astype(f)
    fusw = inputs["fus_w"].astype(f); fusb = inputs["fus_b"].astype(f)

    iotamb24 = np.zeros((128, 24, NR), f)
    delta24 = np.zeros((128, 24, NR), f)
    for p in range(128):
        g = p // BC
        for j in range(24):
            for r in range(NR):
                s = r * NG + g
                a = (s - j + 10) % NS
                if a < A:
                    iotamb24[p, j, r] = a - BIG
            s_new = (j - 1) % NS
            if s_new % NG == g:
                delta24[p, j, s_new // NG] = 2.0
    # iotaJ24[p,j,r,b] = iotamb24[p,j,r] on the diagonal b==p%BC, +BIG off
    iotaJ24 = np.full((128, 24, NR, BC), BIG, f)
    for p in range(128):
        iotaJ24[p, :, :, p % BC] = iotamb24[p]
    diag01 = np.zeros((128, BC), f)
    for p in range(128):
        diag01[p, p % BC] = 1.0
    dup32 = np.zeros((BC, 128), f)
    for b in range(BC):
        for g in range(NG):
            dup32[b, g * BC + b] = 1.0

    fus_statT = np.stack([
        (static[v * BC:(v + 1) * BC] @ fusw[:, H:].T + fusb).T for v in range(NV)
    ])
    staticrep = np.stack([
        np.tile(static[v * BC:(v + 1) * BC].T, (1, 8)) for v in range(NV)
    ])
    cur0 = static @ inputs["init_w"].astype(f).T + inputs["init_b"].astype(f)
    ubase = (static @ a1w1[:, H:].T).T                        # [U, 64]
    ubaseT = np.stack([ubase[:, v * BC:(v + 1) * BC].T for v in range(NV)])

    w1h10 = (a1w1[:, :H] / 10.0).T
    whhT = whh.T.copy()
    whhT[:, 0:2 * H] *= 2.0       # r,z gates see wh/2
    m = {
        "xT": xT,
        "wihT": np.ascontiguousarray(wih.T.reshape(2, 128, G3)),
        "whhT": np.ascontiguousarray(whhT),
        "w1h10T": np.ascontiguousarray(w1h10),
        "negw1h10T": np.ascontiguousarray(-w1h10),
        "w2b1": np.vstack([inputs["a1_w2"].astype(f).T,
                           inputs["a1_b2"].astype(f).reshape(1, A)]),
        "a2w1xT": np.ascontiguousarray(a2w1[:, :D].T.reshape(2, 128, U)),
        "a2w1sT": np.ascontiguousarray(a2w1[:, D:].T),
        "w2b2": np.vstack([inputs["a2_w2"].astype(f).T,
                           inputs["a2_b2"].astype(f).reshape(1, A)]),
        "fuswhT": np.ascontiguousarray(fusw[:, :H].T),
        "fus_statT": fus_statT,
        "staticrep": staticrep,
        "cur0T": np.ascontiguousarray(cur0.T) * 0.5,
        "ubaseT": ubaseT,
        "a1b1": inputs["a1_b1"].astype(f).reshape(U, 1),
        "a2b1": inputs["a2_b1"].astype(f).reshape(U, 1),
        "halfbr": (0.5 * (bih[:H] + bhh[:H])).reshape(H, 1),
        "halfbz": (0.5 * (bih[H:2 * H] + bhh[H:2 * H])).reshape(H, 1),
        "biasn": (bih[2 * H:] + bhh[2 * H:]).reshape(H, 1),
        "iotamb10": np.tile(np.arange(A, dtype=f) - BIG, (128, 1)),
        "iotamb80": np.tile(np.arange(A, dtype=f) - BIG, (128, 8)),
        "iotamb24": iotamb24,
        "delta24": delta24,
        "iotaJ24": iotaJ24,
        "diag01": diag01,
        "dup32": dup32,
        "eye32": np.eye(BC, dtype=f),
        "ident": np.eye(128, dtype=f),
    }
    return {k: np.ascontiguousarray(v, dtype=f) for k, v in m.items()}


def kernel(**inputs):
    from concourse.bass_utils import run_bass_kernel_spmd
    nc, _ = _build(T)
    in_maps = [_prep_core_inputs(inputs, c) for c in range(NCORES)]
    res = run_bass_kernel_spmd(nc, in_maps, core_ids=list(range(NCORES)))
    out = np.empty((B, T, H), np.float32)
    for c in range(NCORES):
        oc = res.results[c]["out"]
        out[c * BCORE:(c + 1) * BCORE] = oc.transpose(2, 1, 0)
    return out


# revision 4
# speedup vs baseline: 1.3320x; 1.0020x over previous
"""Trainium2 Bass kernel for the AgentLayer GRU-with-action-memory model, v5.

B=512 -> 8 cores x 64; two 32-batch streams per core, op-level zippered.
v5 chain restructure over v4 (v4 wall ~7.9us/step-pair, chain-latency bound):
  - obs-window sum lives in a persistent PSUM accumulator (uacc): per-step
    +W1h@h_new / -W1h@h_old matmuls (h-ring in SBUF, [H,NS,BC] layout);
    removes the DVE usum add/sub from the chain head.
  - argmax1 = Pool reduce-max + ONE Pool TSP with accum_out (masked-iota
    sum == first-max index since max is unique); was 3 DVE hops.
  - single 128-partition transposed ring -> 3 fold matmuls (was 6);
    qm built in ONE DVE TSP vs precomputed diag-masked patterns
    (iotaJunk24 / diagpre2 per block).
  - lambda scaling (0.25/0.5) folded into the whs copy (x0.125) with
    whh r,z columns x2 host-side; whs holds wh/2.
  - GRU tail: cur = z*wh + (1-z)*n via off-chain zc/a from thz ->
    only 2 hops after thn.
  - engine rebalance: PSUM-touching chain ops on Pool (idle in v4, no
    modeled PSUM access penalty), ring copy on Act, fusion static add as
    Pool TT (kills 1 matmul + Act copy per step).
"""

import numpy as np
from contextlib import ExitStack

B, T, D, H, S, A, U = 512, 256, 256, 128, 64, 10, 64
NCORES = 8
BCORE = B // NCORES   # 64 per core
NV = 2                # streams per core
BC = BCORE // NV      # 32 per stream
NG = 128 // BC        # partition groups = 4
NS = 12               # ring slots
NR = NS // NG         # ring rows = 3
G3 = 3 * H
BIG = 1024.0
AHEAD = 2

_BUILD_CACHE = {}


def _build(Tn):
    key = Tn
    if key in _BUILD_CACHE:
        return _BUILD_CACHE[key]

    import concourse.bass as bass
    import concourse.bacc as bacc
    import concourse.tile as tile
    from concourse import mybir

    f32 = mybir.dt.float32
    Alu = mybir.AluOpType
    Act = mybir.ActivationFunctionType
    Axis = mybir.AxisListType

    nc = bacc.Bacc("TRN2", target_bir_lowering=False, debug=False)

    d_in = {}

    def din(name, shape):
        d_in[name] = nc.dram_tensor(name, list(shape), f32, kind="ExternalInput").ap()
        return d_in[name]

    xT = din("xT", (2, 128, Tn // 8, NV, 8 * BC))
    wihT = din("wihT", (2, 128, G3))
    whhT = din("whhT", (H, G3))            # r,z cols x2 host-side
    w1h10T = din("w1h10T", (H, U))
    negw1h10T = din("negw1h10T", (H, U))
    w2b1 = din("w2b1", (U + 1, A))
    a2w1xT = din("a2w1xT", (2, 128, U))
    a2w1sT = din("a2w1sT", (S, U))
    w2b2 = din("w2b2", (U + 1, A))
    fuswhT = din("fuswhT", (H, H))
    fus_statT = din("fus_statT", (NV, H, BC))   # [H,BC] = (static@fus_s.T+b).T
    staticrep = din("staticrep", (NV, S, 8 * BC))
    cur0T = din("cur0T", (H, BCORE))       # 0.5 * initial h (wh/2 convention)
    ubaseT = din("ubaseT", (NV, BC, U))    # (W1s@static).T per stream
    a1b1 = din("a1b1", (U, 1))
    a2b1 = din("a2b1", (U, 1))
    halfbr = din("halfbr", (H, 1))
    halfbz = din("halfbz", (H, 1))
    biasn = din("biasn", (H, 1))
    iotamb10 = din("iotamb10", (128, A))
    iotamb80 = din("iotamb80", (128, 8 * A))
    iotamb24 = din("iotamb24", (128, 24, NR))
    delta24 = din("delta24", (128, 24, NR))
    iotaJ24 = din("iotaJ24", (128, 24, NR, BC))
    diag01 = din("diag01", (128, BC))
    dup32 = din("dup32", (BC, 128))
    eye32 = din("eye32", (BC, BC))
    ident = din("ident", (128, 128))

    out_d = nc.dram_tensor("out", [H, Tn, BCORE], f32, kind="ExternalOutput").ap()

    NBLK = Tn // 8
    assert Tn % 16 == 0

    with ExitStack() as ctx:
        tc = ctx.enter_context(tile.TileContext(nc))
        singles = ctx.enter_context(tc.tile_pool(name="singles", bufs=1))
        work = ctx.enter_context(tc.tile_pool(name="work", bufs=3))
        dpre = ctx.enter_context(tc.tile_pool(name="dpre", bufs=3))
        pstate = ctx.enter_context(tc.tile_pool(name="pstate", bufs=1, space="PSUM"))
        pwork = ctx.enter_context(tc.tile_pool(name="pwork", bufs=2, space="PSUM"))
        pA = ctx.enter_context(tc.tile_pool(name="pA", bufs=1, space="PSUM"))
        outsb = ctx.enter_context(tc.tile_pool(name="outsb", bufs=2))

        sb = {}
        for name, ap in d_in.items():
            if name == "xT":
                continue
            if name in ("wihT", "a2w1xT"):
                t = singles.tile([128, 2, ap.shape[2]], f32, tag=f"w_{name}",
                                 name=f"w_{name}")
                for c in range(2):
                    nc.sync.dma_start(out=t[:, c, :], in_=ap[c])
            elif name in ("fus_statT", "staticrep", "ubaseT"):
                t = singles.tile([ap.shape[1], NV, ap.shape[2]], f32,
                                 tag=f"w_{name}", name=f"w_{name}")
                for v in range(NV):
                    nc.sync.dma_start(out=t[:, v, :], in_=ap[v])
            else:
                t = singles.tile(list(ap.shape), f32, tag=f"w_{name}",
                                 name=f"w_{name}")
                nc.sync.dma_start(out=t, in_=ap)
            sb[name] = t

        xsb = singles.tile([128, 2, Tn // 8, NV, 8 * BC], f32, tag="xsb")
        for c in range(2):
            nc.sync.dma_start(out=xsb[:, c], in_=xT[c])

        # ---- per-stream persistent state ----
        # ring2: h_t in natural [H, slot, BC] layout (matmul rhs for uacc)
        ring2 = [singles.tile([H, NS, BC], f32, tag=f"ring2_{v}", name=f"ring2_{v}")
                 for v in range(NV)]
        # ringT: transposed ring for folds: partition p = g*BC+b, row r,
        # slot s = r*NG + g, free = H
        ringT = [singles.tile([128, NR, H], f32, tag=f"ringT{v}", name=f"ringT{v}")
                 for v in range(NV)]
        cur0 = [singles.tile([H, BC], f32, tag=f"cur0{v}", name=f"cur0{v}")
                for v in range(NV)]
        u1t65 = [singles.tile([U + 1, 4 * BC], f32, tag=f"u1t{v}", name=f"u1t{v}")
                 for v in range(NV)]
        u2t65 = [singles.tile([U + 1, 8 * BC], f32, tag=f"u2t{v}", name=f"u2t{v}")
                 for v in range(NV)]

        # PSUM banks (8). start=True lazy-zeroes the WHOLE 2KB row of the
        # touched partitions (pending-zero); matmul writes consume pending
        # (fresh) else accumulate. So: bankU partitions 0:64 hold ONLY uacc;
        # gates bank has ONE start per step (xgates-c0) and every other
        # per-step matmul region (pl1/pwh/ptr/pout) rides the pending row
        # with start=False + skip_group_check (fresh-overwrite / accumulate).
        #  bankU[v] (x2): uacc [0:U, 0:BC] persistent accumulate
        #  bankS[v] (x2): phase-A only: pu2 [0:U, 0:256] (quarter-pumped),
        #                 pl2 [0:BC, 256:336], pdup [:, 336:344]
        #  gates[v] (x2 bufs x2): pr/pz/pin/phn [0:4BC], pren [4BC:5BC],
        #                 pout [5BC:6BC], pl1 [:, 6BC:6BC+10],
        #                 pwh [7BC:8BC], ptr [0:BC, 8BC:12BC]
        bankU = [pstate.tile([128, 512], f32, tag=f"bankU{v}", name=f"bankU{v}")
                 for v in range(NV)]
        bankS = [pA.tile([128, 512], f32, tag=f"bankS{v}", name=f"bankS{v}")
                 for v in range(NV)]

        dpre_tiles = [{} for _ in range(NV)]
        for v in range(NV):
            nc.vector.memset(ring2[v], 0.0)
            nc.vector.memset(ringT[v], 0.0)
            nc.vector.memset(u1t65[v], 1.0)
            nc.vector.memset(u2t65[v], 1.0)
            nc.sync.dma_start(out=cur0[v], in_=cur0T[:, v * BC:(v + 1) * BC])
            # uacc init = W1s@static (ubase): lhsT=[BC,U] rhs=eye32
            nc.tensor.matmul(bankU[v][0:U, 0:BC], sb["ubaseT"][:, v, :],
                             sb["eye32"], start=True, stop=True)

        out_tiles = [{} for _ in range(NV)]

        def bsl(v):
            return slice(v * BC, (v + 1) * BC)

        # ---------- phase A (argmax2 precompute per 8-step block) ----------
        def gen_phaseA(v, blk):
            t0 = blk * 8
            pu2 = bankS[v][0:U, 0:8 * BC]
            for q in range(4):
                qs = slice(q * 64, (q + 1) * 64)
                for c in range(2):
                    nc.tensor.matmul(pu2[:, qs], sb["a2w1xT"][:, c, :],
                                     xsb[:, c, blk, v, qs],
                                     start=(c == 0 and q == 0), stop=False)
                nc.tensor.matmul(pu2[:, qs], sb["a2w1sT"],
                                 sb["staticrep"][:, v, qs],
                                 start=False, stop=(q == 3))
                yield
            nc.scalar.activation(u2t65[v][0:U, :], pu2, Act.Tanh,
                                 bias=sb["a2b1"], scale=1.0)
            yield
            pl2 = bankS[v][0:BC, 256:336].rearrange("p (f a) -> p f a", a=A)
            for j in range(8):
                nc.tensor.matmul(pl2[:, j, :],
                                 u2t65[v][:, j * BC:(j + 1) * BC], sb["w2b2"],
                                 start=(j == 0), stop=(j == 7))
            yield
            rmax2 = work.tile([BC, 8], f32, tag=f"rmax2{v}", name=f"rmax2{v}_{blk}")
            nc.vector.tensor_reduce(out=rmax2, in_=pl2, axis=Axis.X, op=Alu.max)
            yield
            rmax2_b = bass.AP(tensor=rmax2.tensor, offset=rmax2.offset,
                              ap=[rmax2.ap[0], rmax2.ap[1], [0, A]])
            ge2 = work.tile([BC, 8, A], f32, tag=f"ge2{v}", name=f"ge2{v}_{blk}")
            nc.vector.tensor_tensor(out=ge2, in0=pl2, in1=rmax2_b, op=Alu.is_ge)
            yield
            iota_b = sb["iotamb80"][0:BC, :].rearrange("p (f a) -> p f a", a=A)
            mi2 = work.tile([BC, 8, A], f32, tag=f"mi2{v}", name=f"mi2{v}_{blk}")
            nc.gpsimd.tensor_tensor(out=mi2, in0=ge2, in1=iota_b, op=Alu.mult)
            yield
            idx2f = work.tile([BC, 8], f32, tag=f"idx2f{v}", name=f"idx2f{v}_{blk}")
            nc.vector.tensor_reduce(out=idx2f, in_=mi2, axis=Axis.X, op=Alu.min)
            yield
            pdup = bankS[v][:, 336:344]
            nc.tensor.matmul(pdup, sb["dup32"], idx2f, start=True, stop=True)
            yield
            pdup_b = bass.AP(tensor=pdup.tensor, offset=pdup.offset,
                             ap=[pdup.ap[0], pdup.ap[1], [0, NR]])
            tm = t0 % 24
            oh2 = work.tile([128, 8, NR], f32, tag=f"oh2{v}", name=f"oh2{v}_{blk}")
            nc.vector.tensor_tensor(out=oh2, in0=sb["iotamb24"][:, tm:tm + 8, :],
                                    in1=pdup_b, op=Alu.is_equal)
            yield
            pre2 = work.tile([128, 8, NR], f32, tag=f"pre2{v}", name=f"pre2{v}_{blk}")
            nc.vector.tensor_tensor(out=pre2, in0=oh2,
                                    in1=sb["delta24"][:, tm:tm + 8, :], op=Alu.add)
            yield
            # diagpre2[p,j,r,b] = pre2[p,j,r] * diag01[p,b]  (two half-j ops
            # so a pumped op never blocks the DVE chain for more than ~450ns)
            dp = dpre.tile([128, 8, NR, BC], f32, tag=f"dp{v}", name=f"dp{v}_{blk}")
            for jh in range(2):
                js = slice(jh * 4, (jh + 1) * 4)
                pre2_b = bass.AP(tensor=pre2.tensor,
                                 offset=pre2.offset + jh * 4 * NR,
                                 ap=[pre2.ap[0], [NR, 4], [1, NR], [0, BC]])
                diag_b = bass.AP(tensor=sb["diag01"].tensor,
                                 offset=sb["diag01"].offset,
                                 ap=[sb["diag01"].ap[0], [0, 4], [0, NR],
                                     sb["diag01"].ap[1]])
                nc.vector.tensor_tensor(out=dp[:, js], in0=pre2_b, in1=diag_b,
                                        op=Alu.mult)
                yield
            dpre_tiles[v][blk] = dp

        # gates bank layout: pr/pz/pin/phn [0:4BC], thr [4BC:5BC],
        # thz [5BC:6BC], pren [6BC:7BC], thn [7BC:8BC], pout [8BC:9BC]
        def mk_gates(v, t):
            return pwork.tile([128, 512], f32, tag=f"gat{v}", name=f"gat{v}_{t}")

        def gen_gru(v, t, wh_sb, gat):
            # wh_sb holds weighted_h / 2 (whh r,z cols are x2 host-side)
            pr = gat[:, 0:BC]
            pz = gat[:, BC:2 * BC]
            pin = gat[:, 2 * BC:3 * BC]
            phn = gat[:, 3 * BC:4 * BC]
            pren = gat[:, 4 * BC:5 * BC]
            pout = gat[:, 5 * BC:6 * BC]
            thr = work.tile([H, BC], f32, tag=f"thr{v}", name=f"thr{v}_{t}")
            thz = work.tile([H, BC], f32, tag=f"thz{v}", name=f"thz{v}_{t}")
            thn = work.tile([H, BC], f32, tag=f"thn{v}", name=f"thn{v}_{t}")
            # whh parts (need wh_sb; x parts were emitted earlier).
            # pr first: thr is the chain-critical activation.
            nc.tensor.matmul(pr, sb["whhT"][:, 0:H], wh_sb, start=False, stop=False)
            nc.tensor.matmul(phn, sb["whhT"][:, 2 * H:3 * H], wh_sb,
                             start=False, stop=False)
            yield
            nc.tensor.matmul(pz, sb["whhT"][:, H:2 * H], wh_sb, start=False,
                             stop=False, skip_group_check=True)
            yield
            nc.scalar.activation(thr, pr, Act.Tanh, bias=sb["halfbr"], scale=0.5)
            yield
            nc.scalar.activation(thz, pz, Act.Tanh, bias=sb["halfbz"], scale=0.5)
            yield
            # q = (thr+1)*phn_half = sigmoid(r)*phn; pren = q + pin
            # (one DVE block: same-engine deps run back-to-back)
            qsb = work.tile([H, BC], f32, tag=f"qsb{v}", name=f"qsb{v}_{t}")
            nc.vector.scalar_tensor_tensor(out=qsb, in0=thr, scalar=1.0, in1=phn,
                                           op0=Alu.add, op1=Alu.mult)
            nc.vector.tensor_tensor(out=pren, in0=qsb, in1=pin, op=Alu.add)
            yield
            zc = work.tile([H, BC], f32, tag=f"zc{v}", name=f"zc{v}_{t}")
            nc.scalar.activation(zc, thz, Act.Copy, bias=0.5, scale=-0.5)
            av = work.tile([H, BC], f32, tag=f"av{v}", name=f"av{v}_{t}")
            nc.vector.scalar_tensor_tensor(out=av, in0=thz, scalar=1.0, in1=wh_sb,
                                           op0=Alu.add, op1=Alu.mult)
            yield
            nc.scalar.activation(thn, pren, Act.Tanh, bias=sb["biasn"], scale=1.0)
            yield
            pump(v)
            bv = work.tile([H, BC], f32, tag=f"bv{v}", name=f"bv{v}_{t}")
            nc.gpsimd.tensor_tensor(out=bv, in0=zc, in1=thn, op=Alu.mult)
            cur = ring2[v][:, t % NS, :]
            nc.gpsimd.tensor_tensor(out=cur, in0=av, in1=bv, op=Alu.add)
            yield
            # next step's chain head immediately behind cur on PE/Act:
            # uacc window update (+c_t, -c_{t-10}) then u1 tanh
            if t + 1 < TMAX:
                uacc = bankU[v][0:U, 0:BC]
                nc.tensor.matmul(uacc, sb["w1h10T"], ring2[v][:, t % NS, :],
                                 start=False, stop=False, skip_group_check=True)
                nc.tensor.matmul(uacc, sb["negw1h10T"],
                                 ring2[v][:, (t - 10) % NS, :],
                                 start=False, stop=True, skip_group_check=True)
                u1out = u1t65[v][0:U, :].rearrange("p (d b) -> p d b", b=BC)
                uacc_b = bass.AP(tensor=uacc.tensor, offset=uacc.offset,
                                 ap=[uacc.ap[0], [0, 4], uacc.ap[1]])
                nc.scalar.activation(u1out, uacc_b, Act.Tanh,
                                     bias=sb["a1b1"], scale=1.0)
                yield
            # transposed ring write: ptr = cur.T (PE), then Act copy into ringT
            s_me = t % NS
            g_me, r_me = s_me % NG, s_me // NG
            ptr = gat[0:BC, 8 * BC:12 * BC]
            nc.tensor.matmul(ptr, cur, sb["ident"], is_transpose=True,
                             start=False, stop=False, skip_group_check=True)
            yield
            nc.scalar.copy(ringT[v][g_me * BC:(g_me + 1) * BC, r_me, :], ptr)
            yield
            pump(v)
            # fusion output (stop=True closes this step's gates-bank group)
            nc.tensor.matmul(pout, sb["fuswhT"], cur, start=False, stop=True,
                             skip_group_check=True)
            yield
            ob16 = t % 16
            if ob16 == 0:
                out_tiles[v][t // 16] = outsb.tile(
                    [H, 16, BC], f32, tag=f"osb{v}", name=f"osb{v}_{t // 16}")
            ot = out_tiles[v][t // 16]
            nc.vector.tensor_tensor(out=ot[:, ob16, :], in0=pout,
                                    in1=sb["fus_statT"][:, v, :], op=Alu.add)
            yield
            if ob16 == 15 or t == Tn - 1:
                nc.sync.dma_start(
                    out=out_d[:, t - ob16:t + 1, bsl(v)],
                    in_=ot[:, 0:ob16 + 1, :])
                del out_tiles[v][t // 16]

        def emit_xgates(v, t, gat):
            # one PSUM group for pr/pz/pin/phn: single start here (pr@c0),
            # single stop at the last whh matmul (pz) in gen_gru
            xx = xsb[:, :, t // 8, v, (t % 8) * BC:(t % 8) * BC + BC]
            for c in range(2):
                nc.tensor.matmul(gat[:, 0:BC], sb["wihT"][:, c, 0:H], xx[:, c, :],
                                 start=(c == 0), stop=False)
                nc.tensor.matmul(gat[:, BC:2 * BC], sb["wihT"][:, c, H:2 * H],
                                 xx[:, c, :], start=False, stop=False)
                yield
                nc.tensor.matmul(gat[:, 2 * BC:3 * BC], sb["wihT"][:, c, 2 * H:3 * H],
                                 xx[:, c, :], start=False, stop=False)
                yield

        def gen_step(v, t):
            if t % 8 == 0 and (t // 8 + AHEAD - 1) < NBLK:
                while pa_gen[v] is not None:   # should already be drained
                    pump(v)
                pa_gen[v] = gen_phaseA(v, t // 8 + AHEAD - 1)
            gat = mk_gates(v, t)
            tm = t % 24
            # uacc/u1 for this step were emitted in the previous step's tail.
            # x-gate matmuls first: they fill PE while u1 finishes on Act.
            yield from emit_xgates(v, t, gat)
            pl1 = gat[:, 6 * BC:6 * BC + A]
            nc.tensor.matmul(pl1, u1t65[v], sb["w2b1"], start=False, stop=False,
                             skip_group_check=True)
            yield
            # argmax1: rmax -> masked-iota-sum idx -> qm, all DVE, emitted as
            # one block so they run back-to-back (same-engine deps are free)
            rmax = work.tile([128, 1], f32, tag=f"rmax{v}", name=f"rmax{v}_{t}")
            nc.vector.tensor_reduce(out=rmax, in_=pl1, axis=Axis.X, op=Alu.max)
            junk = work.tile([128, A], f32, tag=f"junk{v}", name=f"junk{v}_{t}")
            idxf = work.tile([128, 1], f32, tag=f"idxf{v}", name=f"idxf{v}_{t}")
            nc.vector.scalar_tensor_tensor(out=junk, in0=pl1, scalar=rmax[:, 0:1],
                                           in1=sb["iotamb10"],
                                           op0=Alu.is_ge, op1=Alu.mult,
                                           accum_out=idxf)
            qm = work.tile([128, NR, BC], f32, tag=f"qm{v}", name=f"qm{v}_{t}")
            nc.vector.scalar_tensor_tensor(
                out=qm, in0=sb["iotaJ24"][:, tm, :, :], scalar=idxf[:, 0:1],
                in1=dpre_tiles[v][t // 8][:, t % 8, :, :],
                op0=Alu.is_equal, op1=Alu.add)
            yield
            pump(v)
            pwh = gat[:, 7 * BC:8 * BC]
            for r in range(NR):
                nc.tensor.matmul(pwh, ringT[v][:, r, :], qm[:, r, :],
                                 start=False, stop=False, skip_group_check=True)
            yield
            # whs = pwh * 0.125 = weighted_h / 2
            whs = work.tile([H, BC], f32, tag=f"whs{v}", name=f"whs{v}_{t}")
            nc.vector.tensor_scalar(out=whs, in0=pwh, scalar1=0.125, scalar2=None,
                                    op0=Alu.mult)
            yield
            yield from gen_gru(v, t, whs, gat)

        import os as _os
        TMAX = int(_os.environ.get("K5_TMAX", "0")) or Tn

        pa_gen = [None for _ in range(NV)]

        def pump(v):
            g = pa_gen[v]
            if g is not None:
                try:
                    next(g)
                except StopIteration:
                    pa_gen[v] = None

        def gen_stream(v):
            # stagger stream 1 by 19 zipper slots: measured-best cross-stream
            # phase (keeps B's DVE argmax burst out of A's chain hops)
            if v == 1:
                for _ in range(19):
                    yield
            for blk in range(AHEAD):
                yield from gen_phaseA(v, blk)
            gat0 = mk_gates(v, 0)
            yield from emit_xgates(v, 0, gat0)
            yield from gen_gru(v, 0, cur0[v], gat0)
            for t in range(1, TMAX):
                yield from gen_step(v, t)

        gens = [gen_stream(v) for v in range(NV)]
        live = list(gens)
        while live:
            nxt = []
            for g in live:
                try:
                    next(g)
                    nxt.append(g)
                except StopIteration:
                    pass
            live = nxt

    nc.compile()
    _BUILD_CACHE[key] = (nc, "out")
    return _BUILD_CACHE[key]


def _prep_core_inputs(inputs, core, Tn=T):
    f = np.float32
    b0 = core * BCORE
    x = np.ascontiguousarray(inputs["x"][b0:b0 + BCORE, :Tn, :]).astype(f)
    xT = (x.transpose(2, 1, 0).reshape(2, 128, Tn // 8, 8, NV, BC)
          .transpose(0, 1, 2, 4, 3, 5).reshape(2, 128, Tn // 8, NV, 8 * BC))
    xT = np.ascontiguousarray(xT)
    static = inputs["static"][b0:b0 + BCORE].astype(f)
    wih = inputs["gru_wih"].astype(f); whh = inputs["gru_whh"].astype(f)
    a1w1 = inputs["a1_w1"].astype(f); a2w1 = inputs["a2_w1"].astype(f)
    bih = inputs["gru_bih"].astype(f); bhh = inputs["gru_bhh"].astype(f)
    fusw = inputs["fus_w"].astype(f); fusb = inputs["fus_b"].astype(f)

    iotamb24 = np.zeros((128, 24, NR), f)
    delta24 = np.zeros((128, 24, NR), f)
    for p in range(128):
        g = p // BC
        for j in range(24):
            for r in range(NR):
                s = r * NG + g
                a = (s - j + 10) % NS
                if a < A:
                    iotamb24[p, j, r] = a - BIG
            s_new = (j - 1) % NS
            if s_new % NG == g:
                delta24[p, j, s_new // NG] = 2.0
    # iotaJ24[p,j,r,b] = iotamb24[p,j,r] on the diagonal b==p%BC, +BIG off
    iotaJ24 = np.full((128, 24, NR, BC), BIG, f)
    for p in range(128):
        iotaJ24[p, :, :, p % BC] = iotamb24[p]
    diag01 = np.zeros((128, BC), f)
    for p in range(128):
        diag01[p, p % BC] = 1.0
    dup32 = np.zeros((BC, 128), f)
    for b in range(BC):
        for g in range(NG):
            dup32[b, g * BC + b] = 1.0

    fus_statT = np.stack([
        (static[v * BC:(v + 1) * BC] @ fusw[:, H:].T + fusb).T for v in range(NV)
    ])
    staticrep = np.stack([
        np.tile(static[v * BC:(v + 1) * BC].T, (1, 8)) for v in range(NV)
    ])
    cur0 = static @ inputs["init_w"].astype(f).T + inputs["init_b"].astype(f)
    ubase = (static @ a1w1[:, H:].T).T                        # [U, 64]
    ubaseT = np.stack([ubase[:, v * BC:(v + 1) * BC].T for v in range(NV)])

    w1h10 = (a1w1[:, :H] / 10.0).T
    whhT = whh.T.copy()
    whhT[:, 0:2 * H] *= 2.0       # r,z gates see wh/2
    m = {
        "xT": xT,
        "wihT": np.ascontiguousarray(wih.T.reshape(2, 128, G3)),
        "whhT": np.ascontiguousarray(whhT),
        "w1h10T": np.ascontiguousarray(w1h10),
        "negw1h10T": np.ascontiguousarray(-w1h10),
        "w2b1": np.vstack([inputs["a1_w2"].astype(f).T,
                           inputs["a1_b2"].astype(f).reshape(1, A)]),
        "a2w1xT": np.ascontiguousarray(a2w1[:, :D].T.reshape(2, 128, U)),
        "a2w1sT": np.ascontiguousarray(a2w1[:, D:].T),
        "w2b2": np.vstack([inputs["a2_w2"].astype(f).T,
                           inputs["a2_b2"].astype(f).reshape(1, A)]),
        "fuswhT": np.ascontiguousarray(fusw[:, :H].T),
        "fus_statT": fus_statT,
        "staticrep": staticrep,
        "cur0T": np.ascontiguousarray(cur0.T) * 0.5,
        "ubaseT": ubaseT,
        "a1b1": inputs["a1_b1"].astype(f).reshape(U, 1),
        "a2b1": inputs["a2_b1"].astype(f).reshape(U, 1),
        "halfbr": (0.5 * (bih[:H] + bhh[:H])).reshape(H, 1),
        "halfbz": (0.5 * (bih[H:2 * H] + bhh[H:2 * H])).reshape(H, 1),
        "biasn": (bih[2 * H:] + bhh[2 * H:]).reshape(H, 1),
        "iotamb10": np.tile(np.arange(A, dtype=f) - BIG, (128, 1)),
        "iotamb80": np.tile(np.arange(A, dtype=f) - BIG, (128, 8)),
        "iotamb24": iotamb24,
        "delta24": delta24,
        "iotaJ24": iotaJ24,
        "diag01": diag01,
        "dup32": dup32,
        "eye32": np.eye(BC, dtype=f),
        "ident": np.eye(128, dtype=f),
    }
    return {k: np.ascontiguousarray(v, dtype=f) for k, v in m.items()}


def kernel(**inputs):
    from concourse.bass_utils import run_bass_kernel_spmd
    nc, _ = _build(T)
    in_maps = [_prep_core_inputs(inputs, c) for c in range(NCORES)]
    res = run_bass_kernel_spmd(nc, in_maps, core_ids=list(range(NCORES)))
    out = np.empty((B, T, H), np.float32)
    for c in range(NCORES):
        oc = res.results[c]["out"]
        out[c * BCORE:(c + 1) * BCORE] = oc.transpose(2, 1, 0)
    return out


# revision 5
# speedup vs baseline: 1.3402x; 1.0061x over previous
"""Trainium2 Bass kernel for the AgentLayer GRU-with-action-memory model, v5.

B=512 -> 8 cores x 64; two 32-batch streams per core, op-level zippered.
v5 chain restructure over v4 (v4 wall ~7.9us/step-pair, chain-latency bound):
  - obs-window sum lives in a persistent PSUM accumulator (uacc): per-step
    +W1h@h_new / -W1h@h_old matmuls (h-ring in SBUF, [H,NS,BC] layout);
    removes the DVE usum add/sub from the chain head.
  - argmax1 = Pool reduce-max + ONE Pool TSP with accum_out (masked-iota
    sum == first-max index since max is unique); was 3 DVE hops.
  - single 128-partition transposed ring -> 3 fold matmuls (was 6);
    qm built in ONE DVE TSP vs precomputed diag-masked patterns
    (iotaJunk24 / diagpre2 per block).
  - lambda scaling (0.25/0.5) folded into the whs copy (x0.125) with
    whh r,z columns x2 host-side; whs holds wh/2.
  - GRU tail: cur = z*wh + (1-z)*n via off-chain zc/a from thz ->
    only 2 hops after thn.
  - engine rebalance: PSUM-touching chain ops on Pool (idle in v4, no
    modeled PSUM access penalty), ring copy on Act, fusion static add as
    Pool TT (kills 1 matmul + Act copy per step).
"""

import numpy as np
from contextlib import ExitStack

B, T, D, H, S, A, U = 512, 256, 256, 128, 64, 10, 64
NCORES = 8
BCORE = B // NCORES   # 64 per core
NV = 2                # streams per core
BC = BCORE // NV      # 32 per stream
NG = 128 // BC        # partition groups = 4
NS = 12               # ring slots
NR = NS // NG         # ring rows = 3
G3 = 3 * H
BIG = 1024.0
AHEAD = 2

_BUILD_CACHE = {}


def _build(Tn):
    key = Tn
    if key in _BUILD_CACHE:
        return _BUILD_CACHE[key]

    import concourse.bass as bass
    import concourse.bacc as bacc
    import concourse.tile as tile
    from concourse import mybir

    f32 = mybir.dt.float32
    Alu = mybir.AluOpType
    Act = mybir.ActivationFunctionType
    Axis = mybir.AxisListType

    nc = bacc.Bacc("TRN2", target_bir_lowering=False, debug=False)

    d_in = {}

    def din(name, shape):
        d_in[name] = nc.dram_tensor(name, list(shape), f32, kind="ExternalInput").ap()
        return d_in[name]

    xT = din("xT", (2, 128, Tn // 8, NV, 8 * BC))
    wihT = din("wihT", (2, 128, G3))
    whhT = din("whhT", (H, G3))            # r,z cols x2 host-side
    w1h10T = din("w1h10T", (H, U))
    negw1h10T = din("negw1h10T", (H, U))
    w2b1 = din("w2b1", (U + 1, A))
    a2w1xT = din("a2w1xT", (2, 128, U))
    a2w1sT = din("a2w1sT", (S, U))
    w2b2 = din("w2b2", (U + 1, A))
    fuswhT = din("fuswhT", (H, H))
    fus_statT = din("fus_statT", (NV, H, BC))   # [H,BC] = (static@fus_s.T+b).T
    staticrep = din("staticrep", (NV, S, 8 * BC))
    cur0T = din("cur0T", (H, BCORE))       # 0.5 * initial h (wh/2 convention)
    ubaseT = din("ubaseT", (NV, BC, U))    # (W1s@static).T per stream
    a1b1 = din("a1b1", (U, 1))
    a2b1 = din("a2b1", (U, 1))
    halfbr = din("halfbr", (H, 1))
    halfbz = din("halfbz", (H, 1))
    biasn = din("biasn", (H, 1))
    iotamb10 = din("iotamb10", (128, A))
    iotamb80 = din("iotamb80", (128, 8 * A))
    iotamb24 = din("iotamb24", (128, 24, NR))
    delta24 = din("delta24", (128, 24, NR))
    iotaJ24 = din("iotaJ24", (128, 24, NR, BC))
    diag01 = din("diag01", (128, BC))
    dup32 = din("dup32", (BC, 128))
    eye32 = din("eye32", (BC, BC))
    ident = din("ident", (128, 128))

    out_d = nc.dram_tensor("out", [H, Tn, BCORE], f32, kind="ExternalOutput").ap()

    NBLK = Tn // 8
    assert Tn % 16 == 0

    with ExitStack() as ctx:
        tc = ctx.enter_context(tile.TileContext(nc))
        singles = ctx.enter_context(tc.tile_pool(name="singles", bufs=1))
        work = ctx.enter_context(tc.tile_pool(name="work", bufs=3))
        dpre = ctx.enter_context(tc.tile_pool(name="dpre", bufs=3))
        pstate = ctx.enter_context(tc.tile_pool(name="pstate", bufs=1, space="PSUM"))
        pwork = ctx.enter_context(tc.tile_pool(name="pwork", bufs=2, space="PSUM"))
        pA = ctx.enter_context(tc.tile_pool(name="pA", bufs=1, space="PSUM"))
        outsb = ctx.enter_context(tc.tile_pool(name="outsb", bufs=2))

        sb = {}
        for name, ap in d_in.items():
            if name == "xT":
                continue
            if name in ("wihT", "a2w1xT"):
                t = singles.tile([128, 2, ap.shape[2]], f32, tag=f"w_{name}",
                                 name=f"w_{name}")
                for c in range(2):
                    nc.sync.dma_start(out=t[:, c, :], in_=ap[c])
            elif name in ("fus_statT", "staticrep", "ubaseT"):
                t = singles.tile([ap.shape[1], NV, ap.shape[2]], f32,
                                 tag=f"w_{name}", name=f"w_{name}")
                for v in range(NV):
                    nc.sync.dma_start(out=t[:, v, :], in_=ap[v])
            else:
                t = singles.tile(list(ap.shape), f32, tag=f"w_{name}",
                                 name=f"w_{name}")
                nc.sync.dma_start(out=t, in_=ap)
            sb[name] = t

        xsb = singles.tile([128, 2, Tn // 8, NV, 8 * BC], f32, tag="xsb")
        # x lands in 16 range-DMAs so block-0 compute starts ~40us earlier
        NQB = 16
        for qb in range(NQB):
            qs = slice(qb * (Tn // 8) // NQB, (qb + 1) * (Tn // 8) // NQB)
            for c in range(2):
                nc.sync.dma_start(out=xsb[:, c, qs], in_=xT[c][:, qs])

        # ---- per-stream persistent state ----
        # ring2: h_t in natural [H, slot, BC] layout (matmul rhs for uacc)
        ring2 = [singles.tile([H, NS, BC], f32, tag=f"ring2_{v}", name=f"ring2_{v}")
                 for v in range(NV)]
        # ringT: transposed ring for folds: partition p = g*BC+b, row r,
        # slot s = r*NG + g, free = H
        ringT = [singles.tile([128, NR, H], f32, tag=f"ringT{v}", name=f"ringT{v}")
                 for v in range(NV)]
        cur0 = [singles.tile([H, BC], f32, tag=f"cur0{v}", name=f"cur0{v}")
                for v in range(NV)]
        u1t65 = [singles.tile([U + 1, 4 * BC], f32, tag=f"u1t{v}", name=f"u1t{v}")
                 for v in range(NV)]
        u2t65 = [singles.tile([U + 1, 8 * BC], f32, tag=f"u2t{v}", name=f"u2t{v}")
                 for v in range(NV)]

        # PSUM banks (8). start=True lazy-zeroes the WHOLE 2KB row of the
        # touched partitions (pending-zero); matmul writes consume pending
        # (fresh) else accumulate. So: bankU partitions 0:64 hold ONLY uacc;
        # gates bank has ONE start per step (xgates-c0) and every other
        # per-step matmul region (pl1/pwh/ptr/pout) rides the pending row
        # with start=False + skip_group_check (fresh-overwrite / accumulate).
        #  bankU[v] (x2): uacc [0:U, 0:BC] persistent accumulate
        #  bankS[v] (x2): phase-A only: pu2 [0:U, 0:256] (quarter-pumped),
        #                 pl2 [0:BC, 256:336], pdup [:, 336:344]
        #  gates[v] (x2 bufs x2): pr/pz/pin/phn [0:4BC], pren [4BC:5BC],
        #                 pout [5BC:6BC], pl1 [:, 6BC:6BC+10],
        #                 pwh [7BC:8BC], ptr [0:BC, 8BC:12BC]
        bankU = [pstate.tile([128, 512], f32, tag=f"bankU{v}", name=f"bankU{v}")
                 for v in range(NV)]
        bankS = [pA.tile([128, 512], f32, tag=f"bankS{v}", name=f"bankS{v}")
                 for v in range(NV)]

        dpre_tiles = [{} for _ in range(NV)]
        for v in range(NV):
            nc.vector.memset(ring2[v], 0.0)
            nc.vector.memset(ringT[v], 0.0)
            nc.vector.memset(u1t65[v], 1.0)
            nc.vector.memset(u2t65[v], 1.0)
            nc.sync.dma_start(out=cur0[v], in_=cur0T[:, v * BC:(v + 1) * BC])
            # uacc init = W1s@static (ubase): lhsT=[BC,U] rhs=eye32
            nc.tensor.matmul(bankU[v][0:U, 0:BC], sb["ubaseT"][:, v, :],
                             sb["eye32"], start=True, stop=True)

        out_tiles = [{} for _ in range(NV)]

        def bsl(v):
            return slice(v * BC, (v + 1) * BC)

        # ---------- phase A (argmax2 precompute per 8-step block) ----------
        def gen_phaseA(v, blk):
            t0 = blk * 8
            pu2 = bankS[v][0:U, 0:8 * BC]
            for q in range(4):
                qs = slice(q * 64, (q + 1) * 64)
                for c in range(2):
                    nc.tensor.matmul(pu2[:, qs], sb["a2w1xT"][:, c, :],
                                     xsb[:, c, blk, v, qs],
                                     start=(c == 0 and q == 0), stop=False)
                nc.tensor.matmul(pu2[:, qs], sb["a2w1sT"],
                                 sb["staticrep"][:, v, qs],
                                 start=False, stop=(q == 3))
                yield
            nc.scalar.activation(u2t65[v][0:U, :], pu2, Act.Tanh,
                                 bias=sb["a2b1"], scale=1.0)
            yield
            pl2 = bankS[v][0:BC, 256:336].rearrange("p (f a) -> p f a", a=A)
            for j in range(8):
                nc.tensor.matmul(pl2[:, j, :],
                                 u2t65[v][:, j * BC:(j + 1) * BC], sb["w2b2"],
                                 start=(j == 0), stop=(j == 7))
            yield
            rmax2 = work.tile([BC, 8], f32, tag=f"rmax2{v}", name=f"rmax2{v}_{blk}")
            nc.vector.tensor_reduce(out=rmax2, in_=pl2, axis=Axis.X, op=Alu.max)
            yield
            rmax2_b = bass.AP(tensor=rmax2.tensor, offset=rmax2.offset,
                              ap=[rmax2.ap[0], rmax2.ap[1], [0, A]])
            ge2 = work.tile([BC, 8, A], f32, tag=f"ge2{v}", name=f"ge2{v}_{blk}")
            nc.vector.tensor_tensor(out=ge2, in0=pl2, in1=rmax2_b, op=Alu.is_ge)
            yield
            iota_b = sb["iotamb80"][0:BC, :].rearrange("p (f a) -> p f a", a=A)
            mi2 = work.tile([BC, 8, A], f32, tag=f"mi2{v}", name=f"mi2{v}_{blk}")
            nc.gpsimd.tensor_tensor(out=mi2, in0=ge2, in1=iota_b, op=Alu.mult)
            yield
            idx2f = work.tile([BC, 8], f32, tag=f"idx2f{v}", name=f"idx2f{v}_{blk}")
            nc.vector.tensor_reduce(out=idx2f, in_=mi2, axis=Axis.X, op=Alu.min)
            yield
            pdup = bankS[v][:, 336:344]
            nc.tensor.matmul(pdup, sb["dup32"], idx2f, start=True, stop=True)
            yield
            pdup_b = bass.AP(tensor=pdup.tensor, offset=pdup.offset,
                             ap=[pdup.ap[0], pdup.ap[1], [0, NR]])
            tm = t0 % 24
            oh2 = work.tile([128, 8, NR], f32, tag=f"oh2{v}", name=f"oh2{v}_{blk}")
            nc.vector.tensor_tensor(out=oh2, in0=sb["iotamb24"][:, tm:tm + 8, :],
                                    in1=pdup_b, op=Alu.is_equal)
            yield
            pre2 = work.tile([128, 8, NR], f32, tag=f"pre2{v}", name=f"pre2{v}_{blk}")
            nc.vector.tensor_tensor(out=pre2, in0=oh2,
                                    in1=sb["delta24"][:, tm:tm + 8, :], op=Alu.add)
            yield
            # diagpre2[p,j,r,b] = pre2[p,j,r] * diag01[p,b]  (two half-j ops
            # so a pumped op never blocks the DVE chain for more than ~450ns)
            dp = dpre.tile([128, 8, NR, BC], f32, tag=f"dp{v}", name=f"dp{v}_{blk}")
            for jh in range(2):
                js = slice(jh * 4, (jh + 1) * 4)
                pre2_b = bass.AP(tensor=pre2.tensor,
                                 offset=pre2.offset + jh * 4 * NR,
                                 ap=[pre2.ap[0], [NR, 4], [1, NR], [0, BC]])
                diag_b = bass.AP(tensor=sb["diag01"].tensor,
                                 offset=sb["diag01"].offset,
                                 ap=[sb["diag01"].ap[0], [0, 4], [0, NR],
                                     sb["diag01"].ap[1]])
                nc.vector.tensor_tensor(out=dp[:, js], in0=pre2_b, in1=diag_b,
                                        op=Alu.mult)
                yield
            dpre_tiles[v][blk] = dp

        # gates bank layout: pr/pz/pin/phn [0:4BC], thr [4BC:5BC],
        # thz [5BC:6BC], pren [6BC:7BC], thn [7BC:8BC], pout [8BC:9BC]
        def mk_gates(v, t):
            return pwork.tile([128, 512], f32, tag=f"gat{v}", name=f"gat{v}_{t}")

        def gen_gru(v, t, wh_sb, gat):
            # wh_sb holds weighted_h / 2 (whh r,z cols are x2 host-side)
            pr = gat[:, 0:BC]
            pz = gat[:, BC:2 * BC]
            pin = gat[:, 2 * BC:3 * BC]
            phn = gat[:, 3 * BC:4 * BC]
            pren = gat[:, 4 * BC:5 * BC]
            pout = gat[:, 5 * BC:6 * BC]
            thr = work.tile([H, BC], f32, tag=f"thr{v}", name=f"thr{v}_{t}")
            thz = work.tile([H, BC], f32, tag=f"thz{v}", name=f"thz{v}_{t}")
            thn = work.tile([H, BC], f32, tag=f"thn{v}", name=f"thn{v}_{t}")
            # whh parts (need wh_sb; x parts were emitted earlier).
            # pr first: thr is the chain-critical activation.
            nc.tensor.matmul(pr, sb["whhT"][:, 0:H], wh_sb, start=False, stop=False)
            nc.tensor.matmul(phn, sb["whhT"][:, 2 * H:3 * H], wh_sb,
                             start=False, stop=False)
            yield
            nc.tensor.matmul(pz, sb["whhT"][:, H:2 * H], wh_sb, start=False,
                             stop=False, skip_group_check=True)
            yield
            nc.scalar.activation(thr, pr, Act.Tanh, bias=sb["halfbr"], scale=0.5)
            yield
            nc.scalar.activation(thz, pz, Act.Tanh, bias=sb["halfbz"], scale=0.5)
            yield
            # q = (thr+1)*phn_half = sigmoid(r)*phn; pren = q + pin
            # (one DVE block: same-engine deps run back-to-back)
            qsb = work.tile([H, BC], f32, tag=f"qsb{v}", name=f"qsb{v}_{t}")
            nc.vector.scalar_tensor_tensor(out=qsb, in0=thr, scalar=1.0, in1=phn,
                                           op0=Alu.add, op1=Alu.mult)
            nc.vector.tensor_tensor(out=pren, in0=qsb, in1=pin, op=Alu.add)
            yield
            zc = work.tile([H, BC], f32, tag=f"zc{v}", name=f"zc{v}_{t}")
            nc.scalar.activation(zc, thz, Act.Copy, bias=0.5, scale=-0.5)
            av = work.tile([H, BC], f32, tag=f"av{v}", name=f"av{v}_{t}")
            nc.vector.scalar_tensor_tensor(out=av, in0=thz, scalar=1.0, in1=wh_sb,
                                           op0=Alu.add, op1=Alu.mult)
            yield
            nc.scalar.activation(thn, pren, Act.Tanh, bias=sb["biasn"], scale=1.0)
            yield
            pump(v)
            bv = work.tile([H, BC], f32, tag=f"bv{v}", name=f"bv{v}_{t}")
            nc.gpsimd.tensor_tensor(out=bv, in0=zc, in1=thn, op=Alu.mult)
            cur = ring2[v][:, t % NS, :]
            nc.gpsimd.tensor_tensor(out=cur, in0=av, in1=bv, op=Alu.add)
            yield
            # next step's chain head immediately behind cur on PE/Act:
            # uacc window update (+c_t, -c_{t-10}) then u1 tanh
            if t + 1 < TMAX:
                uacc = bankU[v][0:U, 0:BC]
                nc.tensor.matmul(uacc, sb["w1h10T"], ring2[v][:, t % NS, :],
                                 start=False, stop=False, skip_group_check=True)
                nc.tensor.matmul(uacc, sb["negw1h10T"],
                                 ring2[v][:, (t - 10) % NS, :],
                                 start=False, stop=True, skip_group_check=True)
                u1out = u1t65[v][0:U, :].rearrange("p (d b) -> p d b", b=BC)
                uacc_b = bass.AP(tensor=uacc.tensor, offset=uacc.offset,
                                 ap=[uacc.ap[0], [0, 4], uacc.ap[1]])
                nc.scalar.activation(u1out, uacc_b, Act.Tanh,
                                     bias=sb["a1b1"], scale=1.0)
                yield
            # transposed ring write: ptr = cur.T (PE), then Act copy into ringT
            s_me = t % NS
            g_me, r_me = s_me % NG, s_me // NG
            ptr = gat[0:BC, 8 * BC:12 * BC]
            nc.tensor.matmul(ptr, cur, sb["ident"], is_transpose=True,
                             start=False, stop=False, skip_group_check=True)
            yield
            nc.scalar.copy(ringT[v][g_me * BC:(g_me + 1) * BC, r_me, :], ptr)
            yield
            pump(v)
            # fusion output (stop=True closes this step's gates-bank group)
            nc.tensor.matmul(pout, sb["fuswhT"], cur, start=False, stop=True,
                             skip_group_check=True)
            yield
            ob16 = t % 16
            if ob16 == 0:
                out_tiles[v][t // 16] = outsb.tile(
                    [H, 16, BC], f32, tag=f"osb{v}", name=f"osb{v}_{t // 16}")
            ot = out_tiles[v][t // 16]
            nc.vector.tensor_tensor(out=ot[:, ob16, :], in0=pout,
                                    in1=sb["fus_statT"][:, v, :], op=Alu.add)
            yield
            if ob16 == 15 or t == Tn - 1:
                nc.sync.dma_start(
                    out=out_d[:, t - ob16:t + 1, bsl(v)],
                    in_=ot[:, 0:ob16 + 1, :])
                del out_tiles[v][t // 16]

        def emit_xgates(v, t, gat):
            # one PSUM group for pr/pz/pin/phn: single start here (pr@c0),
            # single stop at the last whh matmul (pz) in gen_gru
            xx = xsb[:, :, t // 8, v, (t % 8) * BC:(t % 8) * BC + BC]
            for c in range(2):
                nc.tensor.matmul(gat[:, 0:BC], sb["wihT"][:, c, 0:H], xx[:, c, :],
                                 start=(c == 0), stop=False)
                nc.tensor.matmul(gat[:, BC:2 * BC], sb["wihT"][:, c, H:2 * H],
                                 xx[:, c, :], start=False, stop=False)
                yield
                nc.tensor.matmul(gat[:, 2 * BC:3 * BC], sb["wihT"][:, c, 2 * H:3 * H],
                                 xx[:, c, :], start=False, stop=False)
                yield

        def gen_step(v, t):
            if t % 8 == 0 and (t // 8 + AHEAD - 1) < NBLK:
                while pa_gen[v] is not None:   # should already be drained
                    pump(v)
                pa_gen[v] = gen_phaseA(v, t // 8 + AHEAD - 1)
            gat = mk_gates(v, t)
            tm = t % 24
            # uacc/u1 for this step were emitted in the previous step's tail.
            # x-gate matmuls first: they fill PE while u1 finishes on Act.
            yield from emit_xgates(v, t, gat)
            pl1 = gat[:, 6 * BC:6 * BC + A]
            nc.tensor.matmul(pl1, u1t65[v], sb["w2b1"], start=False, stop=False,
                             skip_group_check=True)
            yield
            # argmax1: rmax -> masked-iota-sum idx -> qm, all DVE, emitted as
            # one block so they run back-to-back (same-engine deps are free)
            rmax = work.tile([128, 1], f32, tag=f"rmax{v}", name=f"rmax{v}_{t}")
            nc.vector.tensor_reduce(out=rmax, in_=pl1, axis=Axis.X, op=Alu.max)
            junk = work.tile([128, A], f32, tag=f"junk{v}", name=f"junk{v}_{t}")
            idxf = work.tile([128, 1], f32, tag=f"idxf{v}", name=f"idxf{v}_{t}")
            nc.vector.scalar_tensor_tensor(out=junk, in0=pl1, scalar=rmax[:, 0:1],
                                           in1=sb["iotamb10"],
                                           op0=Alu.is_ge, op1=Alu.mult,
                                           accum_out=idxf)
            qm = work.tile([128, NR, BC], f32, tag=f"qm{v}", name=f"qm{v}_{t}")
            nc.vector.scalar_tensor_tensor(
                out=qm, in0=sb["iotaJ24"][:, tm, :, :], scalar=idxf[:, 0:1],
                in1=dpre_tiles[v][t // 8][:, t % 8, :, :],
                op0=Alu.is_equal, op1=Alu.add)
            yield
            pump(v)
            pwh = gat[:, 7 * BC:8 * BC]
            for r in range(NR):
                nc.tensor.matmul(pwh, ringT[v][:, r, :], qm[:, r, :],
                                 start=False, stop=False, skip_group_check=True)
            yield
            # whs = pwh * 0.125 = weighted_h / 2
            whs = work.tile([H, BC], f32, tag=f"whs{v}", name=f"whs{v}_{t}")
            nc.vector.tensor_scalar(out=whs, in0=pwh, scalar1=0.125, scalar2=None,
                                    op0=Alu.mult)
            yield
            yield from gen_gru(v, t, whs, gat)

        import os as _os
        TMAX = int(_os.environ.get("K5_TMAX", "0")) or Tn

        pa_gen = [None for _ in range(NV)]

        def pump(v):
            g = pa_gen[v]
            if g is not None:
                try:
                    next(g)
                except StopIteration:
                    pa_gen[v] = None

        def gen_stream(v):
            # stagger stream 1 by 19 zipper slots: measured-best cross-stream
            # phase (keeps B's DVE argmax burst out of A's chain hops)
            if v == 1:
                for _ in range(19):
                    yield
            for blk in range(AHEAD):
                yield from gen_phaseA(v, blk)
            gat0 = mk_gates(v, 0)
            yield from emit_xgates(v, 0, gat0)
            yield from gen_gru(v, 0, cur0[v], gat0)
            for t in range(1, TMAX):
                yield from gen_step(v, t)

        gens = [gen_stream(v) for v in range(NV)]
        live = list(gens)
        while live:
            nxt = []
            for g in live:
                try:
                    next(g)
                    nxt.append(g)
                except StopIteration:
                    pass
            live = nxt

    nc.compile()
    _BUILD_CACHE[key] = (nc, "out")
    return _BUILD_CACHE[key]


def _prep_core_inputs(inputs, core, Tn=T):
    f = np.float32
    b0 = core * BCORE
    x = np.ascontiguousarray(inputs["x"][b0:b0 + BCORE, :Tn, :]).astype(f)
    xT = (x.transpose(2, 1, 0).reshape(2, 128, Tn // 8, 8, NV, BC)
          .transpose(0, 1, 2, 4, 3, 5).reshape(2, 128, Tn // 8, NV, 8 * BC))
    xT = np.ascontiguousarray(xT)
    static = inputs["static"][b0:b0 + BCORE].astype(f)
    wih = inputs["gru_wih"].astype(f); whh = inputs["gru_whh"].astype(f)
    a1w1 = inputs["a1_w1"].astype(f); a2w1 = inputs["a2_w1"].astype(f)
    bih = inputs["gru_bih"].astype(f); bhh = inputs["gru_bhh"].astype(f)
    fusw = inputs["fus_w"].astype(f); fusb = inputs["fus_b"].astype(f)

    iotamb24 = np.zeros((128, 24, NR), f)
    delta24 = np.zeros((128, 24, NR), f)
    for p in range(128):
        g = p // BC
        for j in range(24):
            for r in range(NR):
                s = r * NG + g
                a = (s - j + 10) % NS
                if a < A:
                    iotamb24[p, j, r] = a - BIG
            s_new = (j - 1) % NS
            if s_new % NG == g:
                delta24[p, j, s_new // NG] = 2.0
    # iotaJ24[p,j,r,b] = iotamb24[p,j,r] on the diagonal b==p%BC, +BIG off
    iotaJ24 = np.full((128, 24, NR, BC), BIG, f)
    for p in range(128):
        iotaJ24[p, :, :, p % BC] = iotamb24[p]
    diag01 = np.zeros((128, BC), f)
    for p in range(128):
        diag01[p, p % BC] = 1.0
    dup32 = np.zeros((BC, 128), f)
    for b in range(BC):
        for g in range(NG):
            dup32[b, g * BC + b] = 1.0

    fus_statT = np.stack([
        (static[v * BC:(v + 1) * BC] @ fusw[:, H:].T + fusb).T for v in range(NV)
    ])
    staticrep = np.stack([
        np.tile(static[v * BC:(v + 1) * BC].T, (1, 8)) for v in range(NV)
    ])
    cur0 = static @ inputs["init_w"].astype(f).T + inputs["init_b"].astype(f)
    ubase = (static @ a1w1[:, H:].T).T                        # [U, 64]
    ubaseT = np.stack([ubase[:, v * BC:(v + 1) * BC].T for v in range(NV)])

    w1h10 = (a1w1[:, :H] / 10.0).T
    whhT = whh.T.copy()
    whhT[:, 0:2 * H] *= 2.0       # r,z gates see wh/2
    m = {
        "xT": xT,
        "wihT": np.ascontiguousarray(wih.T.reshape(2, 128, G3)),
        "whhT": np.ascontiguousarray(whhT),
        "w1h10T": np.ascontiguousarray(w1h10),
        "negw1h10T": np.ascontiguousarray(-w1h10),
        "w2b1": np.vstack([inputs["a1_w2"].astype(f).T,
                           inputs["a1_b2"].astype(f).reshape(1, A)]),
        "a2w1xT": np.ascontiguousarray(a2w1[:, :D].T.reshape(2, 128, U)),
        "a2w1sT": np.ascontiguousarray(a2w1[:, D:].T),
        "w2b2": np.vstack([inputs["a2_w2"].astype(f).T,
                           inputs["a2_b2"].astype(f).reshape(1, A)]),
        "fuswhT": np.ascontiguousarray(fusw[:, :H].T),
        "fus_statT": fus_statT,
        "staticrep": staticrep,
        "cur0T": np.ascontiguousarray(cur0.T) * 0.5,
        "ubaseT": ubaseT,
        "a1b1": inputs["a1_b1"].astype(f).reshape(U, 1),
        "a2b1": inputs["a2_b1"].astype(f).reshape(U, 1),
        "halfbr": (0.5 * (bih[:H] + bhh[:H])).reshape(H, 1),
        "halfbz": (0.5 * (bih[H:2 * H] + bhh[H:2 * H])).reshape(H, 1),
        "biasn": (bih[2 * H:] + bhh[2 * H:]).reshape(H, 1),
        "iotamb10": np.tile(np.arange(A, dtype=f) - BIG, (128, 1)),
        "iotamb80": np.tile(np.arange(A, dtype=f) - BIG, (128, 8)),
        "iotamb24": iotamb24,
        "delta24": delta24,
        "iotaJ24": iotaJ24,
        "diag01": diag01,
        "dup32": dup32,
        "eye32": np.eye(BC, dtype=f),
        "ident": np.eye(128, dtype=f),
    }
    return {k: np.ascontiguousarray(v, dtype=f) for k, v in m.items()}


def kernel(**inputs):
    from concourse.bass_utils import run_bass_kernel_spmd
    nc, _ = _build(T)
    in_maps = [_prep_core_inputs(inputs, c) for c in range(NCORES)]
    res = run_bass_kernel_spmd(nc, in_maps, core_ids=list(range(NCORES)))
    out = np.empty((B, T, H), np.float32)
    for c in range(NCORES):
        oc = res.results[c]["out"]
        out[c * BCORE:(c + 1) * BCORE] = oc.transpose(2, 1, 0)
    return out


# revision 6
# speedup vs baseline: 1.3406x; 1.0003x over previous
"""Trainium2 Bass kernel for the AgentLayer GRU-with-action-memory model, v5.

B=512 -> 8 cores x 64; two 32-batch streams per core, op-level zippered.
v5 chain restructure over v4 (v4 wall ~7.9us/step-pair, chain-latency bound):
  - obs-window sum lives in a persistent PSUM accumulator (uacc): per-step
    +W1h@h_new / -W1h@h_old matmuls (h-ring in SBUF, [H,NS,BC] layout);
    removes the DVE usum add/sub from the chain head.
  - argmax1 = Pool reduce-max + ONE Pool TSP with accum_out (masked-iota
    sum == first-max index since max is unique); was 3 DVE hops.
  - single 128-partition transposed ring -> 3 fold matmuls (was 6);
    qm built in ONE DVE TSP vs precomputed diag-masked patterns
    (iotaJunk24 / diagpre2 per block).
  - lambda scaling (0.25/0.5) folded into the whs copy (x0.125) with
    whh r,z columns x2 host-side; whs holds wh/2.
  - GRU tail: cur = z*wh + (1-z)*n via off-chain zc/a from thz ->
    only 2 hops after thn.
  - engine rebalance: PSUM-touching chain ops on Pool (idle in v4, no
    modeled PSUM access penalty), ring copy on Act, fusion static add as
    Pool TT (kills 1 matmul + Act copy per step).
"""

import numpy as np
from contextlib import ExitStack

B, T, D, H, S, A, U = 512, 256, 256, 128, 64, 10, 64
NCORES = 8
BCORE = B // NCORES   # 64 per core
NV = 2                # streams per core
BC = BCORE // NV      # 32 per stream
NG = 128 // BC        # partition groups = 4
NS = 12               # ring slots
NR = NS // NG         # ring rows = 3
G3 = 3 * H
BIG = 1024.0
AHEAD = 2

_BUILD_CACHE = {}


def _build(Tn):
    key = Tn
    if key in _BUILD_CACHE:
        return _BUILD_CACHE[key]

    import concourse.bass as bass
    import concourse.bacc as bacc
    import concourse.tile as tile
    from concourse import mybir

    f32 = mybir.dt.float32
    Alu = mybir.AluOpType
    Act = mybir.ActivationFunctionType
    Axis = mybir.AxisListType

    nc = bacc.Bacc("TRN2", target_bir_lowering=False, debug=False)

    d_in = {}

    def din(name, shape):
        d_in[name] = nc.dram_tensor(name, list(shape), f32, kind="ExternalInput").ap()
        return d_in[name]

    xT = din("xT", (2, 128, Tn // 8, NV, 8 * BC))
    wihT = din("wihT", (2, 128, G3))
    whhT = din("whhT", (H, G3))            # r,z cols x2 host-side
    w1h10T = din("w1h10T", (H, U))
    negw1h10T = din("negw1h10T", (H, U))
    w2b1 = din("w2b1", (U + 1, A))
    a2w1xT = din("a2w1xT", (2, 128, U))
    a2w1sT = din("a2w1sT", (S, U))
    w2b2 = din("w2b2", (U + 1, A))
    fuswhT = din("fuswhT", (H, H))
    fus_statT = din("fus_statT", (NV, H, BC))   # [H,BC] = (static@fus_s.T+b).T
    staticrep = din("staticrep", (NV, S, 8 * BC))
    cur0T = din("cur0T", (H, BCORE))       # 0.5 * initial h (wh/2 convention)
    ubaseT = din("ubaseT", (NV, BC, U))    # (W1s@static).T per stream
    a1b1 = din("a1b1", (U, 1))
    a2b1 = din("a2b1", (U, 1))
    halfbr = din("halfbr", (H, 1))
    halfbz = din("halfbz", (H, 1))
    biasn = din("biasn", (H, 1))
    iotamb10 = din("iotamb10", (128, A))
    iotamb80 = din("iotamb80", (128, 8 * A))
    iotamb24 = din("iotamb24", (128, 24, NR))
    delta24 = din("delta24", (128, 24, NR))
    iotaJ24 = din("iotaJ24", (128, 24, NR, BC))
    diag01 = din("diag01", (128, BC))
    dup32 = din("dup32", (BC, 128))
    eye32 = din("eye32", (BC, BC))
    ident = din("ident", (128, 128))

    out_d = nc.dram_tensor("out", [H, Tn, BCORE], f32, kind="ExternalOutput").ap()

    NBLK = Tn // 8
    assert Tn % 16 == 0

    with ExitStack() as ctx:
        tc = ctx.enter_context(tile.TileContext(nc))
        singles = ctx.enter_context(tc.tile_pool(name="singles", bufs=1))
        work = ctx.enter_context(tc.tile_pool(name="work", bufs=3))
        dpre = ctx.enter_context(tc.tile_pool(name="dpre", bufs=3))
        pstate = ctx.enter_context(tc.tile_pool(name="pstate", bufs=1, space="PSUM"))
        pwork = ctx.enter_context(tc.tile_pool(name="pwork", bufs=2, space="PSUM"))
        pA = ctx.enter_context(tc.tile_pool(name="pA", bufs=1, space="PSUM"))
        outsb = ctx.enter_context(tc.tile_pool(name="outsb", bufs=2))

        sb = {}
        for name, ap in d_in.items():
            if name == "xT":
                continue
            if name in ("wihT", "a2w1xT"):
                t = singles.tile([128, 2, ap.shape[2]], f32, tag=f"w_{name}",
                                 name=f"w_{name}")
                for c in range(2):
                    nc.sync.dma_start(out=t[:, c, :], in_=ap[c])
            elif name in ("fus_statT", "staticrep", "ubaseT"):
                t = singles.tile([ap.shape[1], NV, ap.shape[2]], f32,
                                 tag=f"w_{name}", name=f"w_{name}")
                for v in range(NV):
                    nc.sync.dma_start(out=t[:, v, :], in_=ap[v])
            else:
                t = singles.tile(list(ap.shape), f32, tag=f"w_{name}",
                                 name=f"w_{name}")
                nc.sync.dma_start(out=t, in_=ap)
            sb[name] = t

        xsb = singles.tile([128, 2, Tn // 8, NV, 8 * BC], f32, tag="xsb")
        # x lands in 16 range-DMAs so block-0 compute starts ~40us earlier
        NQB = 16
        for qb in range(NQB):
            qs = slice(qb * (Tn // 8) // NQB, (qb + 1) * (Tn // 8) // NQB)
            for c in range(2):
                nc.sync.dma_start(out=xsb[:, c, qs], in_=xT[c][:, qs])

        # ---- per-stream persistent state ----
        # ring2: h_t in natural [H, slot, BC] layout (matmul rhs for uacc)
        ring2 = [singles.tile([H, NS, BC], f32, tag=f"ring2_{v}", name=f"ring2_{v}")
                 for v in range(NV)]
        # ringT: transposed ring for folds: partition p = g*BC+b, row r,
        # slot s = r*NG + g, free = H
        ringT = [singles.tile([128, NR, H], f32, tag=f"ringT{v}", name=f"ringT{v}")
                 for v in range(NV)]
        cur0 = [singles.tile([H, BC], f32, tag=f"cur0{v}", name=f"cur0{v}")
                for v in range(NV)]
        u1t65 = [singles.tile([U + 1, 4 * BC], f32, tag=f"u1t{v}", name=f"u1t{v}")
                 for v in range(NV)]
        u2t65 = [singles.tile([U + 1, 8 * BC], f32, tag=f"u2t{v}", name=f"u2t{v}")
                 for v in range(NV)]

        # PSUM banks (8). start=True lazy-zeroes the WHOLE 2KB row of the
        # touched partitions (pending-zero); matmul writes consume pending
        # (fresh) else accumulate. So: bankU partitions 0:64 hold ONLY uacc;
        # gates bank has ONE start per step (xgates-c0) and every other
        # per-step matmul region (pl1/pwh/ptr/pout) rides the pending row
        # with start=False + skip_group_check (fresh-overwrite / accumulate).
        #  bankU[v] (x2): uacc [0:U, 0:BC] persistent accumulate
        #  bankS[v] (x2): phase-A only: pu2 [0:U, 0:256] (quarter-pumped),
        #                 pl2 [0:BC, 256:336], pdup [:, 336:344]
        #  gates[v] (x2 bufs x2): pr/pz/pin/phn [0:4BC], pren [4BC:5BC],
        #                 pout [5BC:6BC], pl1 [:, 6BC:6BC+10],
        #                 pwh [7BC:8BC], ptr [0:BC, 8BC:12BC]
        bankU = [pstate.tile([128, 512], f32, tag=f"bankU{v}", name=f"bankU{v}")
                 for v in range(NV)]
        bankS = [pA.tile([128, 512], f32, tag=f"bankS{v}", name=f"bankS{v}")
                 for v in range(NV)]

        dpre_tiles = [{} for _ in range(NV)]
        for v in range(NV):
            nc.vector.memset(ring2[v], 0.0)
            nc.vector.memset(ringT[v], 0.0)
            nc.vector.memset(u1t65[v], 1.0)
            nc.vector.memset(u2t65[v], 1.0)
            nc.sync.dma_start(out=cur0[v], in_=cur0T[:, v * BC:(v + 1) * BC])
            # uacc init = W1s@static (ubase): lhsT=[BC,U] rhs=eye32
            nc.tensor.matmul(bankU[v][0:U, 0:BC], sb["ubaseT"][:, v, :],
                             sb["eye32"], start=True, stop=True)

        out_tiles = [{} for _ in range(NV)]

        def bsl(v):
            return slice(v * BC, (v + 1) * BC)

        # ---------- phase A (argmax2 precompute per 8-step block) ----------
        def gen_phaseA(v, blk):
            t0 = blk * 8
            pu2 = bankS[v][0:U, 0:8 * BC]
            for q in range(4):
                qs = slice(q * 64, (q + 1) * 64)
                for c in range(2):
                    nc.tensor.matmul(pu2[:, qs], sb["a2w1xT"][:, c, :],
                                     xsb[:, c, blk, v, qs],
                                     start=(c == 0 and q == 0), stop=False)
                nc.tensor.matmul(pu2[:, qs], sb["a2w1sT"],
                                 sb["staticrep"][:, v, qs],
                                 start=False, stop=(q == 3))
                yield
            nc.scalar.activation(u2t65[v][0:U, :], pu2, Act.Tanh,
                                 bias=sb["a2b1"], scale=1.0)
            yield
            pl2 = bankS[v][0:BC, 256:336].rearrange("p (f a) -> p f a", a=A)
            for j in range(8):
                nc.tensor.matmul(pl2[:, j, :],
                                 u2t65[v][:, j * BC:(j + 1) * BC], sb["w2b2"],
                                 start=(j == 0), stop=(j == 7))
            yield
            rmax2 = work.tile([BC, 8], f32, tag=f"rmax2{v}", name=f"rmax2{v}_{blk}")
            nc.vector.tensor_reduce(out=rmax2, in_=pl2, axis=Axis.X, op=Alu.max)
            yield
            rmax2_b = bass.AP(tensor=rmax2.tensor, offset=rmax2.offset,
                              ap=[rmax2.ap[0], rmax2.ap[1], [0, A]])
            ge2 = work.tile([BC, 8, A], f32, tag=f"ge2{v}", name=f"ge2{v}_{blk}")
            nc.vector.tensor_tensor(out=ge2, in0=pl2, in1=rmax2_b, op=Alu.is_ge)
            yield
            iota_b = sb["iotamb80"][0:BC, :].rearrange("p (f a) -> p f a", a=A)
            mi2 = work.tile([BC, 8, A], f32, tag=f"mi2{v}", name=f"mi2{v}_{blk}")
            nc.gpsimd.tensor_tensor(out=mi2, in0=ge2, in1=iota_b, op=Alu.mult)
            yield
            idx2f = work.tile([BC, 8], f32, tag=f"idx2f{v}", name=f"idx2f{v}_{blk}")
            nc.vector.tensor_reduce(out=idx2f, in_=mi2, axis=Axis.X, op=Alu.min)
            yield
            pdup = bankS[v][:, 336:344]
            nc.tensor.matmul(pdup, sb["dup32"], idx2f, start=True, stop=True)
            yield
            pdup_b = bass.AP(tensor=pdup.tensor, offset=pdup.offset,
                             ap=[pdup.ap[0], pdup.ap[1], [0, NR]])
            tm = t0 % 24
            oh2 = work.tile([128, 8, NR], f32, tag=f"oh2{v}", name=f"oh2{v}_{blk}")
            nc.vector.tensor_tensor(out=oh2, in0=sb["iotamb24"][:, tm:tm + 8, :],
                                    in1=pdup_b, op=Alu.is_equal)
            yield
            pre2 = work.tile([128, 8, NR], f32, tag=f"pre2{v}", name=f"pre2{v}_{blk}")
            nc.vector.tensor_tensor(out=pre2, in0=oh2,
                                    in1=sb["delta24"][:, tm:tm + 8, :], op=Alu.add)
            yield
            # diagpre2[p,j,r,b] = pre2[p,j,r] * diag01[p,b]  (two half-j ops
            # so a pumped op never blocks the DVE chain for more than ~450ns)
            dp = dpre.tile([128, 8, NR, BC], f32, tag=f"dp{v}", name=f"dp{v}_{blk}")
            for jh in range(2):
                js = slice(jh * 4, (jh + 1) * 4)
                pre2_b = bass.AP(tensor=pre2.tensor,
                                 offset=pre2.offset + jh * 4 * NR,
                                 ap=[pre2.ap[0], [NR, 4], [1, NR], [0, BC]])
                diag_b = bass.AP(tensor=sb["diag01"].tensor,
                                 offset=sb["diag01"].offset,
                                 ap=[sb["diag01"].ap[0], [0, 4], [0, NR],
                                     sb["diag01"].ap[1]])
                nc.vector.tensor_tensor(out=dp[:, js], in0=pre2_b, in1=diag_b,
                                        op=Alu.mult)
                yield
            dpre_tiles[v][blk] = dp

        # gates bank layout: pr/pz/pin/phn [0:4BC], thr [4BC:5BC],
        # thz [5BC:6BC], pren [6BC:7BC], thn [7BC:8BC], pout [8BC:9BC]
        def mk_gates(v, t):
            return pwork.tile([128, 512], f32, tag=f"gat{v}", name=f"gat{v}_{t}")

        def gen_gru(v, t, wh_sb, gat):
            # wh_sb holds weighted_h / 2 (whh r,z cols are x2 host-side)
            pr = gat[:, 0:BC]
            pz = gat[:, BC:2 * BC]
            pin = gat[:, 2 * BC:3 * BC]
            phn = gat[:, 3 * BC:4 * BC]
            pren = gat[:, 4 * BC:5 * BC]
            pout = gat[:, 5 * BC:6 * BC]
            thr = work.tile([H, BC], f32, tag=f"thr{v}", name=f"thr{v}_{t}")
            thz = work.tile([H, BC], f32, tag=f"thz{v}", name=f"thz{v}_{t}")
            thn = work.tile([H, BC], f32, tag=f"thn{v}", name=f"thn{v}_{t}")
            # whh parts (need wh_sb; x parts were emitted earlier).
            # pr first: thr is the chain-critical activation.
            nc.tensor.matmul(pr, sb["whhT"][:, 0:H], wh_sb, start=False, stop=False)
            nc.tensor.matmul(phn, sb["whhT"][:, 2 * H:3 * H], wh_sb,
                             start=False, stop=False)
            yield
            nc.tensor.matmul(pz, sb["whhT"][:, H:2 * H], wh_sb, start=False,
                             stop=False, skip_group_check=True)
            yield
            nc.scalar.activation(thr, pr, Act.Tanh, bias=sb["halfbr"], scale=0.5)
            yield
            nc.scalar.activation(thz, pz, Act.Tanh, bias=sb["halfbz"], scale=0.5)
            yield
            # q = (thr+1)*phn_half = sigmoid(r)*phn; pren = q + pin
            # (one DVE block: same-engine deps run back-to-back)
            qsb = work.tile([H, BC], f32, tag=f"qsb{v}", name=f"qsb{v}_{t}")
            nc.vector.scalar_tensor_tensor(out=qsb, in0=thr, scalar=1.0, in1=phn,
                                           op0=Alu.add, op1=Alu.mult)
            nc.vector.tensor_tensor(out=pren, in0=qsb, in1=pin, op=Alu.add)
            yield
            zc = work.tile([H, BC], f32, tag=f"zc{v}", name=f"zc{v}_{t}")
            nc.scalar.activation(zc, thz, Act.Copy, bias=0.5, scale=-0.5)
            av = work.tile([H, BC], f32, tag=f"av{v}", name=f"av{v}_{t}")
            nc.vector.scalar_tensor_tensor(out=av, in0=thz, scalar=1.0, in1=wh_sb,
                                           op0=Alu.add, op1=Alu.mult)
            yield
            nc.scalar.activation(thn, pren, Act.Tanh, bias=sb["biasn"], scale=1.0)
            yield
            pump(v)
            bv = work.tile([H, BC], f32, tag=f"bv{v}", name=f"bv{v}_{t}")
            nc.gpsimd.tensor_tensor(out=bv, in0=zc, in1=thn, op=Alu.mult)
            cur = ring2[v][:, t % NS, :]
            nc.gpsimd.tensor_tensor(out=cur, in0=av, in1=bv, op=Alu.add)
            yield
            # next step's chain head immediately behind cur on PE/Act:
            # uacc window update (+c_t, -c_{t-10}) then u1 tanh
            if t + 1 < TMAX:
                uacc = bankU[v][0:U, 0:BC]
                nc.tensor.matmul(uacc, sb["w1h10T"], ring2[v][:, t % NS, :],
                                 start=False, stop=False, skip_group_check=True)
                nc.tensor.matmul(uacc, sb["negw1h10T"],
                                 ring2[v][:, (t - 10) % NS, :],
                                 start=False, stop=True, skip_group_check=True)
                u1out = u1t65[v][0:U, :].rearrange("p (d b) -> p d b", b=BC)
                uacc_b = bass.AP(tensor=uacc.tensor, offset=uacc.offset,
                                 ap=[uacc.ap[0], [0, 4], uacc.ap[1]])
                nc.scalar.activation(u1out, uacc_b, Act.Tanh,
                                     bias=sb["a1b1"], scale=1.0)
                yield
            # transposed ring write: ptr = cur.T (PE), then Act copy into ringT
            s_me = t % NS
            g_me, r_me = s_me % NG, s_me // NG
            ptr = gat[0:BC, 8 * BC:12 * BC]
            nc.tensor.matmul(ptr, cur, sb["ident"], is_transpose=True,
                             start=False, stop=False, skip_group_check=True)
            yield
            nc.scalar.copy(ringT[v][g_me * BC:(g_me + 1) * BC, r_me, :], ptr)
            yield
            pump(v)
            # fusion output (stop=True closes this step's gates-bank group)
            nc.tensor.matmul(pout, sb["fuswhT"], cur, start=False, stop=True,
                             skip_group_check=True)
            yield
            ob16 = t % 16
            if ob16 == 0:
                out_tiles[v][t // 16] = outsb.tile(
                    [H, 16, BC], f32, tag=f"osb{v}", name=f"osb{v}_{t // 16}")
            ot = out_tiles[v][t // 16]
            nc.vector.tensor_tensor(out=ot[:, ob16, :], in0=pout,
                                    in1=sb["fus_statT"][:, v, :], op=Alu.add)
            yield
            if ob16 == 15 or t == Tn - 1:
                nc.sync.dma_start(
                    out=out_d[:, t - ob16:t + 1, bsl(v)],
                    in_=ot[:, 0:ob16 + 1, :])
                del out_tiles[v][t // 16]

        def emit_xgates(v, t, gat):
            # one PSUM group for pr/pz/pin/phn: single start here (pr@c0),
            # single stop at the last whh matmul (pz) in gen_gru
            xx = xsb[:, :, t // 8, v, (t % 8) * BC:(t % 8) * BC + BC]
            for c in range(2):
                nc.tensor.matmul(gat[:, 0:BC], sb["wihT"][:, c, 0:H], xx[:, c, :],
                                 start=(c == 0), stop=False)
                nc.tensor.matmul(gat[:, BC:2 * BC], sb["wihT"][:, c, H:2 * H],
                                 xx[:, c, :], start=False, stop=False)
                yield
                nc.tensor.matmul(gat[:, 2 * BC:3 * BC], sb["wihT"][:, c, 2 * H:3 * H],
                                 xx[:, c, :], start=False, stop=False)
                yield

        def gen_step(v, t):
            if t % 8 == 0 and (t // 8 + AHEAD - 1) < NBLK:
                while pa_gen[v] is not None:   # should already be drained
                    pump(v)
                pa_gen[v] = gen_phaseA(v, t // 8 + AHEAD - 1)
            gat = mk_gates(v, t)
            tm = t % 24
            # uacc/u1 for this step were emitted in the previous step's tail.
            # x-gate matmuls first: they fill PE while u1 finishes on Act.
            yield from emit_xgates(v, t, gat)
            pl1 = gat[:, 6 * BC:6 * BC + A]
            nc.tensor.matmul(pl1, u1t65[v], sb["w2b1"], start=False, stop=False,
                             skip_group_check=True)
            yield
            # argmax1: rmax -> masked-iota-sum idx -> qm, all DVE, emitted as
            # one block so they run back-to-back (same-engine deps are free)
            rmax = work.tile([128, 1], f32, tag=f"rmax{v}", name=f"rmax{v}_{t}")
            nc.vector.tensor_reduce(out=rmax, in_=pl1, axis=Axis.X, op=Alu.max)
            junk = work.tile([128, A], f32, tag=f"junk{v}", name=f"junk{v}_{t}")
            idxf = work.tile([128, 1], f32, tag=f"idxf{v}", name=f"idxf{v}_{t}")
            nc.vector.scalar_tensor_tensor(out=junk, in0=pl1, scalar=rmax[:, 0:1],
                                           in1=sb["iotamb10"],
                                           op0=Alu.is_ge, op1=Alu.mult,
                                           accum_out=idxf)
            qm = work.tile([128, NR, BC], f32, tag=f"qm{v}", name=f"qm{v}_{t}")
            nc.vector.scalar_tensor_tensor(
                out=qm, in0=sb["iotaJ24"][:, tm, :, :], scalar=idxf[:, 0:1],
                in1=dpre_tiles[v][t // 8][:, t % 8, :, :],
                op0=Alu.is_equal, op1=Alu.add)
            yield
            pump(v)
            pwh = gat[:, 7 * BC:8 * BC]
            for r in range(NR):
                nc.tensor.matmul(pwh, ringT[v][:, r, :], qm[:, r, :],
                                 start=False, stop=False, skip_group_check=True)
            yield
            # whs = pwh * 0.125 = weighted_h / 2
            whs = work.tile([H, BC], f32, tag=f"whs{v}", name=f"whs{v}_{t}")
            nc.vector.tensor_scalar(out=whs, in0=pwh, scalar1=0.125, scalar2=None,
                                    op0=Alu.mult)
            yield
            yield from gen_gru(v, t, whs, gat)

        import os as _os
        TMAX = int(_os.environ.get("K5_TMAX", "0")) or Tn

        pa_gen = [None for _ in range(NV)]

        def pump(v):
            g = pa_gen[v]
            if g is not None:
                try:
                    next(g)
                except StopIteration:
                    pa_gen[v] = None

        def gen_stream(v):
            # stagger stream 1 by 19 zipper slots: measured-best cross-stream
            # phase (keeps B's DVE argmax burst out of A's chain hops)
            if v == 1:
                for _ in range(19):
                    yield
            yield from gen_phaseA(v, 0)
            pa_gen[v] = gen_phaseA(v, 1)   # drained by the step pumps
            gat0 = mk_gates(v, 0)
            yield from emit_xgates(v, 0, gat0)
            yield from gen_gru(v, 0, cur0[v], gat0)
            for t in range(1, TMAX):
                yield from gen_step(v, t)

        gens = [gen_stream(v) for v in range(NV)]
        live = list(gens)
        while live:
            nxt = []
            for g in live:
                try:
                    next(g)
                    nxt.append(g)
                except StopIteration:
                    pass
            live = nxt

    nc.compile()
    _BUILD_CACHE[key] = (nc, "out")
    return _BUILD_CACHE[key]


def _prep_core_inputs(inputs, core, Tn=T):
    f = np.float32
    b0 = core * BCORE
    x = np.ascontiguousarray(inputs["x"][b0:b0 + BCORE, :Tn, :]).astype(f)
    xT = (x.transpose(2, 1, 0).reshape(2, 128, Tn // 8, 8, NV, BC)
          .transpose(0, 1, 2, 4, 3, 5).reshape(2, 128, Tn // 8, NV, 8 * BC))
    xT = np.ascontiguousarray(xT)
    static = inputs["static"][b0:b0 + BCORE].astype(f)
    wih = inputs["gru_wih"].astype(f); whh = inputs["gru_whh"].astype(f)
    a1w1 = inputs["a1_w1"].astype(f); a2w1 = inputs["a2_w1"].astype(f)
    bih = inputs["gru_bih"].astype(f); bhh = inputs["gru_bhh"].astype(f)
    fusw = inputs["fus_w"].astype(f); fusb = inputs["fus_b"].astype(f)

    iotamb24 = np.zeros((128, 24, NR), f)
    delta24 = np.zeros((128, 24, NR), f)
    for p in range(128):
        g = p // BC
        for j in range(24):
            for r in range(NR):
                s = r * NG + g
                a = (s - j + 10) % NS
                if a < A:
                    iotamb24[p, j, r] = a - BIG
            s_new = (j - 1) % NS
            if s_new % NG == g:
                delta24[p, j, s_new // NG] = 2.0
    # iotaJ24[p,j,r,b] = iotamb24[p,j,r] on the diagonal b==p%BC, +BIG off
    iotaJ24 = np.full((128, 24, NR, BC), BIG, f)
    for p in range(128):
        iotaJ24[p, :, :, p % BC] = iotamb24[p]
    diag01 = np.zeros((128, BC), f)
    for p in range(128):
        diag01[p, p % BC] = 1.0
    dup32 = np.zeros((BC, 128), f)
    for b in range(BC):
        for g in range(NG):
            dup32[b, g * BC + b] = 1.0

    fus_statT = np.stack([
        (static[v * BC:(v + 1) * BC] @ fusw[:, H:].T + fusb).T for v in range(NV)
    ])
    staticrep = np.stack([
        np.tile(static[v * BC:(v + 1) * BC].T, (1, 8)) for v in range(NV)
    ])
    cur0 = static @ inputs["init_w"].astype(f).T + inputs["init_b"].astype(f)
    ubase = (static @ a1w1[:, H:].T).T                        # [U, 64]
    ubaseT = np.stack([ubase[:, v * BC:(v + 1) * BC].T for v in range(NV)])

    w1h10 = (a1w1[:, :H] / 10.0).T
    whhT = whh.T.copy()
    whhT[:, 0:2 * H] *= 2.0       # r,z gates see wh/2
    m = {
        "xT": xT,
        "wihT": np.ascontiguousarray(wih.T.reshape(2, 128, G3)),
        "whhT": np.ascontiguousarray(whhT),
        "w1h10T": np.ascontiguousarray(w1h10),
        "negw1h10T": np.ascontiguousarray(-w1h10),
        "w2b1": np.vstack([inputs["a1_w2"].astype(f).T,
                           inputs["a1_b2"].astype(f).reshape(1, A)]),
        "a2w1xT": np.ascontiguousarray(a2w1[:, :D].T.reshape(2, 128, U)),
        "a2w1sT": np.ascontiguousarray(a2w1[:, D:].T),
        "w2b2": np.vstack([inputs["a2_w2"].astype(f).T,
                           inputs["a2_b2"].astype(f).reshape(1, A)]),
        "fuswhT": np.ascontiguousarray(fusw[:, :H].T),
        "fus_statT": fus_statT,
        "staticrep": staticrep,
        "cur0T": np.ascontiguousarray(cur0.T) * 0.5,
        "ubaseT": ubaseT,
        "a1b1": inputs["a1_b1"].astype(f).reshape(U, 1),
        "a2b1": inputs["a2_b1"].astype(f).reshape(U, 1),
        "halfbr": (0.5 * (bih[:H] + bhh[:H])).reshape(H, 1),
        "halfbz": (0.5 * (bih[H:2 * H] + bhh[H:2 * H])).reshape(H, 1),
        "biasn": (bih[2 * H:] + bhh[2 * H:]).reshape(H, 1),
        "iotamb10": np.tile(np.arange(A, dtype=f) - BIG, (128, 1)),
        "iotamb80": np.tile(np.arange(A, dtype=f) - BIG, (128, 8)),
        "iotamb24": iotamb24,
        "delta24": delta24,
        "iotaJ24": iotaJ24,
        "diag01": diag01,
        "dup32": dup32,
        "eye32": np.eye(BC, dtype=f),
        "ident": np.eye(128, dtype=f),
    }
    return {k: np.ascontiguousarray(v, dtype=f) for k, v in m.items()}


def kernel(**inputs):
    from concourse.bass_utils import run_bass_kernel_spmd
    nc, _ = _build(T)
    in_maps = [_prep_core_inputs(inputs, c) for c in range(NCORES)]
    res = run_bass_kernel_spmd(nc, in_maps, core_ids=list(range(NCORES)))
    out = np.empty((B, T, H), np.float32)
    for c in range(NCORES):
        oc = res.results[c]["out"]
        out[c * BCORE:(c + 1) * BCORE] = oc.transpose(2, 1, 0)
    return out


# revision 9
# speedup vs baseline: 1.3921x; 1.0384x over previous
"""Trainium2 Bass kernel for the AgentLayer GRU-with-action-memory model, v5.

B=512 -> 8 cores x 64; two 32-batch streams per core, op-level zippered.
v5 chain restructure over v4 (v4 wall ~7.9us/step-pair, chain-latency bound):
  - obs-window sum lives in a persistent PSUM accumulator (uacc): per-step
    +W1h@h_new / -W1h@h_old matmuls (h-ring in SBUF, [H,NS,BC] layout);
    removes the DVE usum add/sub from the chain head.
  - argmax1 = Pool reduce-max + ONE Pool TSP with accum_out (masked-iota
    sum == first-max index since max is unique); was 3 DVE hops.
  - single 128-partition transposed ring -> 3 fold matmuls (was 6);
    qm built in ONE DVE TSP vs precomputed diag-masked patterns
    (iotaJunk24 / diagpre2 per block).
  - lambda scaling (0.25/0.5) folded into the whs copy (x0.125) with
    whh r,z columns x2 host-side; whs holds wh/2.
  - GRU tail: cur = z*wh + (1-z)*n via off-chain zc/a from thz ->
    only 2 hops after thn.
  - engine rebalance: PSUM-touching chain ops on Pool (idle in v4, no
    modeled PSUM access penalty), ring copy on Act, fusion static add as
    Pool TT (kills 1 matmul + Act copy per step).
"""

import numpy as np
from contextlib import ExitStack

B, T, D, H, S, A, U = 512, 256, 256, 128, 64, 10, 64
NCORES = 8
BCORE = B // NCORES   # 64 per core
NV = 2                # streams per core
BC = BCORE // NV      # 32 per stream
NG = 128 // BC        # partition groups = 4
NS = 12               # ring slots
NR = NS // NG         # ring rows = 3
G3 = 3 * H
BIG = 1024.0
AHEAD = 2

_BUILD_CACHE = {}


def _build(Tn):
    key = Tn
    if key in _BUILD_CACHE:
        return _BUILD_CACHE[key]

    import concourse.bass as bass
    import concourse.bacc as bacc
    import concourse.tile as tile
    from concourse import mybir

    f32 = mybir.dt.float32
    Alu = mybir.AluOpType
    Act = mybir.ActivationFunctionType
    Axis = mybir.AxisListType

    nc = bacc.Bacc("TRN2", target_bir_lowering=False, debug=False)

    d_in = {}

    def din(name, shape):
        d_in[name] = nc.dram_tensor(name, list(shape), f32, kind="ExternalInput").ap()
        return d_in[name]

    xT = din("xT", (2, 128, Tn // 8, NV, 8 * BC))
    wihT = din("wihT", (2, 128, G3))
    whhT = din("whhT", (H, G3))            # r,z cols x2 host-side
    w1h10T = din("w1h10T", (H, U))
    negw1h10T = din("negw1h10T", (H, U))
    w2b1 = din("w2b1", (U + 1, A))
    a2w1xT = din("a2w1xT", (2, 128, U))
    a2w1sT = din("a2w1sT", (S, U))
    w2b2 = din("w2b2", (U + 1, A))
    fuswhT = din("fuswhT", (H, H))
    fus_statT = din("fus_statT", (NV, H, BC))   # [H,BC] = (static@fus_s.T+b).T
    staticrep = din("staticrep", (NV, S, 8 * BC))
    cur0T = din("cur0T", (H, BCORE))       # 0.5 * initial h (wh/2 convention)
    ubaseT = din("ubaseT", (NV, BC, U))    # (W1s@static).T per stream
    a1b1 = din("a1b1", (U, 1))
    a2b1 = din("a2b1", (U, 1))
    halfbr = din("halfbr", (H, 1))
    halfbz = din("halfbz", (H, 1))
    biasn = din("biasn", (H, 1))
    iotamb10 = din("iotamb10", (128, A))
    iotamb80 = din("iotamb80", (128, 8 * A))
    iotamb24 = din("iotamb24", (128, 24, NR))
    delta24 = din("delta24", (128, 24, NR))
    iotaJ24 = din("iotaJ24", (128, 24, NR, BC))
    diag01 = din("diag01", (128, BC))
    dup32 = din("dup32", (BC, 128))
    eye32 = din("eye32", (BC, BC))
    ident = din("ident", (128, 128))

    out_d = nc.dram_tensor("out", [H, Tn, BCORE], f32, kind="ExternalOutput").ap()

    NBLK = Tn // 8
    assert Tn % 16 == 0

    with ExitStack() as ctx:
        tc = ctx.enter_context(tile.TileContext(nc))
        singles = ctx.enter_context(tc.tile_pool(name="singles", bufs=1))
        work = ctx.enter_context(tc.tile_pool(name="work", bufs=3))
        dpre = ctx.enter_context(tc.tile_pool(name="dpre", bufs=3))
        pstate = ctx.enter_context(tc.tile_pool(name="pstate", bufs=1, space="PSUM"))
        pwork = ctx.enter_context(tc.tile_pool(name="pwork", bufs=2, space="PSUM"))
        pA = ctx.enter_context(tc.tile_pool(name="pA", bufs=1, space="PSUM"))
        outsb = ctx.enter_context(tc.tile_pool(name="outsb", bufs=2))

        # DMA dispatches serialize ~650ns apiece on the SP sequencer, so
        # dispatch order IS the critical path at startup: x quarter 0 and the
        # phase-A/t0-critical weights go first, bulk constants later.
        xsb = singles.tile([128, 2, Tn // 8, NV, 8 * BC], f32, tag="xsb")
        NQB = 16

        def dma_xq(qb):
            qs = slice(qb * (Tn // 8) // NQB, (qb + 1) * (Tn // 8) // NQB)
            for c in range(2):
                nc.sync.dma_start(out=xsb[:, c, qs], in_=xT[c][:, qs])

        sb = {}

        def load_w(name):
            ap = d_in[name]
            if name in ("wihT", "a2w1xT"):
                t = singles.tile([128, 2, ap.shape[2]], f32, tag=f"w_{name}",
                                 name=f"w_{name}")
                for c in range(2):
                    nc.sync.dma_start(out=t[:, c, :], in_=ap[c])
            elif name in ("fus_statT", "staticrep", "ubaseT"):
                t = singles.tile([ap.shape[1], NV, ap.shape[2]], f32,
                                 tag=f"w_{name}", name=f"w_{name}")
                for v in range(NV):
                    nc.sync.dma_start(out=t[:, v, :], in_=ap[v])
            else:
                t = singles.tile(list(ap.shape), f32, tag=f"w_{name}",
                                 name=f"w_{name}")
                nc.sync.dma_start(out=t, in_=ap)
            sb[name] = t

        dma_xq(0)
        early = ["a2w1xT", "staticrep", "a2w1sT", "w2b2", "iotamb80",
                 "iotamb24", "delta24", "diag01", "dup32", "wihT", "whhT",
                 "cur0T", "eye32", "ubaseT", "w1h10T", "ident"]
        for name in early:
            load_w(name)
        dma_xq(1)
        for name in d_in:
            if name != "xT" and name not in sb:
                load_w(name)
        for qb in range(2, NQB):
            dma_xq(qb)

        # ---- per-stream persistent state ----
        # ring2: h_t in natural [H, slot, BC] layout (matmul rhs for uacc)
        ring2 = [singles.tile([H, NS, BC], f32, tag=f"ring2_{v}", name=f"ring2_{v}")
                 for v in range(NV)]
        # av/bv rings: uacc updates contract av and bv separately (av is ready
        # ~1.2us before cur), and retires subtract the SAME split so the
        # window-sum residual cancels exactly
        ringA = [singles.tile([H, NS, BC], f32, tag=f"ringA{v}", name=f"ringA{v}")
                 for v in range(NV)]
        ringB = [singles.tile([H, NS, BC], f32, tag=f"ringB{v}", name=f"ringB{v}")
                 for v in range(NV)]
        # ringT: transposed ring for folds: partition p = g*BC+b, row r,
        # slot s = r*NG + g, free = H
        ringT = [singles.tile([128, NR, H], f32, tag=f"ringT{v}", name=f"ringT{v}")
                 for v in range(NV)]
        cur0 = [singles.tile([H, BC], f32, tag=f"cur0{v}", name=f"cur0{v}")
                for v in range(NV)]
        u1t65 = [singles.tile([U + 1, 4 * BC], f32, tag=f"u1t{v}", name=f"u1t{v}")
                 for v in range(NV)]
        u2t65 = [singles.tile([U + 1, 8 * BC], f32, tag=f"u2t{v}", name=f"u2t{v}")
                 for v in range(NV)]

        # PSUM banks (8). start=True lazy-zeroes the WHOLE 2KB row of the
        # touched partitions (pending-zero); matmul writes consume pending
        # (fresh) else accumulate. So: bankU partitions 0:64 hold ONLY uacc;
        # gates bank has ONE start per step (xgates-c0) and every other
        # per-step matmul region (pl1/pwh/ptr/pout) rides the pending row
        # with start=False + skip_group_check (fresh-overwrite / accumulate).
        #  bankU[v] (x2): uacc [0:U, 0:BC] persistent accumulate
        #  bankS[v] (x2): phase-A only: pu2 [0:U, 0:256] (quarter-pumped),
        #                 pl2 [0:BC, 256:336], pdup [:, 336:344]
        #  gates[v] (x2 bufs x2): pr/pz/pin/phn [0:4BC], pren [4BC:5BC],
        #                 pout [5BC:6BC], pl1 [:, 6BC:6BC+10],
        #                 pwh [7BC:8BC], ptr [0:BC, 8BC:12BC]
        bankU = [pstate.tile([128, 512], f32, tag=f"bankU{v}", name=f"bankU{v}")
                 for v in range(NV)]
        bankS = [pA.tile([128, 512], f32, tag=f"bankS{v}", name=f"bankS{v}")
                 for v in range(NV)]

        dpre_tiles = [{} for _ in range(NV)]
        for v in range(NV):
            nc.vector.memset(ring2[v], 0.0)
            nc.vector.memset(ringA[v], 0.0)
            nc.vector.memset(ringB[v], 0.0)
            nc.vector.memset(ringT[v], 0.0)
            nc.vector.memset(u1t65[v], 1.0)
            nc.vector.memset(u2t65[v], 1.0)
            nc.sync.dma_start(out=cur0[v], in_=cur0T[:, v * BC:(v + 1) * BC])
            # uacc init = W1s@static (ubase): lhsT=[BC,U] rhs=eye32
            nc.tensor.matmul(bankU[v][0:U, 0:BC], sb["ubaseT"][:, v, :],
                             sb["eye32"], start=True, stop=True)

        out_tiles = [{} for _ in range(NV)]

        def bsl(v):
            return slice(v * BC, (v + 1) * BC)

        # ---------- phase A (argmax2 precompute per 8-step block) ----------
        def gen_phaseA(v, blk):
            t0 = blk * 8
            pu2 = bankS[v][0:U, 0:8 * BC]
            for q in range(4):
                qs = slice(q * 64, (q + 1) * 64)
                for c in range(2):
                    nc.tensor.matmul(pu2[:, qs], sb["a2w1xT"][:, c, :],
                                     xsb[:, c, blk, v, qs],
                                     start=(c == 0 and q == 0), stop=False)
                nc.tensor.matmul(pu2[:, qs], sb["a2w1sT"],
                                 sb["staticrep"][:, v, qs],
                                 start=False, stop=(q == 3))
                yield
            nc.scalar.activation(u2t65[v][0:U, :], pu2, Act.Tanh,
                                 bias=sb["a2b1"], scale=1.0)
            yield
            pl2 = bankS[v][0:BC, 256:336].rearrange("p (f a) -> p f a", a=A)
            for j in range(8):
                nc.tensor.matmul(pl2[:, j, :],
                                 u2t65[v][:, j * BC:(j + 1) * BC], sb["w2b2"],
                                 start=(j == 0), stop=(j == 7))
            yield
            rmax2 = work.tile([BC, 8], f32, tag=f"rmax2{v}", name=f"rmax2{v}_{blk}")
            nc.vector.tensor_reduce(out=rmax2, in_=pl2, axis=Axis.X, op=Alu.max)
            yield
            rmax2_b = bass.AP(tensor=rmax2.tensor, offset=rmax2.offset,
                              ap=[rmax2.ap[0], rmax2.ap[1], [0, A]])
            ge2 = work.tile([BC, 8, A], f32, tag=f"ge2{v}", name=f"ge2{v}_{blk}")
            nc.vector.tensor_tensor(out=ge2, in0=pl2, in1=rmax2_b, op=Alu.is_ge)
            yield
            iota_b = sb["iotamb80"][0:BC, :].rearrange("p (f a) -> p f a", a=A)
            mi2 = work.tile([BC, 8, A], f32, tag=f"mi2{v}", name=f"mi2{v}_{blk}")
            nc.gpsimd.tensor_tensor(out=mi2, in0=ge2, in1=iota_b, op=Alu.mult)
            yield
            idx2f = work.tile([BC, 8], f32, tag=f"idx2f{v}", name=f"idx2f{v}_{blk}")
            nc.vector.tensor_reduce(out=idx2f, in_=mi2, axis=Axis.X, op=Alu.min)
            yield
            pdup = bankS[v][:, 336:344]
            nc.tensor.matmul(pdup, sb["dup32"], idx2f, start=True, stop=True)
            yield
            pdup_b = bass.AP(tensor=pdup.tensor, offset=pdup.offset,
                             ap=[pdup.ap[0], pdup.ap[1], [0, NR]])
            tm = t0 % 24
            oh2 = work.tile([128, 8, NR], f32, tag=f"oh2{v}", name=f"oh2{v}_{blk}")
            nc.vector.tensor_tensor(out=oh2, in0=sb["iotamb24"][:, tm:tm + 8, :],
                                    in1=pdup_b, op=Alu.is_equal)
            yield
            pre2 = work.tile([128, 8, NR], f32, tag=f"pre2{v}", name=f"pre2{v}_{blk}")
            nc.vector.tensor_tensor(out=pre2, in0=oh2,
                                    in1=sb["delta24"][:, tm:tm + 8, :], op=Alu.add)
            yield
            # diagpre2[p,j,r,b] = pre2[p,j,r] * diag01[p,b]  (two half-j ops
            # so a pumped op never blocks the DVE chain for more than ~450ns)
            dp = dpre.tile([128, 8, NR, BC], f32, tag=f"dp{v}", name=f"dp{v}_{blk}")
            for jh in range(2):
                js = slice(jh * 4, (jh + 1) * 4)
                pre2_b = bass.AP(tensor=pre2.tensor,
                                 offset=pre2.offset + jh * 4 * NR,
                                 ap=[pre2.ap[0], [NR, 4], [1, NR], [0, BC]])
                diag_b = bass.AP(tensor=sb["diag01"].tensor,
                                 offset=sb["diag01"].offset,
                                 ap=[sb["diag01"].ap[0], [0, 4], [0, NR],
                                     sb["diag01"].ap[1]])
                nc.vector.tensor_tensor(out=dp[:, js], in0=pre2_b, in1=diag_b,
                                        op=Alu.mult)
                yield
            dpre_tiles[v][blk] = dp

        # gates bank layout: pr/pz/pin/phn [0:4BC], thr [4BC:5BC],
        # thz [5BC:6BC], pren [6BC:7BC], thn [7BC:8BC], pout [8BC:9BC]
        def mk_gates(v, t):
            return pwork.tile([128, 512], f32, tag=f"gat{v}", name=f"gat{v}_{t}")

        def gen_gru(v, t, wh_sb, gat):
            # wh_sb holds weighted_h / 2 (whh r,z cols are x2 host-side)
            pr = gat[:, 0:BC]
            pz = gat[:, BC:2 * BC]
            pin = gat[:, 2 * BC:3 * BC]
            phn = gat[:, 3 * BC:4 * BC]
            pren = gat[:, 4 * BC:5 * BC]
            pout = gat[:, 5 * BC:6 * BC]
            thr = work.tile([H, BC], f32, tag=f"thr{v}", name=f"thr{v}_{t}")
            thz = work.tile([H, BC], f32, tag=f"thz{v}", name=f"thz{v}_{t}")
            thn = work.tile([H, BC], f32, tag=f"thn{v}", name=f"thn{v}_{t}")
            # whh parts (need wh_sb; x parts were emitted earlier).
            # pr first: thr is the chain-critical activation.
            nc.tensor.matmul(pr, sb["whhT"][:, 0:H], wh_sb, start=False, stop=False)
            nc.tensor.matmul(phn, sb["whhT"][:, 2 * H:3 * H], wh_sb,
                             start=False, stop=False)
            yield
            nc.tensor.matmul(pz, sb["whhT"][:, H:2 * H], wh_sb, start=False,
                             stop=False, skip_group_check=True)
            yield
            nc.scalar.activation(thr, pr, Act.Tanh, bias=sb["halfbr"], scale=0.5)
            yield
            nc.scalar.activation(thz, pz, Act.Tanh, bias=sb["halfbz"], scale=0.5)
            yield
            # q = (thr+1)*phn_half = sigmoid(r)*phn; pren = q + pin
            # (one DVE block: same-engine deps run back-to-back)
            qsb = work.tile([H, BC], f32, tag=f"qsb{v}", name=f"qsb{v}_{t}")
            nc.vector.scalar_tensor_tensor(out=qsb, in0=thr, scalar=1.0, in1=phn,
                                           op0=Alu.add, op1=Alu.mult)
            yield
            nc.vector.tensor_tensor(out=pren, in0=qsb, in1=pin, op=Alu.add)
            yield
            zc = work.tile([H, BC], f32, tag=f"zc{v}", name=f"zc{v}_{t}")
            nc.scalar.activation(zc, thz, Act.Copy, bias=0.5, scale=-0.5)
            av = ringA[v][:, t % NS, :]
            nc.vector.scalar_tensor_tensor(out=av, in0=thz, scalar=1.0, in1=wh_sb,
                                           op0=Alu.add, op1=Alu.mult)
            yield
            if t + 1 < TMAX:
                # early uacc work for step t+1: +W1h@av_t and the two-part
                # retire of step t-10 (exactly cancels that step's split adds)
                uacc = bankU[v][0:U, 0:BC]
                nc.tensor.matmul(uacc, sb["w1h10T"], av,
                                 start=False, stop=True, skip_group_check=True)
                nc.tensor.matmul(uacc, sb["negw1h10T"],
                                 ringA[v][:, (t - 10) % NS, :],
                                 start=False, stop=False, skip_group_check=True)
                nc.tensor.matmul(uacc, sb["negw1h10T"],
                                 ringB[v][:, (t - 10) % NS, :],
                                 start=False, stop=True, skip_group_check=True)
            yield
            nc.scalar.activation(thn, pren, Act.Tanh, bias=sb["biasn"], scale=1.0)
            yield
            pump(v)
            bv = ringB[v][:, t % NS, :]
            nc.gpsimd.tensor_tensor(out=bv, in0=zc, in1=thn, op=Alu.mult)
            cur = ring2[v][:, t % NS, :]
            nc.gpsimd.tensor_tensor(out=cur, in0=av, in1=bv, op=Alu.add)
            yield
            # next step's chain head immediately behind cur on PE/Act:
            # uacc window update (+c_t, -c_{t-10}) then u1 tanh
            if t + 1 < TMAX:
                uacc = bankU[v][0:U, 0:BC]
                nc.tensor.matmul(uacc, sb["w1h10T"], ringB[v][:, t % NS, :],
                                 start=False, stop=True, skip_group_check=True)
                u1out = u1t65[v][0:U, :].rearrange("p (d b) -> p d b", b=BC)
                uacc_b = bass.AP(tensor=uacc.tensor, offset=uacc.offset,
                                 ap=[uacc.ap[0], [0, 4], uacc.ap[1]])
                nc.scalar.activation(u1out, uacc_b, Act.Tanh,
                                     bias=sb["a1b1"], scale=1.0)
                yield
            # transposed ring write: ptr = cur.T (PE), then Act copy into ringT
            s_me = t % NS
            g_me, r_me = s_me % NG, s_me // NG
            ptr = gat[0:BC, 8 * BC:12 * BC]
            nc.tensor.matmul(ptr, cur, sb["ident"], is_transpose=True,
                             start=False, stop=False, skip_group_check=True)
            yield
            nc.scalar.copy(ringT[v][g_me * BC:(g_me + 1) * BC, r_me, :], ptr)
            yield
            pump(v)
            # fusion output (stop=True closes this step's gates-bank group)
            nc.tensor.matmul(pout, sb["fuswhT"], cur, start=False, stop=True,
                             skip_group_check=True)
            yield
            ob16 = t % 16
            if ob16 == 0:
                out_tiles[v][t // 16] = outsb.tile(
                    [H, 16, BC], f32, tag=f"osb{v}", name=f"osb{v}_{t // 16}")
            ot = out_tiles[v][t // 16]
            nc.vector.tensor_tensor(out=ot[:, ob16, :], in0=pout,
                                    in1=sb["fus_statT"][:, v, :], op=Alu.add)
            yield
            if ob16 == 15 or t == Tn - 1:
                nc.sync.dma_start(
                    out=out_d[:, t - ob16:t + 1, bsl(v)],
                    in_=ot[:, 0:ob16 + 1, :])
                del out_tiles[v][t // 16]

        def emit_xgates(v, t, gat):
            # one PSUM group for pr/pz/pin/phn: single start here (pr@c0),
            # single stop at the last whh matmul (pz) in gen_gru
            xx = xsb[:, :, t // 8, v, (t % 8) * BC:(t % 8) * BC + BC]
            for c in range(2):
                nc.tensor.matmul(gat[:, 0:BC], sb["wihT"][:, c, 0:H], xx[:, c, :],
                                 start=(c == 0), stop=False)
                nc.tensor.matmul(gat[:, BC:2 * BC], sb["wihT"][:, c, H:2 * H],
                                 xx[:, c, :], start=False, stop=False)
                yield
                nc.tensor.matmul(gat[:, 2 * BC:3 * BC], sb["wihT"][:, c, 2 * H:3 * H],
                                 xx[:, c, :], start=False, stop=False)
                yield

        def gen_step(v, t):
            if t % 8 == 0 and (t // 8 + AHEAD - 1) < NBLK:
                while pa_gen[v] is not None:   # should already be drained
                    pump(v)
                pa_gen[v] = gen_phaseA(v, t // 8 + AHEAD - 1)
            gat = mk_gates(v, t)
            tm = t % 24
            # uacc/u1 for this step were emitted in the previous step's tail.
            # x-gate matmuls first: they fill PE while u1 finishes on Act.
            yield from emit_xgates(v, t, gat)
            pl1 = gat[:, 6 * BC:6 * BC + A]
            nc.tensor.matmul(pl1, u1t65[v], sb["w2b1"], start=False, stop=False,
                             skip_group_check=True)
            yield
            # argmax1: rmax -> masked-iota-sum idx -> qm, all DVE, emitted as
            # one block so they run back-to-back (same-engine deps are free)
            rmax = work.tile([128, 1], f32, tag=f"rmax{v}", name=f"rmax{v}_{t}")
            nc.vector.tensor_reduce(out=rmax, in_=pl1, axis=Axis.X, op=Alu.max)
            junk = work.tile([128, A], f32, tag=f"junk{v}", name=f"junk{v}_{t}")
            idxf = work.tile([128, 1], f32, tag=f"idxf{v}", name=f"idxf{v}_{t}")
            nc.vector.scalar_tensor_tensor(out=junk, in0=pl1, scalar=rmax[:, 0:1],
                                           in1=sb["iotamb10"],
                                           op0=Alu.is_ge, op1=Alu.mult,
                                           accum_out=idxf)
            qm = work.tile([128, NR, BC], f32, tag=f"qm{v}", name=f"qm{v}_{t}")
            nc.vector.scalar_tensor_tensor(
                out=qm, in0=sb["iotaJ24"][:, tm, :, :], scalar=idxf[:, 0:1],
                in1=dpre_tiles[v][t // 8][:, t % 8, :, :],
                op0=Alu.is_equal, op1=Alu.add)
            yield
            pump(v)
            pwh = gat[:, 7 * BC:8 * BC]
            for r in range(NR):
                nc.tensor.matmul(pwh, ringT[v][:, r, :], qm[:, r, :],
                                 start=False, stop=False, skip_group_check=True)
            yield
            # whs = pwh * 0.125 = weighted_h / 2
            whs = work.tile([H, BC], f32, tag=f"whs{v}", name=f"whs{v}_{t}")
            nc.vector.tensor_scalar(out=whs, in0=pwh, scalar1=0.125, scalar2=None,
                                    op0=Alu.mult)
            yield
            yield from gen_gru(v, t, whs, gat)

        import os as _os
        TMAX = int(_os.environ.get("K5_TMAX", "0")) or Tn

        pa_gen = [None for _ in range(NV)]

        def pump(v):
            g = pa_gen[v]
            if g is not None:
                try:
                    next(g)
                except StopIteration:
                    pa_gen[v] = None

        def gen_stream(v):
            # stagger stream 1 by 19 zipper slots: measured-best cross-stream
            # phase (keeps B's DVE argmax burst out of A's chain hops)
            if v == 1:
                for _ in range(19):
                    yield
            yield from gen_phaseA(v, 0)
            pa_gen[v] = gen_phaseA(v, 1)   # drained by the step pumps
            gat0 = mk_gates(v, 0)
            yield from emit_xgates(v, 0, gat0)
            yield from gen_gru(v, 0, cur0[v], gat0)
            for t in range(1, TMAX):
                yield from gen_step(v, t)

        gens = [gen_stream(v) for v in range(NV)]
        live = list(gens)
        while live:
            nxt = []
            for g in live:
                try:
                    next(g)
                    nxt.append(g)
                except StopIteration:
                    pass
            live = nxt

    nc.compile()
    _BUILD_CACHE[key] = (nc, "out")
    return _BUILD_CACHE[key]


def _prep_core_inputs(inputs, core, Tn=T):
    f = np.float32
    b0 = core * BCORE
    x = np.ascontiguousarray(inputs["x"][b0:b0 + BCORE, :Tn, :]).astype(f)
    xT = (x.transpose(2, 1, 0).reshape(2, 128, Tn // 8, 8, NV, BC)
          .transpose(0, 1, 2, 4, 3, 5).reshape(2, 128, Tn // 8, NV, 8 * BC))
    xT = np.ascontiguousarray(xT)
    static = inputs["static"][b0:b0 + BCORE].astype(f)
    wih = inputs["gru_wih"].astype(f); whh = inputs["gru_whh"].astype(f)
    a1w1 = inputs["a1_w1"].astype(f); a2w1 = inputs["a2_w1"].astype(f)
    bih = inputs["gru_bih"].astype(f); bhh = inputs["gru_bhh"].astype(f)
    fusw = inputs["fus_w"].astype(f); fusb = inputs["fus_b"].astype(f)

    iotamb24 = np.zeros((128, 24, NR), f)
    delta24 = np.zeros((128, 24, NR), f)
    for p in range(128):
        g = p // BC
        for j in range(24):
            for r in range(NR):
                s = r * NG + g
                a = (s - j + 10) % NS
                if a < A:
                    iotamb24[p, j, r] = a - BIG
            s_new = (j - 1) % NS
            if s_new % NG == g:
                delta24[p, j, s_new // NG] = 2.0
    # iotaJ24[p,j,r,b] = iotamb24[p,j,r] on the diagonal b==p%BC, +BIG off
    iotaJ24 = np.full((128, 24, NR, BC), BIG, f)
    for p in range(128):
        iotaJ24[p, :, :, p % BC] = iotamb24[p]
    diag01 = np.zeros((128, BC), f)
    for p in range(128):
        diag01[p, p % BC] = 1.0
    dup32 = np.zeros((BC, 128), f)
    for b in range(BC):
        for g in range(NG):
            dup32[b, g * BC + b] = 1.0

    fus_statT = np.stack([
        (static[v * BC:(v + 1) * BC] @ fusw[:, H:].T + fusb).T for v in range(NV)
    ])
    staticrep = np.stack([
        np.tile(static[v * BC:(v + 1) * BC].T, (1, 8)) for v in range(NV)
    ])
    cur0 = static @ inputs["init_w"].astype(f).T + inputs["init_b"].astype(f)
    ubase = (static @ a1w1[:, H:].T).T                        # [U, 64]
    ubaseT = np.stack([ubase[:, v * BC:(v + 1) * BC].T for v in range(NV)])

    w1h10 = (a1w1[:, :H] / 10.0).T
    whhT = whh.T.copy()
    whhT[:, 0:2 * H] *= 2.0       # r,z gates see wh/2
    m = {
        "xT": xT,
        "wihT": np.ascontiguousarray(wih.T.reshape(2, 128, G3)),
        "whhT": np.ascontiguousarray(whhT),
        "w1h10T": np.ascontiguousarray(w1h10),
        "negw1h10T": np.ascontiguousarray(-w1h10),
        "w2b1": np.vstack([inputs["a1_w2"].astype(f).T,
                           inputs["a1_b2"].astype(f).reshape(1, A)]),
        "a2w1xT": np.ascontiguousarray(a2w1[:, :D].T.reshape(2, 128, U)),
        "a2w1sT": np.ascontiguousarray(a2w1[:, D:].T),
        "w2b2": np.vstack([inputs["a2_w2"].astype(f).T,
                           inputs["a2_b2"].astype(f).reshape(1, A)]),
        "fuswhT": np.ascontiguousarray(fusw[:, :H].T),
        "fus_statT": fus_statT,
        "staticrep": staticrep,
        "cur0T": np.ascontiguousarray(cur0.T) * 0.5,
        "ubaseT": ubaseT,
        "a1b1": inputs["a1_b1"].astype(f).reshape(U, 1),
        "a2b1": inputs["a2_b1"].astype(f).reshape(U, 1),
        "halfbr": (0.5 * (bih[:H] + bhh[:H])).reshape(H, 1),
        "halfbz": (0.5 * (bih[H:2 * H] + bhh[H:2 * H])).reshape(H, 1),
        "biasn": (bih[2 * H:] + bhh[2 * H:]).reshape(H, 1),
        "iotamb10": np.tile(np.arange(A, dtype=f) - BIG, (128, 1)),
        "iotamb80": np.tile(np.arange(A, dtype=f) - BIG, (128, 8)),
        "iotamb24": iotamb24,
        "delta24": delta24,
        "iotaJ24": iotaJ24,
        "diag01": diag01,
        "dup32": dup32,
        "eye32": np.eye(BC, dtype=f),
        "ident": np.eye(128, dtype=f),
    }
    return {k: np.ascontiguousarray(v, dtype=f) for k, v in m.items()}


def kernel(**inputs):
    from concourse.bass_utils import run_bass_kernel_spmd
    nc, _ = _build(T)
    in_maps = [_prep_core_inputs(inputs, c) for c in range(NCORES)]
    res = run_bass_kernel_spmd(nc, in_maps, core_ids=list(range(NCORES)))
    out = np.empty((B, T, H), np.float32)
    for c in range(NCORES):
        oc = res.results[c]["out"]
        out[c * BCORE:(c + 1) * BCORE] = oc.transpose(2, 1, 0)
    return out


# revision 10
# speedup vs baseline: 1.3934x; 1.0010x over previous
"""Trainium2 Bass kernel for the AgentLayer GRU-with-action-memory model, v5.

B=512 -> 8 cores x 64; two 32-batch streams per core, op-level zippered.
v5 chain restructure over v4 (v4 wall ~7.9us/step-pair, chain-latency bound):
  - obs-window sum lives in a persistent PSUM accumulator (uacc): per-step
    +W1h@h_new / -W1h@h_old matmuls (h-ring in SBUF, [H,NS,BC] layout);
    removes the DVE usum add/sub from the chain head.
  - argmax1 = Pool reduce-max + ONE Pool TSP with accum_out (masked-iota
    sum == first-max index since max is unique); was 3 DVE hops.
  - single 128-partition transposed ring -> 3 fold matmuls (was 6);
    qm built in ONE DVE TSP vs precomputed diag-masked patterns
    (iotaJunk24 / diagpre2 per block).
  - lambda scaling (0.25/0.5) folded into the whs copy (x0.125) with
    whh r,z columns x2 host-side; whs holds wh/2.
  - GRU tail: cur = z*wh + (1-z)*n via off-chain zc/a from thz ->
    only 2 hops after thn.
  - engine rebalance: PSUM-touching chain ops on Pool (idle in v4, no
    modeled PSUM access penalty), ring copy on Act, fusion static add as
    Pool TT (kills 1 matmul + Act copy per step).
"""

import numpy as np
from contextlib import ExitStack

B, T, D, H, S, A, U = 512, 256, 256, 128, 64, 10, 64
NCORES = 8
BCORE = B // NCORES   # 64 per core
NV = 2                # streams per core
BC = BCORE // NV      # 32 per stream
NG = 128 // BC        # partition groups = 4
NS = 12               # ring slots
NR = NS // NG         # ring rows = 3
G3 = 3 * H
BIG = 1024.0
AHEAD = 2

_BUILD_CACHE = {}


def _build(Tn):
    key = Tn
    if key in _BUILD_CACHE:
        return _BUILD_CACHE[key]

    import concourse.bass as bass
    import concourse.bacc as bacc
    import concourse.tile as tile
    from concourse import mybir

    f32 = mybir.dt.float32
    Alu = mybir.AluOpType
    Act = mybir.ActivationFunctionType
    Axis = mybir.AxisListType

    nc = bacc.Bacc("TRN2", target_bir_lowering=False, debug=False)

    d_in = {}

    def din(name, shape):
        d_in[name] = nc.dram_tensor(name, list(shape), f32, kind="ExternalInput").ap()
        return d_in[name]

    xT = din("xT", (2, 128, Tn // 8, NV, 8 * BC))
    wihT = din("wihT", (2, 128, G3))
    whhT = din("whhT", (H, G3))            # r,z cols x2 host-side
    w1h10T = din("w1h10T", (H, U))
    negw1h10T = din("negw1h10T", (H, U))
    w2b1 = din("w2b1", (U + 1, A))
    a2w1xT = din("a2w1xT", (2, 128, U))
    a2w1sT = din("a2w1sT", (S, U))
    w2b2 = din("w2b2", (U + 1, A))
    fuswhT = din("fuswhT", (H, H))
    fus_statT = din("fus_statT", (NV, H, BC))   # [H,BC] = (static@fus_s.T+b).T
    staticrep = din("staticrep", (NV, S, 8 * BC))
    cur0T = din("cur0T", (H, BCORE))       # 0.5 * initial h (wh/2 convention)
    ubaseT = din("ubaseT", (NV, BC, U))    # (W1s@static).T per stream
    a1b1 = din("a1b1", (U, 1))
    a2b1 = din("a2b1", (U, 1))
    halfbr = din("halfbr", (H, 1))
    halfbz = din("halfbz", (H, 1))
    biasn = din("biasn", (H, 1))
    iotamb10 = din("iotamb10", (128, A))
    iotamb80 = din("iotamb80", (128, 8 * A))
    iotamb24 = din("iotamb24", (128, 24, NR))
    delta24 = din("delta24", (128, 24, NR))
    iotaJ24 = din("iotaJ24", (128, 24, NR, BC))
    diag01 = din("diag01", (128, BC))
    dup32 = din("dup32", (BC, 128))
    eye32 = din("eye32", (BC, BC))
    ident = din("ident", (128, 128))

    out_d = nc.dram_tensor("out", [H, Tn, BCORE], f32, kind="ExternalOutput").ap()

    NBLK = Tn // 8
    assert Tn % 16 == 0

    with ExitStack() as ctx:
        tc = ctx.enter_context(tile.TileContext(nc))
        singles = ctx.enter_context(tc.tile_pool(name="singles", bufs=1))
        work = ctx.enter_context(tc.tile_pool(name="work", bufs=3))
        dpre = ctx.enter_context(tc.tile_pool(name="dpre", bufs=3))
        pstate = ctx.enter_context(tc.tile_pool(name="pstate", bufs=1, space="PSUM"))
        pwork = ctx.enter_context(tc.tile_pool(name="pwork", bufs=2, space="PSUM"))
        pA = ctx.enter_context(tc.tile_pool(name="pA", bufs=1, space="PSUM"))
        outsb = ctx.enter_context(tc.tile_pool(name="outsb", bufs=2))

        # DMA dispatches serialize ~650ns apiece on the SP sequencer, so
        # dispatch order IS the critical path at startup: x quarter 0 and the
        # phase-A/t0-critical weights go first, bulk constants later.
        xsb = singles.tile([128, 2, Tn // 8, NV, 8 * BC], f32, tag="xsb")
        NQB = 16

        def dma_xq(qb):
            qs = slice(qb * (Tn // 8) // NQB, (qb + 1) * (Tn // 8) // NQB)
            for c in range(2):
                nc.sync.dma_start(out=xsb[:, c, qs], in_=xT[c][:, qs])

        sb = {}

        def load_w(name):
            ap = d_in[name]
            if name in ("wihT", "a2w1xT"):
                t = singles.tile([128, 2, ap.shape[2]], f32, tag=f"w_{name}",
                                 name=f"w_{name}")
                for c in range(2):
                    nc.sync.dma_start(out=t[:, c, :], in_=ap[c])
            elif name in ("fus_statT", "staticrep", "ubaseT"):
                t = singles.tile([ap.shape[1], NV, ap.shape[2]], f32,
                                 tag=f"w_{name}", name=f"w_{name}")
                for v in range(NV):
                    nc.sync.dma_start(out=t[:, v, :], in_=ap[v])
            else:
                t = singles.tile(list(ap.shape), f32, tag=f"w_{name}",
                                 name=f"w_{name}")
                nc.sync.dma_start(out=t, in_=ap)
            sb[name] = t

        dma_xq(0)
        early = ["a2w1xT", "staticrep", "a2w1sT", "w2b2", "iotamb80",
                 "iotamb24", "delta24", "diag01", "dup32", "wihT", "whhT",
                 "cur0T", "eye32", "ubaseT", "w1h10T", "ident"]
        for name in early:
            load_w(name)
        dma_xq(1)
        for name in d_in:
            if name != "xT" and name not in sb:
                load_w(name)
        for qb in range(2, NQB):
            dma_xq(qb)

        # ---- per-stream persistent state ----
        # ring2: h_t in natural [H, slot, BC] layout (matmul rhs for uacc)
        ring2 = [singles.tile([H, NS, BC], f32, tag=f"ring2_{v}", name=f"ring2_{v}")
                 for v in range(NV)]
        # av/bv rings: uacc updates contract av and bv separately (av is ready
        # ~1.2us before cur), and retires subtract the SAME split so the
        # window-sum residual cancels exactly
        ringA = [singles.tile([H, NS, BC], f32, tag=f"ringA{v}", name=f"ringA{v}")
                 for v in range(NV)]
        ringB = [singles.tile([H, NS, BC], f32, tag=f"ringB{v}", name=f"ringB{v}")
                 for v in range(NV)]
        # ringT: transposed ring for folds: partition p = g*BC+b, row r,
        # slot s = r*NG + g, free = H
        ringT = [singles.tile([128, NR, H], f32, tag=f"ringT{v}", name=f"ringT{v}")
                 for v in range(NV)]
        cur0 = [singles.tile([H, BC], f32, tag=f"cur0{v}", name=f"cur0{v}")
                for v in range(NV)]
        u1t65 = [singles.tile([U + 1, 4 * BC], f32, tag=f"u1t{v}", name=f"u1t{v}")
                 for v in range(NV)]
        u2t65 = [singles.tile([U + 1, 8 * BC], f32, tag=f"u2t{v}", name=f"u2t{v}")
                 for v in range(NV)]

        # PSUM banks (8). start=True lazy-zeroes the WHOLE 2KB row of the
        # touched partitions (pending-zero); matmul writes consume pending
        # (fresh) else accumulate. So: bankU partitions 0:64 hold ONLY uacc;
        # gates bank has ONE start per step (xgates-c0) and every other
        # per-step matmul region (pl1/pwh/ptr/pout) rides the pending row
        # with start=False + skip_group_check (fresh-overwrite / accumulate).
        #  bankU[v] (x2): uacc [0:U, 0:BC] persistent accumulate
        #  bankS[v] (x2): phase-A only: pu2 [0:U, 0:256] (quarter-pumped),
        #                 pl2 [0:BC, 256:336], pdup [:, 336:344]
        #  gates[v] (x2 bufs x2): pr/pz/pin/phn [0:4BC], pren [4BC:5BC],
        #                 pout [5BC:6BC], pl1 [:, 6BC:6BC+10],
        #                 pwh [7BC:8BC], ptr [0:BC, 8BC:12BC]
        bankU = [pstate.tile([128, 512], f32, tag=f"bankU{v}", name=f"bankU{v}")
                 for v in range(NV)]
        bankS = [pA.tile([128, 512], f32, tag=f"bankS{v}", name=f"bankS{v}")
                 for v in range(NV)]

        dpre_tiles = [{} for _ in range(NV)]
        for v in range(NV):
            nc.vector.memset(ring2[v], 0.0)
            nc.vector.memset(ringA[v], 0.0)
            nc.vector.memset(ringB[v], 0.0)
            nc.vector.memset(ringT[v], 0.0)
            nc.vector.memset(u1t65[v], 1.0)
            nc.vector.memset(u2t65[v], 1.0)
            nc.sync.dma_start(out=cur0[v], in_=cur0T[:, v * BC:(v + 1) * BC])
            # uacc init = W1s@static (ubase): lhsT=[BC,U] rhs=eye32
            nc.tensor.matmul(bankU[v][0:U, 0:BC], sb["ubaseT"][:, v, :],
                             sb["eye32"], start=True, stop=True)

        out_tiles = [{} for _ in range(NV)]

        def bsl(v):
            return slice(v * BC, (v + 1) * BC)

        # ---------- phase A (argmax2 precompute per 8-step block) ----------
        def gen_phaseA(v, blk):
            t0 = blk * 8
            pu2 = bankS[v][0:U, 0:8 * BC]
            for q in range(4):
                qs = slice(q * 64, (q + 1) * 64)
                for c in range(2):
                    nc.tensor.matmul(pu2[:, qs], sb["a2w1xT"][:, c, :],
                                     xsb[:, c, blk, v, qs],
                                     start=(c == 0 and q == 0), stop=False)
                nc.tensor.matmul(pu2[:, qs], sb["a2w1sT"],
                                 sb["staticrep"][:, v, qs],
                                 start=False, stop=(q == 3))
                yield
            nc.scalar.activation(u2t65[v][0:U, :], pu2, Act.Tanh,
                                 bias=sb["a2b1"], scale=1.0)
            yield
            pl2 = bankS[v][0:BC, 256:336].rearrange("p (f a) -> p f a", a=A)
            for j in range(8):
                nc.tensor.matmul(pl2[:, j, :],
                                 u2t65[v][:, j * BC:(j + 1) * BC], sb["w2b2"],
                                 start=(j == 0), stop=(j == 7))
            yield
            rmax2 = work.tile([BC, 8], f32, tag=f"rmax2{v}", name=f"rmax2{v}_{blk}")
            nc.vector.tensor_reduce(out=rmax2, in_=pl2, axis=Axis.X, op=Alu.max)
            yield
            rmax2_b = bass.AP(tensor=rmax2.tensor, offset=rmax2.offset,
                              ap=[rmax2.ap[0], rmax2.ap[1], [0, A]])
            ge2 = work.tile([BC, 8, A], f32, tag=f"ge2{v}", name=f"ge2{v}_{blk}")
            nc.vector.tensor_tensor(out=ge2, in0=pl2, in1=rmax2_b, op=Alu.is_ge)
            yield
            iota_b = sb["iotamb80"][0:BC, :].rearrange("p (f a) -> p f a", a=A)
            mi2 = work.tile([BC, 8, A], f32, tag=f"mi2{v}", name=f"mi2{v}_{blk}")
            nc.vector.tensor_tensor(out=mi2, in0=ge2, in1=iota_b, op=Alu.mult)
            yield
            idx2f = work.tile([BC, 8], f32, tag=f"idx2f{v}", name=f"idx2f{v}_{blk}")
            nc.vector.tensor_reduce(out=idx2f, in_=mi2, axis=Axis.X, op=Alu.min)
            yield
            pdup = bankS[v][:, 336:344]
            nc.tensor.matmul(pdup, sb["dup32"], idx2f, start=True, stop=True)
            yield
            pdup_b = bass.AP(tensor=pdup.tensor, offset=pdup.offset,
                             ap=[pdup.ap[0], pdup.ap[1], [0, NR]])
            tm = t0 % 24
            oh2 = work.tile([128, 8, NR], f32, tag=f"oh2{v}", name=f"oh2{v}_{blk}")
            nc.vector.tensor_tensor(out=oh2, in0=sb["iotamb24"][:, tm:tm + 8, :],
                                    in1=pdup_b, op=Alu.is_equal)
            yield
            pre2 = work.tile([128, 8, NR], f32, tag=f"pre2{v}", name=f"pre2{v}_{blk}")
            nc.vector.tensor_tensor(out=pre2, in0=oh2,
                                    in1=sb["delta24"][:, tm:tm + 8, :], op=Alu.add)
            yield
            # diagpre2[p,j,r,b] = pre2[p,j,r] * diag01[p,b]  (two half-j ops
            # so a pumped op never blocks the DVE chain for more than ~450ns)
            dp = dpre.tile([128, 8, NR, BC], f32, tag=f"dp{v}", name=f"dp{v}_{blk}")
            for jh in range(2):
                js = slice(jh * 4, (jh + 1) * 4)
                pre2_b = bass.AP(tensor=pre2.tensor,
                                 offset=pre2.offset + jh * 4 * NR,
                                 ap=[pre2.ap[0], [NR, 4], [1, NR], [0, BC]])
                diag_b = bass.AP(tensor=sb["diag01"].tensor,
                                 offset=sb["diag01"].offset,
                                 ap=[sb["diag01"].ap[0], [0, 4], [0, NR],
                                     sb["diag01"].ap[1]])
                nc.vector.tensor_tensor(out=dp[:, js], in0=pre2_b, in1=diag_b,
                                        op=Alu.mult)
                yield
            dpre_tiles[v][blk] = dp

        # gates bank layout: pr/pz/pin/phn [0:4BC], thr [4BC:5BC],
        # thz [5BC:6BC], pren [6BC:7BC], thn [7BC:8BC], pout [8BC:9BC]
        def mk_gates(v, t):
            return pwork.tile([128, 512], f32, tag=f"gat{v}", name=f"gat{v}_{t}")

        def gen_gru(v, t, wh_sb, gat):
            # wh_sb holds weighted_h / 2 (whh r,z cols are x2 host-side)
            pr = gat[:, 0:BC]
            pz = gat[:, BC:2 * BC]
            pin = gat[:, 2 * BC:3 * BC]
            phn = gat[:, 3 * BC:4 * BC]
            pren = gat[:, 4 * BC:5 * BC]
            pout = gat[:, 5 * BC:6 * BC]
            thr = work.tile([H, BC], f32, tag=f"thr{v}", name=f"thr{v}_{t}")
            thz = work.tile([H, BC], f32, tag=f"thz{v}", name=f"thz{v}_{t}")
            thn = work.tile([H, BC], f32, tag=f"thn{v}", name=f"thn{v}_{t}")
            # whh parts (need wh_sb; x parts were emitted earlier).
            # pr first: thr is the chain-critical activation.
            nc.tensor.matmul(pr, sb["whhT"][:, 0:H], wh_sb, start=False, stop=False)
            nc.tensor.matmul(phn, sb["whhT"][:, 2 * H:3 * H], wh_sb,
                             start=False, stop=False)
            yield
            nc.tensor.matmul(pz, sb["whhT"][:, H:2 * H], wh_sb, start=False,
                             stop=False, skip_group_check=True)
            yield
            nc.scalar.activation(thr, pr, Act.Tanh, bias=sb["halfbr"], scale=0.5)
            yield
            nc.scalar.activation(thz, pz, Act.Tanh, bias=sb["halfbz"], scale=0.5)
            yield
            # q = (thr+1)*phn_half = sigmoid(r)*phn; pren = q + pin
            # (one DVE block: same-engine deps run back-to-back)
            qsb = work.tile([H, BC], f32, tag=f"qsb{v}", name=f"qsb{v}_{t}")
            nc.vector.scalar_tensor_tensor(out=qsb, in0=thr, scalar=1.0, in1=phn,
                                           op0=Alu.add, op1=Alu.mult)
            yield
            nc.vector.tensor_tensor(out=pren, in0=qsb, in1=pin, op=Alu.add)
            yield
            zc = work.tile([H, BC], f32, tag=f"zc{v}", name=f"zc{v}_{t}")
            nc.scalar.activation(zc, thz, Act.Copy, bias=0.5, scale=-0.5)
            av = ringA[v][:, t % NS, :]
            nc.vector.scalar_tensor_tensor(out=av, in0=thz, scalar=1.0, in1=wh_sb,
                                           op0=Alu.add, op1=Alu.mult)
            yield
            if t + 1 < TMAX:
                # early uacc work for step t+1: +W1h@av_t and the two-part
                # retire of step t-10 (exactly cancels that step's split adds)
                uacc = bankU[v][0:U, 0:BC]
                nc.tensor.matmul(uacc, sb["w1h10T"], av,
                                 start=False, stop=True, skip_group_check=True)
                nc.tensor.matmul(uacc, sb["negw1h10T"],
                                 ringA[v][:, (t - 10) % NS, :],
                                 start=False, stop=False, skip_group_check=True)
                nc.tensor.matmul(uacc, sb["negw1h10T"],
                                 ringB[v][:, (t - 10) % NS, :],
                                 start=False, stop=True, skip_group_check=True)
            yield
            nc.scalar.activation(thn, pren, Act.Tanh, bias=sb["biasn"], scale=1.0)
            yield
            pump(v)
            bv = ringB[v][:, t % NS, :]
            nc.gpsimd.tensor_tensor(out=bv, in0=zc, in1=thn, op=Alu.mult)
            cur = ring2[v][:, t % NS, :]
            nc.gpsimd.tensor_tensor(out=cur, in0=av, in1=bv, op=Alu.add)
            yield
            # next step's chain head immediately behind cur on PE/Act:
            # uacc window update (+c_t, -c_{t-10}) then u1 tanh
            if t + 1 < TMAX:
                uacc = bankU[v][0:U, 0:BC]
                nc.tensor.matmul(uacc, sb["w1h10T"], ringB[v][:, t % NS, :],
                                 start=False, stop=True, skip_group_check=True)
                u1out = u1t65[v][0:U, :].rearrange("p (d b) -> p d b", b=BC)
                uacc_b = bass.AP(tensor=uacc.tensor, offset=uacc.offset,
                                 ap=[uacc.ap[0], [0, 4], uacc.ap[1]])
                nc.scalar.activation(u1out, uacc_b, Act.Tanh,
                                     bias=sb["a1b1"], scale=1.0)
                yield
            # transposed ring write: ptr = cur.T (PE), then Act copy into ringT
            s_me = t % NS
            g_me, r_me = s_me % NG, s_me // NG
            ptr = gat[0:BC, 8 * BC:12 * BC]
            nc.tensor.matmul(ptr, cur, sb["ident"], is_transpose=True,
                             start=False, stop=False, skip_group_check=True)
            yield
            nc.scalar.copy(ringT[v][g_me * BC:(g_me + 1) * BC, r_me, :], ptr)
            yield
            pump(v)
            # fusion output (stop=True closes this step's gates-bank group)
            nc.tensor.matmul(pout, sb["fuswhT"], cur, start=False, stop=True,
                             skip_group_check=True)
            yield
            ob16 = t % 16
            if ob16 == 0:
                out_tiles[v][t // 16] = outsb.tile(
                    [H, 16, BC], f32, tag=f"osb{v}", name=f"osb{v}_{t // 16}")
            ot = out_tiles[v][t // 16]
            nc.vector.tensor_tensor(out=ot[:, ob16, :], in0=pout,
                                    in1=sb["fus_statT"][:, v, :], op=Alu.add)
            yield
            if ob16 == 15 or t == Tn - 1:
                nc.sync.dma_start(
                    out=out_d[:, t - ob16:t + 1, bsl(v)],
                    in_=ot[:, 0:ob16 + 1, :])
                del out_tiles[v][t // 16]

        def emit_xgates(v, t, gat):
            # one PSUM group for pr/pz/pin/phn: single start here (pr@c0),
            # single stop at the last whh matmul (pz) in gen_gru
            xx = xsb[:, :, t // 8, v, (t % 8) * BC:(t % 8) * BC + BC]
            for c in range(2):
                nc.tensor.matmul(gat[:, 0:BC], sb["wihT"][:, c, 0:H], xx[:, c, :],
                                 start=(c == 0), stop=False)
                nc.tensor.matmul(gat[:, BC:2 * BC], sb["wihT"][:, c, H:2 * H],
                                 xx[:, c, :], start=False, stop=False)
                yield
                nc.tensor.matmul(gat[:, 2 * BC:3 * BC], sb["wihT"][:, c, 2 * H:3 * H],
                                 xx[:, c, :], start=False, stop=False)
                yield

        def gen_step(v, t):
            if t % 8 == 0 and (t // 8 + AHEAD - 1) < NBLK:
                while pa_gen[v] is not None:   # should already be drained
                    pump(v)
                pa_gen[v] = gen_phaseA(v, t // 8 + AHEAD - 1)
            gat = mk_gates(v, t)
            tm = t % 24
            # uacc/u1 for this step were emitted in the previous step's tail.
            # x-gate matmuls first: they fill PE while u1 finishes on Act.
            yield from emit_xgates(v, t, gat)
            pl1 = gat[:, 6 * BC:6 * BC + A]
            nc.tensor.matmul(pl1, u1t65[v], sb["w2b1"], start=False, stop=False,
                             skip_group_check=True)
            yield
            # argmax1: rmax -> masked-iota-sum idx -> qm, all DVE, emitted as
            # one block so they run back-to-back (same-engine deps are free)
            rmax = work.tile([128, 1], f32, tag=f"rmax{v}", name=f"rmax{v}_{t}")
            nc.vector.tensor_reduce(out=rmax, in_=pl1, axis=Axis.X, op=Alu.max)
            junk = work.tile([128, A], f32, tag=f"junk{v}", name=f"junk{v}_{t}")
            idxf = work.tile([128, 1], f32, tag=f"idxf{v}", name=f"idxf{v}_{t}")
            nc.vector.scalar_tensor_tensor(out=junk, in0=pl1, scalar=rmax[:, 0:1],
                                           in1=sb["iotamb10"],
                                           op0=Alu.is_ge, op1=Alu.mult,
                                           accum_out=idxf)
            qm = work.tile([128, NR, BC], f32, tag=f"qm{v}", name=f"qm{v}_{t}")
            nc.vector.scalar_tensor_tensor(
                out=qm, in0=sb["iotaJ24"][:, tm, :, :], scalar=idxf[:, 0:1],
                in1=dpre_tiles[v][t // 8][:, t % 8, :, :],
                op0=Alu.is_equal, op1=Alu.add)
            yield
            pump(v)
            pwh = gat[:, 7 * BC:8 * BC]
            for r in range(NR):
                nc.tensor.matmul(pwh, ringT[v][:, r, :], qm[:, r, :],
                                 start=False, stop=False, skip_group_check=True)
            yield
            # whs = pwh * 0.125 = weighted_h / 2
            whs = work.tile([H, BC], f32, tag=f"whs{v}", name=f"whs{v}_{t}")
            nc.vector.tensor_scalar(out=whs, in0=pwh, scalar1=0.125, scalar2=None,
                                    op0=Alu.mult)
            yield
            yield from gen_gru(v, t, whs, gat)

        import os as _os
        TMAX = int(_os.environ.get("K5_TMAX", "0")) or Tn

        pa_gen = [None for _ in range(NV)]

        def pump(v):
            g = pa_gen[v]
            if g is not None:
                try:
                    next(g)
                except StopIteration:
                    pa_gen[v] = None

        def gen_stream(v):
            # stagger stream 1 by 19 zipper slots: measured-best cross-stream
            # phase (keeps B's DVE argmax burst out of A's chain hops)
            if v == 1:
                for _ in range(19):
                    yield
            yield from gen_phaseA(v, 0)
            pa_gen[v] = gen_phaseA(v, 1)   # drained by the step pumps
            gat0 = mk_gates(v, 0)
            yield from emit_xgates(v, 0, gat0)
            yield from gen_gru(v, 0, cur0[v], gat0)
            for t in range(1, TMAX):
                yield from gen_step(v, t)

        gens = [gen_stream(v) for v in range(NV)]
        live = list(gens)
        while live:
            nxt = []
            for g in live:
                try:
                    next(g)
                    nxt.append(g)
                except StopIteration:
                    pass
            live = nxt

    nc.compile()
    _BUILD_CACHE[key] = (nc, "out")
    return _BUILD_CACHE[key]


def _prep_core_inputs(inputs, core, Tn=T):
    f = np.float32
    b0 = core * BCORE
    x = np.ascontiguousarray(inputs["x"][b0:b0 + BCORE, :Tn, :]).astype(f)
    xT = (x.transpose(2, 1, 0).reshape(2, 128, Tn // 8, 8, NV, BC)
          .transpose(0, 1, 2, 4, 3, 5).reshape(2, 128, Tn // 8, NV, 8 * BC))
    xT = np.ascontiguousarray(xT)
    static = inputs["static"][b0:b0 + BCORE].astype(f)
    wih = inputs["gru_wih"].astype(f); whh = inputs["gru_whh"].astype(f)
    a1w1 = inputs["a1_w1"].astype(f); a2w1 = inputs["a2_w1"].astype(f)
    bih = inputs["gru_bih"].astype(f); bhh = inputs["gru_bhh"].astype(f)
    fusw = inputs["fus_w"].astype(f); fusb = inputs["fus_b"].astype(f)

    iotamb24 = np.zeros((128, 24, NR), f)
    delta24 = np.zeros((128, 24, NR), f)
    for p in range(128):
        g = p // BC
        for j in range(24):
            for r in range(NR):
                s = r * NG + g
                a = (s - j + 10) % NS
                if a < A:
                    iotamb24[p, j, r] = a - BIG
            s_new = (j - 1) % NS
            if s_new % NG == g:
                delta24[p, j, s_new // NG] = 2.0
    # iotaJ24[p,j,r,b] = iotamb24[p,j,r] on the diagonal b==p%BC, +BIG off
    iotaJ24 = np.full((128, 24, NR, BC), BIG, f)
    for p in range(128):
        iotaJ24[p, :, :, p % BC] = iotamb24[p]
    diag01 = np.zeros((128, BC), f)
    for p in range(128):
        diag01[p, p % BC] = 1.0
    dup32 = np.zeros((BC, 128), f)
    for b in range(BC):
        for g in range(NG):
            dup32[b, g * BC + b] = 1.0

    fus_statT = np.stack([
        (static[v * BC:(v + 1) * BC] @ fusw[:, H:].T + fusb).T for v in range(NV)
    ])
    staticrep = np.stack([
        np.tile(static[v * BC:(v + 1) * BC].T, (1, 8)) for v in range(NV)
    ])
    cur0 = static @ inputs["init_w"].astype(f).T + inputs["init_b"].astype(f)
    ubase = (static @ a1w1[:, H:].T).T                        # [U, 64]
    ubaseT = np.stack([ubase[:, v * BC:(v + 1) * BC].T for v in range(NV)])

    w1h10 = (a1w1[:, :H] / 10.0).T
    whhT = whh.T.copy()
    whhT[:, 0:2 * H] *= 2.0       # r,z gates see wh/2
    m = {
        "xT": xT,
        "wihT": np.ascontiguousarray(wih.T.reshape(2, 128, G3)),
        "whhT": np.ascontiguousarray(whhT),
        "w1h10T": np.ascontiguousarray(w1h10),
        "negw1h10T": np.ascontiguousarray(-w1h10),
        "w2b1": np.vstack([inputs["a1_w2"].astype(f).T,
                           inputs["a1_b2"].astype(f).reshape(1, A)]),
        "a2w1xT": np.ascontiguousarray(a2w1[:, :D].T.reshape(2, 128, U)),
        "a2w1sT": np.ascontiguousarray(a2w1[:, D:].T),
        "w2b2": np.vstack([inputs["a2_w2"].astype(f).T,
                           inputs["a2_b2"].astype(f).reshape(1, A)]),
        "fuswhT": np.ascontiguousarray(fusw[:, :H].T),
        "fus_statT": fus_statT,
        "staticrep": staticrep,
        "cur0T": np.ascontiguousarray(cur0.T) * 0.5,
        "ubaseT": ubaseT,
        "a1b1": inputs["a1_b1"].astype(f).reshape(U, 1),
        "a2b1": inputs["a2_b1"].astype(f).reshape(U, 1),
        "halfbr": (0.5 * (bih[:H] + bhh[:H])).reshape(H, 1),
        "halfbz": (0.5 * (bih[H:2 * H] + bhh[H:2 * H])).reshape(H, 1),
        "biasn": (bih[2 * H:] + bhh[2 * H:]).reshape(H, 1),
        "iotamb10": np.tile(np.arange(A, dtype=f) - BIG, (128, 1)),
        "iotamb80": np.tile(np.arange(A, dtype=f) - BIG, (128, 8)),
        "iotamb24": iotamb24,
        "delta24": delta24,
        "iotaJ24": iotaJ24,
        "diag01": diag01,
        "dup32": dup32,
        "eye32": np.eye(BC, dtype=f),
        "ident": np.eye(128, dtype=f),
    }
    return {k: np.ascontiguousarray(v, dtype=f) for k, v in m.items()}


def kernel(**inputs):
    from concourse.bass_utils import run_bass_kernel_spmd
    nc, _ = _build(T)
    in_maps = [_prep_core_inputs(inputs, c) for c in range(NCORES)]
    res = run_bass_kernel_spmd(nc, in_maps, core_ids=list(range(NCORES)))
    out = np.empty((B, T, H), np.float32)
    for c in range(NCORES):
        oc = res.results[c]["out"]
        out[c * BCORE:(c + 1) * BCORE] = oc.transpose(2, 1, 0)
    return out
